# revision 2
# baseline (speedup 1.0000x reference)
import numpy as np
import ml_dtypes

import concourse.bacc as bacc
import concourse.bass as bass
import concourse.tile as tile
from concourse import mybir
from concourse.bass_utils import run_bass_kernel_spmd

F32 = mybir.dt.float32
BF16 = mybir.dt.bfloat16
AF = mybir.ActivationFunctionType

NCORES = 8
BPC = 4
C = 512
T = 1024
NCH = 4
NTK = 8
GPC = 8
EPS = 1e-5
SM_SCALE = float(C) ** -0.5


def _emit(nc, tc, ctx, flags):
    trivial_gn, qk_bias, o_bias = flags

    x_d = nc.dram_tensor("x", (BPC, C, T), F32, kind="ExternalInput")
    y_d = nc.dram_tensor("y", (BPC, C, T), F32, kind="ExternalOutput")
    w_d = {
        n: nc.dram_tensor(n, (C, C), BF16, kind="ExternalInput")
        for n in ("wqT", "wkT", "wvT", "wpT")
    }
    ones_d = nc.dram_tensor("ones128", (128, 128), BF16, kind="ExternalInput")
    maskg_d = nc.dram_tensor("mask_g", (128, GPC), F32, kind="ExternalInput")
    bcp_d = nc.dram_tensor("bc_pos", (GPC, 128), F32, kind="ExternalInput")
    bcn_d = nc.dram_tensor("bc_neg", (GPC, 128), F32, kind="ExternalInput")
    if not trivial_gn:
        gnw_d = nc.dram_tensor("gnw", (C,), F32, kind="ExternalInput")
        gnb_d = nc.dram_tensor("gnb", (C,), F32, kind="ExternalInput")
    if qk_bias:
        bq_d = nc.dram_tensor("bq", (C,), F32, kind="ExternalInput")
        bk_d = nc.dram_tensor("bk", (C,), F32, kind="ExternalInput")
    if o_bias:
        bo_d = nc.dram_tensor("bias_o", (C,), F32, kind="ExternalInput")

    x_ap = x_d.ap().rearrange("b (c p) t -> b p c t", p=128)
    y_ap = y_d.ap().rearrange("b (c p) t -> b p c t", p=128)

    singles = ctx.enter_context(tc.tile_pool(name="singles", bufs=1))
    x_pool = ctx.enter_context(tc.tile_pool(name="x", bufs=2))
    h_pool = ctx.enter_context(tc.tile_pool(name="h", bufs=2))
    q_pool = ctx.enter_context(tc.tile_pool(name="q", bufs=2))
    k_pool = ctx.enter_context(tc.tile_pool(name="k", bufs=2))
    v_pool = ctx.enter_context(tc.tile_pool(name="v", bufs=2))
    e_pool = ctx.enter_context(tc.tile_pool(name="e", bufs=2))
    on_pool = ctx.enter_context(tc.tile_pool(name="on", bufs=2))
    out_pool = ctx.enter_context(tc.tile_pool(name="out", bufs=2))
    scr_pool = ctx.enter_context(tc.tile_pool(name="scr", bufs=2))
    st_pool = ctx.enter_context(tc.tile_pool(name="st", bufs=2))
    ps_work = ctx.enter_context(tc.tile_pool(name="ps_work", bufs=3, space="PSUM"))
    ps_acc = ctx.enter_context(tc.tile_pool(name="ps_acc", bufs=5, space="PSUM"))

    wsb = {}
    for n in ("wqT", "wkT", "wvT", "wpT"):
        tiles = []
        for k in range(NCH):
            wt = singles.tile([128, C], BF16, tag=f"{n}{k}")
            nc.sync.dma_start(out=wt[:], in_=w_d[n].ap()[k * 128:(k + 1) * 128, :])
            tiles.append(wt)
        wsb[n] = tiles
    ones_sb = singles.tile([128, 128], BF16, tag="ones")
    nc.sync.dma_start(out=ones_sb[:], in_=ones_d.ap())
    maskg_sb = singles.tile([128, GPC], F32, tag="maskg")
    nc.sync.dma_start(out=maskg_sb[:], in_=maskg_d.ap())
    bcp_sb = singles.tile([GPC, 128], F32, tag="bcp")
    nc.sync.dma_start(out=bcp_sb[:], in_=bcp_d.ap())
    bcn_sb = singles.tile([GPC, 128], F32, tag="bcn")
    nc.sync.dma_start(out=bcn_sb[:], in_=bcn_d.ap())
    eps_sb = singles.tile([GPC, 1], F32, tag="eps")
    nc.vector.memset(eps_sb[:], EPS)
    if not trivial_gn:
        gnw_sb = singles.tile([128, NCH], F32, tag="gnw")
        gnb_sb = singles.tile([128, NCH], F32, tag="gnb")
        gw = gnw_d.ap().rearrange("(c p) -> p c", p=128)
        gb = gnb_d.ap().rearrange("(c p) -> p c", p=128)
        nc.sync.dma_start(out=gnw_sb[:], in_=gw)
        nc.sync.dma_start(out=gnb_sb[:], in_=gb)
    if qk_bias:
        bq_sb = singles.tile([128, NCH], F32, tag="bq")
        bk_sb = singles.tile([128, NCH], F32, tag="bk")
        nc.sync.dma_start(out=bq_sb[:], in_=bq_d.ap().rearrange("(c p) -> p c", p=128))
        nc.sync.dma_start(out=bk_sb[:], in_=bk_d.ap().rearrange("(c p) -> p c", p=128))
    if o_bias:
        bo_sb = singles.tile([128, NCH], F32, tag="bo")
        nc.sync.dma_start(out=bo_sb[:], in_=bo_d.ap().rearrange("(c p) -> p c", p=128))

    for b in range(BPC):
        xt = x_pool.tile([128, NCH, T], F32, tag="x")
        nc.sync.dma_start(out=xt[:], in_=x_ap[b])

        mv6 = st_pool.tile([128, NCH, 2, 6], F32, tag="mv6")
        mv = st_pool.tile([128, NCH, 2], F32, tag="mv")
        for ci in range(NCH):
            for s in range(2):
                nc.vector.bn_stats(
                    out=mv6[:, ci, s, :], in_=xt[:, ci, s * 512:(s + 1) * 512]
                )
            nc.vector.bn_aggr(out=mv[:, ci, :], in_=mv6[:, ci, :, :])
        msq = st_pool.tile([128, NCH, 1], F32, tag="msq")
        m2 = st_pool.tile([128, NCH, 1], F32, tag="m2")
        nc.vector.tensor_mul(msq[:], mv[:, :, 0:1], mv[:, :, 0:1])
        nc.vector.tensor_add(m2[:], mv[:, :, 1:2], msq[:])
        psum_g = ps_work.tile([GPC, 2 * NCH], F32, tag="w")
        for ci in range(NCH):
            nc.tensor.matmul(
                psum_g[:, ci:ci + 1], maskg_sb[:], mv[:, ci, 0:1],
                start=True, stop=True,
            )
            nc.tensor.matmul(
                psum_g[:, NCH + ci:NCH + ci + 1], maskg_sb[:], m2[:, ci, :],
                start=True, stop=True,
            )
        gstat = st_pool.tile([GPC, 2 * NCH], F32, tag="gstat")
        nc.vector.tensor_scalar_mul(gstat[:], psum_g[:], 1.0 / 16.0)
        sqg = st_pool.tile([GPC, NCH], F32, tag="sqg")
        varg = st_pool.tile([GPC, NCH], F32, tag="varg")
        nc.vector.tensor_mul(sqg[:], gstat[:, 0:NCH], gstat[:, 0:NCH])
        nc.vector.tensor_sub(varg[:], gstat[:, NCH:2 * NCH], sqg[:])
        lnv = st_pool.tile([GPC, NCH], F32, tag="lnv")
        nc.scalar.activation(lnv[:], varg[:], AF.Ln, bias=eps_sb[:])
        rstd = st_pool.tile([GPC, NCH], F32, tag="rstd")
        nc.scalar.activation(rstd[:], lnv[:], AF.Exp, scale=-0.5)
        psum_bc = ps_work.tile([128, 2 * NCH], F32, tag="w")
        for ci in range(NCH):
            nc.tensor.matmul(
                psum_bc[:, ci:ci + 1], bcn_sb[:], gstat[:, ci:ci + 1],
                start=True, stop=True,
            )
            nc.tensor.matmul(
                psum_bc[:, NCH + ci:NCH + ci + 1], bcp_sb[:], rstd[:, ci:ci + 1],
                start=True, stop=True,
            )
        bc = st_pool.tile([128, 2 * NCH], F32, tag="bc")
        nc.vector.tensor_copy(bc[:], psum_bc[:])
        if trivial_gn:
            scale_t = bc[:, NCH:2 * NCH]
            shift_t = st_pool.tile([128, NCH], F32, tag="shift")
            nc.vector.tensor_mul(shift_t[:], bc[:, 0:NCH], bc[:, NCH:2 * NCH])
            shift_ap = shift_t
        else:
            scale_full = st_pool.tile([128, NCH], F32, tag="scalef")
            nc.vector.tensor_mul(scale_full[:], bc[:, NCH:2 * NCH], gnw_sb[:])
            tmp = st_pool.tile([128, NCH], F32, tag="tmpf")
            nc.vector.tensor_mul(tmp[:], bc[:, 0:NCH], scale_full[:])
            shift_t = st_pool.tile([128, NCH], F32, tag="shift")
            nc.vector.tensor_add(shift_t[:], tmp[:], gnb_sb[:])
            scale_t = scale_full
            shift_ap = shift_t

        ht = h_pool.tile([128, NCH, T], BF16, tag="h")
        for ci in range(NCH):
            nc.scalar.activation(
                ht[:, ci, :], xt[:, ci, :], AF.Identity,
                bias=shift_ap[:, ci:ci + 1], scale=scale_t[:, ci:ci + 1],
            )

        qt = q_pool.tile([128, NCH, T], BF16, tag="q")
        kt = k_pool.tile([128, NCH, T], BF16, tag="k")
        vt = v_pool.tile([128, NTK, C], BF16, tag="v")
        for m in range(NCH):
            for n2 in range(2):
                ps = ps_work.tile([128, 512], F32, tag="w")
                for k in range(NCH):
                    nc.tensor.matmul(
                        ps[:],
                        wsb["wqT"][k][:, m * 128:(m + 1) * 128],
                        ht[:, k, n2 * 512:(n2 + 1) * 512],
                        start=(k == 0), stop=(k == NCH - 1),
                    )
                if qk_bias:
                    nc.scalar.activation(
                        qt[:, m, n2 * 512:(n2 + 1) * 512], ps[:], AF.Identity,
                        bias=bq_sb[:, m:m + 1],
                    )
                else:
                    nc.scalar.copy(qt[:, m, n2 * 512:(n2 + 1) * 512], ps[:])
                ps = ps_work.tile([128, 512], F32, tag="w")
                for k in range(NCH):
                    nc.tensor.matmul(
                        ps[:],
                        wsb["wkT"][k][:, m * 128:(m + 1) * 128],
                        ht[:, k, n2 * 512:(n2 + 1) * 512],
                        start=(k == 0), stop=(k == NCH - 1),
                    )
                if qk_bias:
                    nc.vector.tensor_scalar_add(
                        kt[:, m, n2 * 512:(n2 + 1) * 512], ps[:], bk_sb[:, m:m + 1]
                    )
                else:
                    nc.vector.tensor_copy(kt[:, m, n2 * 512:(n2 + 1) * 512], ps[:])
        for m in range(NTK):
            ps = ps_work.tile([128, 512], F32, tag="w")
            for k in range(NCH):
                nc.tensor.matmul(
                    ps[:],
                    ht[:, k, m * 128:(m + 1) * 128],
                    wsb["wvT"][k][:],
                    start=(k == 0), stop=(k == NCH - 1),
                )
            nc.scalar.copy(vt[:, m, :], ps[:])

        ont = on_pool.tile([128, NCH, T], BF16, tag="on")
        for n2 in range(2):
            tq = slice(n2 * 512, (n2 + 1) * 512)
            et = e_pool.tile([128, NTK, 512], BF16, tag="e")
            po = [
                ps_acc.tile([128, 512], F32, tag="acc", name=f"po{m}")
                for m in range(NCH)
            ]
            pc = ps_acc.tile([128, 512], F32, tag="acc")
            for tk in range(NTK):
                ps_s = ps_work.tile([128, 512], F32, tag="w")
                for k in range(NCH):
                    nc.tensor.matmul(
                        ps_s[:],
                        kt[:, k, tk * 128:(tk + 1) * 128],
                        qt[:, k, tq],
                        start=(k == 0), stop=(k == NCH - 1),
                    )
                nc.scalar.activation(et[:, tk, :], ps_s[:], AF.Exp, scale=SM_SCALE)
                for m in range(NCH):
                    nc.tensor.matmul(
                        po[m][:],
                        vt[:, tk, m * 128:(m + 1) * 128],
                        et[:, tk, :],
                        start=(tk == 0), stop=(tk == NTK - 1),
                    )
                nc.tensor.matmul(
                    pc[:], ones_sb[:], et[:, tk, :],
                    start=(tk == 0), stop=(tk == NTK - 1),
                )
            lncs = scr_pool.tile([128, 512], F32, tag="lncs")
            nc.scalar.activation(lncs[:], pc[:], AF.Ln)
            rec = scr_pool.tile([128, 512], BF16, tag="rec")
            nc.scalar.activation(rec[:], lncs[:], AF.Exp, scale=-1.0)
            for m in range(NCH):
                nc.vector.tensor_mul(ont[:, m, tq], po[m][:], rec[:])

        out_t = out_pool.tile([128, NCH, T], F32, tag="out")
        for m in range(NCH):
            for n2 in range(2):
                tq = slice(n2 * 512, (n2 + 1) * 512)
                ps = ps_work.tile([128, 512], F32, tag="w")
                for k in range(NCH):
                    nc.tensor.matmul(
                        ps[:],
                        wsb["wpT"][k][:, m * 128:(m + 1) * 128],
                        ont[:, k, tq],
                        start=(k == 0), stop=(k == NCH - 1),
                    )
                if o_bias:
                    nc.vector.scalar_tensor_tensor(
                        out_t[:, m, tq], ps[:], bo_sb[:, m:m + 1], xt[:, m, tq],
                        op0=mybir.AluOpType.add, op1=mybir.AluOpType.add,
                    )
                else:
                    nc.vector.tensor_add(out_t[:, m, tq], ps[:], xt[:, m, tq])
        nc.sync.dma_start(out=y_ap[b], in_=out_t[:])


def _build(flags):
    from contextlib import ExitStack

    nc = bacc.Bacc(
        "TRN2",
        target_bir_lowering=False,
        debug=False,
        enable_asserts=False,
        num_devices=NCORES,
    )
    with tile.TileContext(nc) as tc:
        with ExitStack() as ctx:
            _emit(nc, tc, ctx, flags)
    nc.compile()
    return nc


_CACHE = {}


def _consts():
    p = np.arange(128)
    maskg = (p[:, None] // 16 == np.arange(GPC)[None, :]).astype(np.float32)
    bcp = maskg.T.copy()
    return {
        "ones128": np.ones((128, 128), ml_dtypes.bfloat16),
        "mask_g": maskg,
        "bc_pos": np.ascontiguousarray(bcp),
        "bc_neg": np.ascontiguousarray(-bcp),
    }


def kernel(x, gn_w, gn_b, wq, bq, wk, bk, wv, bv, wp, bp):
    x = np.ascontiguousarray(np.asarray(x, dtype=np.float32))
    B = x.shape[0]
    assert (B, x.shape[1], x.shape[2] * x.shape[3]) == (NCORES * BPC, C, T)
    H, W = x.shape[2], x.shape[3]
    xr = x.reshape(B, C, T)
    gn_w = np.asarray(gn_w, np.float32)
    gn_b = np.asarray(gn_b, np.float32)
    trivial_gn = bool(np.all(gn_w == 1.0) and np.all(gn_b == 0.0))
    qk_bias = bool(np.any(np.asarray(bq)) or np.any(np.asarray(bk)))
    o_bias = bool(np.any(np.asarray(bv)) or np.any(np.asarray(bp)))
    flags = (trivial_gn, qk_bias, o_bias)
    if flags not in _CACHE:
        _CACHE[flags] = _build(flags)
    nc = _CACHE[flags]

    bf = ml_dtypes.bfloat16
    common = dict(_consts())
    for n, w in (("wqT", wq), ("wkT", wk), ("wvT", wv), ("wpT", wp)):
        common[n] = np.ascontiguousarray(np.asarray(w, np.float32).T).astype(bf)
    if not trivial_gn:
        common["gnw"] = gn_w
        common["gnb"] = gn_b
    if qk_bias:
        common["bq"] = np.asarray(bq, np.float32)
        common["bk"] = np.asarray(bk, np.float32)
    if o_bias:
        common["bias_o"] = (
            np.asarray(wp, np.float32) @ np.asarray(bv, np.float32)
            + np.asarray(bp, np.float32)
        ).astype(np.float32)

    in_maps = [
        {"x": np.ascontiguousarray(xr[c * BPC:(c + 1) * BPC]), **common}
        for c in range(NCORES)
    ]
    res = run_bass_kernel_spmd(nc, in_maps, core_ids=list(range(NCORES)))
    y = np.concatenate([res.results[c]["y"] for c in range(NCORES)], axis=0)
    return np.ascontiguousarray(y.reshape(B, C, H, W).astype(np.float32))


# revision 11
# speedup vs baseline: 1.5182x; 1.5182x over previous
import numpy as np
import ml_dtypes

import concourse.bacc as bacc
import concourse.bass as bass
import concourse.tile as tile
from concourse import mybir
from concourse.bass_utils import run_bass_kernel_spmd

F32 = mybir.dt.float32
BF16 = mybir.dt.bfloat16
FP8 = mybir.dt.float8e4
DR = mybir.MatmulPerfMode.DoubleRow
AF = mybir.ActivationFunctionType
EXP_BIAS = -3.5

NCORES = 8
BPC = 4
C = 512
T = 1024
NCH = 4
NTK = 8
GPC = 8
EPS = 1e-5
SM_SCALE = float(C) ** -0.5


def _emit(nc, tc, ctx, flags):
    trivial_gn, qk_bias, o_bias = flags

    x_d = nc.dram_tensor("x", (BPC, C, T), F32, kind="ExternalInput")
    y_d = nc.dram_tensor("y", (BPC, C, T), F32, kind="ExternalOutput")
    w_d = {
        n: nc.dram_tensor(n, (C, C), BF16, kind="ExternalInput")
        for n in ("wqT", "wkT", "wvT", "wpT")
    }
    ones_d = nc.dram_tensor("ones256", (128, 2, 128), FP8, kind="ExternalInput")
    maskg_d = nc.dram_tensor("mask_g", (128, GPC), F32, kind="ExternalInput")
    bcp_d = nc.dram_tensor("bc_pos", (GPC, 128), F32, kind="ExternalInput")
    bcn_d = nc.dram_tensor("bc_neg", (GPC, 128), F32, kind="ExternalInput")
    if not trivial_gn:
        gnw_d = nc.dram_tensor("gnw", (C,), F32, kind="ExternalInput")
        gnb_d = nc.dram_tensor("gnb", (C,), F32, kind="ExternalInput")
    if qk_bias:
        bq_d = nc.dram_tensor("bq", (C,), F32, kind="ExternalInput")
        bk_d = nc.dram_tensor("bk", (C,), F32, kind="ExternalInput")
    if o_bias:
        bo_d = nc.dram_tensor("bias_o", (C,), F32, kind="ExternalInput")

    from concourse.hw_specs import get_activation_tables

    tabs = list(get_activation_tables(nc.m.arch))
    lnexp_id = tabs.index("natural_log_exp_and_others")
    nc.scalar.add_instruction(
        mybir.InstLoadActFuncSet(
            name=nc.get_next_instruction_name(),
            ins=[],
            outs=[],
            act_func_set_id=lnexp_id,
        )
    )

    x_ap = x_d.ap().rearrange("b (c p) t -> b p c t", p=128)
    y_ap = y_d.ap().rearrange("b (c p) t -> b p c t", p=128)

    singles = ctx.enter_context(tc.tile_pool(name="singles", bufs=1))
    x_pool = ctx.enter_context(tc.tile_pool(name="x", bufs=2))
    h_pool = ctx.enter_context(tc.tile_pool(name="h", bufs=2))
    q_pool = ctx.enter_context(tc.tile_pool(name="q", bufs=2))
    k_pool = ctx.enter_context(tc.tile_pool(name="k", bufs=2))
    v_pool = ctx.enter_context(tc.tile_pool(name="v", bufs=2))
    e_pool = ctx.enter_context(tc.tile_pool(name="e", bufs=2))
    on_pool = ctx.enter_context(tc.tile_pool(name="on", bufs=2))
    out_pool = ctx.enter_context(tc.tile_pool(name="out", bufs=1))
    scr_pool = ctx.enter_context(tc.tile_pool(name="scr", bufs=2))
    st_pool = ctx.enter_context(tc.tile_pool(name="st", bufs=2))
    ps_big = ctx.enter_context(tc.tile_pool(name="ps_big", bufs=2, space="PSUM"))
    ps_sm = ctx.enter_context(tc.tile_pool(name="ps_sm", bufs=4, space="PSUM"))

    wsb = {}
    for n in ("wqT", "wkT", "wvT", "wpT"):
        tiles = []
        for k in range(NCH):
            wt = singles.tile([128, C], BF16, tag=f"{n}{k}")
            nc.sync.dma_start(out=wt[:], in_=w_d[n].ap()[k * 128:(k + 1) * 128, :])
            tiles.append(wt)
        wsb[n] = tiles
    ones_sb = singles.tile([128, 2, 128], FP8, tag="ones")
    nc.sync.dma_start(out=ones_sb[:], in_=ones_d.ap())
    maskg_sb = singles.tile([128, GPC], F32, tag="maskg")
    nc.sync.dma_start(out=maskg_sb[:], in_=maskg_d.ap())
    bcp_sb = singles.tile([GPC, 128], F32, tag="bcp")
    nc.sync.dma_start(out=bcp_sb[:], in_=bcp_d.ap())
    bcn_sb = singles.tile([GPC, 128], F32, tag="bcn")
    nc.sync.dma_start(out=bcn_sb[:], in_=bcn_d.ap())
    eps_sb = singles.tile([GPC, 1], F32, tag="eps")
    nc.vector.memset(eps_sb[:], EPS)
    ebias_sb = singles.tile([128, 1], F32, tag="ebias")
    nc.vector.memset(ebias_sb[:], EXP_BIAS)
    if not trivial_gn:
        gnw_sb = singles.tile([128, NCH], F32, tag="gnw")
        gnb_sb = singles.tile([128, NCH], F32, tag="gnb")
        nc.sync.dma_start(out=gnw_sb[:], in_=gnw_d.ap().rearrange("(c p) -> p c", p=128))
        nc.sync.dma_start(out=gnb_sb[:], in_=gnb_d.ap().rearrange("(c p) -> p c", p=128))
    if qk_bias:
        bq_sb = singles.tile([128, NCH], F32, tag="bq")
        bk_sb = singles.tile([128, NCH], F32, tag="bk")
        nc.sync.dma_start(out=bq_sb[:], in_=bq_d.ap().rearrange("(c p) -> p c", p=128))
        nc.sync.dma_start(out=bk_sb[:], in_=bk_d.ap().rearrange("(c p) -> p c", p=128))
    if o_bias:
        bo_sb = singles.tile([128, NCH], F32, tag="bo")
        nc.sync.dma_start(out=bo_sb[:], in_=bo_d.ap().rearrange("(c p) -> p c", p=128))

    for b in range(BPC):
        xt = x_pool.tile([128, NCH, T], F32, tag="x")
        nc.sync.dma_start(out=xt[:], in_=x_ap[b])

        mv6 = st_pool.tile([128, NCH, 2, 6], F32, tag="mv6")
        mv = st_pool.tile([128, NCH, 2], F32, tag="mv")
        for ci in range(NCH):
            for s in range(2):
                nc.vector.bn_stats(
                    out=mv6[:, ci, s, :], in_=xt[:, ci, s * 512:(s + 1) * 512]
                )
            nc.vector.bn_aggr(out=mv[:, ci, :], in_=mv6[:, ci, :, :])
        msq = st_pool.tile([128, NCH, 1], F32, tag="msq")
        m2 = st_pool.tile([128, NCH, 1], F32, tag="m2")
        nc.vector.tensor_mul(msq[:], mv[:, :, 0:1], mv[:, :, 0:1])
        nc.vector.tensor_add(m2[:], mv[:, :, 1:2], msq[:])
        psum_g = ps_sm.tile([GPC, 2 * NCH], F32, tag="acc", name="psum_g")
        for ci in range(NCH):
            nc.tensor.matmul(
                psum_g[:, ci:ci + 1], maskg_sb[:], mv[:, ci, 0:1],
                start=True, stop=True,
            )
            nc.tensor.matmul(
                psum_g[:, NCH + ci:NCH + ci + 1], maskg_sb[:], m2[:, ci, :],
                start=True, stop=True,
            )
        gstat = st_pool.tile([GPC, 2 * NCH], F32, tag="gstat")
        nc.vector.tensor_scalar_mul(gstat[:], psum_g[:], 1.0 / 16.0)
        sqg = st_pool.tile([GPC, NCH], F32, tag="sqg")
        varg = st_pool.tile([GPC, NCH], F32, tag="varg")
        nc.vector.tensor_mul(sqg[:], gstat[:, 0:NCH], gstat[:, 0:NCH])
        nc.vector.tensor_sub(varg[:], gstat[:, NCH:2 * NCH], sqg[:])
        lnv = st_pool.tile([GPC, NCH], F32, tag="lnv")
        nc.scalar.activation(lnv[:], varg[:], AF.Ln, bias=eps_sb[:])
        rstd = st_pool.tile([GPC, NCH], F32, tag="rstd")
        nc.scalar.activation(rstd[:], lnv[:], AF.Exp, scale=-0.5)
        psum_bc = ps_sm.tile([128, 2 * NCH], F32, tag="acc", name="psum_bc")
        for ci in range(NCH):
            nc.tensor.matmul(
                psum_bc[:, ci:ci + 1], bcn_sb[:], gstat[:, ci:ci + 1],
                start=True, stop=True,
            )
            nc.tensor.matmul(
                psum_bc[:, NCH + ci:NCH + ci + 1], bcp_sb[:], rstd[:, ci:ci + 1],
                start=True, stop=True,
            )
        bc = st_pool.tile([128, 2 * NCH], F32, tag="bc")
        nc.vector.tensor_copy(bc[:], psum_bc[:])
        if trivial_gn:
            scale_t = bc[:, NCH:2 * NCH]
            shift_t = st_pool.tile([128, NCH], F32, tag="shift")
            nc.vector.tensor_mul(shift_t[:], bc[:, 0:NCH], bc[:, NCH:2 * NCH])
        else:
            scale_full = st_pool.tile([128, NCH], F32, tag="scalef")
            nc.vector.tensor_mul(scale_full[:], bc[:, NCH:2 * NCH], gnw_sb[:])
            tmp = st_pool.tile([128, NCH], F32, tag="tmpf")
            nc.vector.tensor_mul(tmp[:], bc[:, 0:NCH], scale_full[:])
            shift_t = st_pool.tile([128, NCH], F32, tag="shift")
            nc.vector.tensor_add(shift_t[:], tmp[:], gnb_sb[:])
            scale_t = scale_full

        ht = h_pool.tile([128, NCH, T], BF16, tag="h")
        for ci in range(NCH):
            nc.scalar.activation(
                ht[:, ci, :], xt[:, ci, :], AF.Identity,
                bias=shift_t[:, ci:ci + 1], scale=scale_t[:, ci:ci + 1],
            )

        qt = q_pool.tile([128, NCH, T], FP8, tag="q")
        kt = k_pool.tile([128, NCH, T], FP8, tag="k")
        vt = v_pool.tile([128, NTK, C], FP8, tag="v")
        for m in range(NCH):
            ps = ps_big.tile([128, T], F32, tag="big", name="ps_q")
            for n2 in range(2):
                for k in range(NCH):
                    nc.tensor.matmul(
                        ps[:, n2 * 512:(n2 + 1) * 512],
                        wsb["wqT"][k][:, m * 128:(m + 1) * 128],
                        ht[:, k, n2 * 512:(n2 + 1) * 512],
                        start=(k == 0), stop=(k == NCH - 1),
                    )
            if qk_bias:
                nc.scalar.activation(
                    qt[:, m, :], ps[:], AF.Identity, bias=bq_sb[:, m:m + 1]
                )
            else:
                nc.scalar.copy(qt[:, m, :], ps[:])
            ps = ps_big.tile([128, T], F32, tag="big", name="ps_k")
            for n2 in range(2):
                for k in range(NCH):
                    nc.tensor.matmul(
                        ps[:, n2 * 512:(n2 + 1) * 512],
                        wsb["wkT"][k][:, m * 128:(m + 1) * 128],
                        ht[:, k, n2 * 512:(n2 + 1) * 512],
                        start=(k == 0), stop=(k == NCH - 1),
                    )
            if qk_bias:
                nc.vector.tensor_scalar_add(kt[:, m, :], ps[:], bk_sb[:, m:m + 1])
            else:
                nc.vector.tensor_copy(kt[:, m, :], ps[:])
        for mp in range(NTK // 2):
            ps = ps_big.tile([128, T], F32, tag="big", name="ps_v")
            for half in range(2):
                m = 2 * mp + half
                for k in range(NCH):
                    nc.tensor.matmul(
                        ps[:, half * 512:(half + 1) * 512],
                        ht[:, k, m * 128:(m + 1) * 128],
                        wsb["wvT"][k][:],
                        start=(k == 0), stop=(k == NCH - 1),
                    )
            nc.scalar.copy(vt[:, 2 * mp:2 * mp + 2, :], ps[:])

        et = e_pool.tile([128, NTK, T], FP8, tag="e")
        for tk in range(NTK):
            ps_s = ps_big.tile([128, T], F32, tag="big", name="ps_s")
            for n2 in range(2):
                for j in range(NCH // 2):
                    nc.tensor.matmul(
                        ps_s[:, n2 * 512:(n2 + 1) * 512],
                        kt[:, 2 * j:2 * j + 2, tk * 128:(tk + 1) * 128],
                        qt[:, 2 * j:2 * j + 2, n2 * 512:(n2 + 1) * 512],
                        start=(j == 0), stop=(j == NCH // 2 - 1),
                        perf_mode=DR,
                    )
            nc.scalar.activation(
                et[:, tk, :], ps_s[:], AF.Exp, scale=SM_SCALE, bias=ebias_sb[:]
            )
        ont = on_pool.tile([128, NCH, T], BF16, tag="on")
        for n2 in range(2):
            tq = slice(n2 * 512, (n2 + 1) * 512)
            pc = ps_sm.tile([128, 512], F32, tag="acc", name="pc")
            for j in range(NTK // 2):
                nc.tensor.matmul(
                    pc[:], ones_sb[:], et[:, 2 * j:2 * j + 2, tq],
                    start=(j == 0), stop=(j == NTK // 2 - 1),
                    perf_mode=DR,
                )
            rec = scr_pool.tile([128, 512], F32, tag="rec")
            nc.vector.reciprocal_approx_fast(out=rec[:], in_=pc[:])
            for m in range(NCH):
                po = ps_sm.tile([128, 512], F32, tag="acc", name="po")
                for j in range(NTK // 2):
                    nc.tensor.matmul(
                        po[:],
                        vt[:, 2 * j:2 * j + 2, m * 128:(m + 1) * 128],
                        et[:, 2 * j:2 * j + 2, tq],
                        start=(j == 0), stop=(j == NTK // 2 - 1),
                        perf_mode=DR,
                    )
                nc.vector.tensor_mul(ont[:, m, tq], po[:], rec[:])

        out_t = out_pool.tile([128, NCH, T], F32, tag="out")
        for m in range(NCH):
            ps = ps_big.tile([128, T], F32, tag="big", name="ps_p")
            for n2 in range(2):
                for k in range(NCH):
                    nc.tensor.matmul(
                        ps[:, n2 * 512:(n2 + 1) * 512],
                        wsb["wpT"][k][:, m * 128:(m + 1) * 128],
                        ont[:, k, n2 * 512:(n2 + 1) * 512],
                        start=(k == 0), stop=(k == NCH - 1),
                    )
            if o_bias:
                nc.vector.scalar_tensor_tensor(
                    out_t[:, m, :], ps[:], bo_sb[:, m:m + 1], xt[:, m, :],
                    op0=mybir.AluOpType.add, op1=mybir.AluOpType.add,
                )
            else:
                nc.vector.tensor_add(out_t[:, m, :], ps[:], xt[:, m, :])
        nc.sync.dma_start(out=y_ap[b], in_=out_t[:])


def _build(flags):
    from contextlib import ExitStack

    nc = bacc.Bacc(
        "TRN2",
        target_bir_lowering=False,
        debug=False,
        enable_asserts=False,
        num_devices=NCORES,
    )
    with tile.TileContext(nc) as tc:
        with ExitStack() as ctx:
            _emit(nc, tc, ctx, flags)
    nc.compile()
    return nc


_CACHE = {}


def _consts():
    p = np.arange(128)
    maskg = (p[:, None] // 16 == np.arange(GPC)[None, :]).astype(np.float32)
    bcp = maskg.T.copy()
    return {
        "ones256": np.ones((128, 2, 128), mybir.dt.np(FP8)),
        "mask_g": maskg,
        "bc_pos": np.ascontiguousarray(bcp),
        "bc_neg": np.ascontiguousarray(-bcp),
    }


def kernel(x, gn_w, gn_b, wq, bq, wk, bk, wv, bv, wp, bp):
    x = np.ascontiguousarray(np.asarray(x, dtype=np.float32))
    B = x.shape[0]
    assert (B, x.shape[1], x.shape[2] * x.shape[3]) == (NCORES * BPC, C, T)
    H, W = x.shape[2], x.shape[3]
    xr = x.reshape(B, C, T)
    gn_w = np.asarray(gn_w, np.float32)
    gn_b = np.asarray(gn_b, np.float32)
    trivial_gn = bool(np.all(gn_w == 1.0) and np.all(gn_b == 0.0))
    qk_bias = bool(np.any(np.asarray(bq)) or np.any(np.asarray(bk)))
    o_bias = bool(np.any(np.asarray(bv)) or np.any(np.asarray(bp)))
    flags = (trivial_gn, qk_bias, o_bias)
    if flags not in _CACHE:
        _CACHE[flags] = _build(flags)
    nc = _CACHE[flags]

    bf = ml_dtypes.bfloat16
    common = dict(_consts())
    for n, w in (("wqT", wq), ("wkT", wk), ("wvT", wv), ("wpT", wp)):
        common[n] = np.ascontiguousarray(np.asarray(w, np.float32).T).astype(bf)
    if not trivial_gn:
        common["gnw"] = gn_w
        common["gnb"] = gn_b
    if qk_bias:
        common["bq"] = np.asarray(bq, np.float32)
        common["bk"] = np.asarray(bk, np.float32)
    if o_bias:
        common["bias_o"] = (
            np.asarray(wp, np.float32) @ np.asarray(bv, np.float32)
            + np.asarray(bp, np.float32)
        ).astype(np.float32)

    in_maps = [
        {"x": np.ascontiguousarray(xr[c * BPC:(c + 1) * BPC]), **common}
        for c in range(NCORES)
    ]
    res = run_bass_kernel_spmd(nc, in_maps, core_ids=list(range(NCORES)))
    y = np.concatenate([res.results[c]["y"] for c in range(NCORES)], axis=0)
    return np.ascontiguousarray(y.reshape(B, C, H, W).astype(np.float32))


# revision 19
# speedup vs baseline: 1.7419x; 1.1473x over previous
import numpy as np
import ml_dtypes

import concourse.bacc as bacc
import concourse.bass as bass
import concourse.tile as tile
from concourse import mybir
from concourse.bass_utils import run_bass_kernel_spmd

F32 = mybir.dt.float32
BF16 = mybir.dt.bfloat16
FP8 = mybir.dt.float8e4
DR = mybir.MatmulPerfMode.DoubleRow
AF = mybir.ActivationFunctionType
EXP_BIAS = -3.5

NCORES = 8
BPC = 4
C = 512
T = 1024
NCH = 4
NTK = 8
GPC = 8
EPS = 1e-5
SM_SCALE = float(C) ** -0.5


def _emit(nc, tc, ctx, flags):
    trivial_gn, qk_bias, o_bias = flags

    x_d = nc.dram_tensor("x", (BPC, C, T), F32, kind="ExternalInput")
    y_d = nc.dram_tensor("y", (BPC, C, T), F32, kind="ExternalOutput")
    w_d = {
        n: nc.dram_tensor(n, (NCH // 2, 128, 2, C), FP8, kind="ExternalInput")
        for n in ("wqT", "wkT", "wvT", "wpT")
    }
    ds_d = nc.dram_tensor("descale", (4,), F32, kind="ExternalInput")
    ones_d = nc.dram_tensor("ones256", (128, 2, 128), FP8, kind="ExternalInput")
    maskg_d = nc.dram_tensor("mask_g", (128, GPC), F32, kind="ExternalInput")
    bcp_d = nc.dram_tensor("bc_pos", (GPC, 128), F32, kind="ExternalInput")
    bcn_d = nc.dram_tensor("bc_neg", (GPC, 128), F32, kind="ExternalInput")
    if not trivial_gn:
        gnw_d = nc.dram_tensor("gnw", (C,), F32, kind="ExternalInput")
        gnb_d = nc.dram_tensor("gnb", (C,), F32, kind="ExternalInput")
    if qk_bias:
        bq_d = nc.dram_tensor("bq", (C,), F32, kind="ExternalInput")
        bk_d = nc.dram_tensor("bk", (C,), F32, kind="ExternalInput")
    if o_bias:
        bo_d = nc.dram_tensor("bias_o", (C,), F32, kind="ExternalInput")

    from concourse.hw_specs import get_activation_tables

    tabs = list(get_activation_tables(nc.m.arch))
    lnexp_id = tabs.index("natural_log_exp_and_others")
    nc.scalar.add_instruction(
        mybir.InstLoadActFuncSet(
            name=nc.get_next_instruction_name(),
            ins=[],
            outs=[],
            act_func_set_id=lnexp_id,
        )
    )

    x_ap = x_d.ap().rearrange("b (c p) t -> b p c t", p=128)
    y_ap = y_d.ap().rearrange("b (c p) t -> b p c t", p=128)

    singles = ctx.enter_context(tc.tile_pool(name="singles", bufs=1))
    x_pool = ctx.enter_context(tc.tile_pool(name="x", bufs=2))
    h_pool = ctx.enter_context(tc.tile_pool(name="h", bufs=2))
    q_pool = ctx.enter_context(tc.tile_pool(name="q", bufs=2))
    k_pool = ctx.enter_context(tc.tile_pool(name="k", bufs=2))
    v_pool = ctx.enter_context(tc.tile_pool(name="v", bufs=2))
    e_pool = ctx.enter_context(tc.tile_pool(name="e", bufs=2))
    on_pool = ctx.enter_context(tc.tile_pool(name="on", bufs=2))
    out_pool = ctx.enter_context(tc.tile_pool(name="out", bufs=1))
    scr_pool = ctx.enter_context(tc.tile_pool(name="scr", bufs=2))
    st_pool = ctx.enter_context(tc.tile_pool(name="st", bufs=2))
    ps_big = ctx.enter_context(tc.tile_pool(name="ps_big", bufs=2, space="PSUM"))
    ps_sm = ctx.enter_context(tc.tile_pool(name="ps_sm", bufs=4, space="PSUM"))

    wsb = {}
    for n in ("wqT", "wkT", "wvT", "wpT"):
        tiles = []
        for j in range(NCH // 2):
            wt = singles.tile([128, 2, C], FP8, tag=f"{n}{j}")
            nc.sync.dma_start(out=wt[:], in_=w_d[n].ap()[j])
            tiles.append(wt)
        wsb[n] = tiles
    ds_sb = singles.tile([128, 4], F32, tag="descale")
    for i in range(4):
        nc.sync.dma_start(
            out=ds_sb[:, i:i + 1],
            in_=bass.AP(tensor=ds_d, offset=i, ap=[[0, 128], [1, 1]]),
        )
    ones_sb = singles.tile([128, 2, 128], FP8, tag="ones")
    nc.sync.dma_start(out=ones_sb[:], in_=ones_d.ap())
    maskg_sb = singles.tile([128, GPC], F32, tag="maskg")
    nc.sync.dma_start(out=maskg_sb[:], in_=maskg_d.ap())
    bcp_sb = singles.tile([GPC, 128], F32, tag="bcp")
    nc.sync.dma_start(out=bcp_sb[:], in_=bcp_d.ap())
    bcn_sb = singles.tile([GPC, 128], F32, tag="bcn")
    nc.sync.dma_start(out=bcn_sb[:], in_=bcn_d.ap())
    eps_sb = singles.tile([GPC, 1], F32, tag="eps")
    nc.vector.memset(eps_sb[:], EPS)
    ebias_sb = singles.tile([128, 1], F32, tag="ebias")
    nc.vector.memset(ebias_sb[:], EXP_BIAS)
    if not trivial_gn:
        gnw_sb = singles.tile([128, NCH], F32, tag="gnw")
        gnb_sb = singles.tile([128, NCH], F32, tag="gnb")
        nc.sync.dma_start(out=gnw_sb[:], in_=gnw_d.ap().rearrange("(c p) -> p c", p=128))
        nc.sync.dma_start(out=gnb_sb[:], in_=gnb_d.ap().rearrange("(c p) -> p c", p=128))
    if qk_bias:
        bq_sb = singles.tile([128, NCH], F32, tag="bq")
        bk_sb = singles.tile([128, NCH], F32, tag="bk")
        nc.sync.dma_start(out=bq_sb[:], in_=bq_d.ap().rearrange("(c p) -> p c", p=128))
        nc.sync.dma_start(out=bk_sb[:], in_=bk_d.ap().rearrange("(c p) -> p c", p=128))
    if o_bias:
        bo_sb = singles.tile([128, NCH], F32, tag="bo")
        nc.sync.dma_start(out=bo_sb[:], in_=bo_d.ap().rearrange("(c p) -> p c", p=128))

    for b in range(BPC):
        xt = x_pool.tile([128, NCH, T], F32, tag="x")
        nc.sync.dma_start(out=xt[:], in_=x_ap[b])

        mv6 = st_pool.tile([128, NCH, 2, 6], F32, tag="mv6")
        mv = st_pool.tile([128, NCH, 2], F32, tag="mv")
        for ci in range(NCH):
            for s in range(2):
                nc.vector.bn_stats(
                    out=mv6[:, ci, s, :], in_=xt[:, ci, s * 512:(s + 1) * 512]
                )
            nc.vector.bn_aggr(out=mv[:, ci, :], in_=mv6[:, ci, :, :])
        msq = st_pool.tile([128, NCH, 1], F32, tag="msq")
        m2 = st_pool.tile([128, NCH, 1], F32, tag="m2")
        nc.vector.tensor_mul(msq[:], mv[:, :, 0:1], mv[:, :, 0:1])
        nc.vector.tensor_add(m2[:], mv[:, :, 1:2], msq[:])
        psum_g = ps_sm.tile([GPC, 2 * NCH], F32, tag="acc", name="psum_g")
        for ci in range(NCH):
            nc.tensor.matmul(
                psum_g[:, ci:ci + 1], maskg_sb[:], mv[:, ci, 0:1],
                start=True, stop=True,
            )
            nc.tensor.matmul(
                psum_g[:, NCH + ci:NCH + ci + 1], maskg_sb[:], m2[:, ci, :],
                start=True, stop=True,
            )
        gstat = st_pool.tile([GPC, 2 * NCH], F32, tag="gstat")
        nc.vector.tensor_scalar_mul(gstat[:], psum_g[:], 1.0 / 16.0)
        sqg = st_pool.tile([GPC, NCH], F32, tag="sqg")
        varg = st_pool.tile([GPC, NCH], F32, tag="varg")
        nc.vector.tensor_mul(sqg[:], gstat[:, 0:NCH], gstat[:, 0:NCH])
        nc.vector.tensor_sub(varg[:], gstat[:, NCH:2 * NCH], sqg[:])
        lnv = st_pool.tile([GPC, NCH], F32, tag="lnv")
        nc.scalar.activation(lnv[:], varg[:], AF.Ln, bias=eps_sb[:])
        rstd = st_pool.tile([GPC, NCH], F32, tag="rstd")
        nc.scalar.activation(rstd[:], lnv[:], AF.Exp, scale=-0.5)
        psum_bc = ps_sm.tile([128, 2 * NCH], F32, tag="acc", name="psum_bc")
        for ci in range(NCH):
            nc.tensor.matmul(
                psum_bc[:, ci:ci + 1], bcn_sb[:], gstat[:, ci:ci + 1],
                start=True, stop=True,
            )
            nc.tensor.matmul(
                psum_bc[:, NCH + ci:NCH + ci + 1], bcp_sb[:], rstd[:, ci:ci + 1],
                start=True, stop=True,
            )
        bc = st_pool.tile([128, 2 * NCH], F32, tag="bc")
        nc.vector.tensor_copy(bc[:], psum_bc[:])
        if trivial_gn:
            scale_t = bc[:, NCH:2 * NCH]
            shift_t = st_pool.tile([128, NCH], F32, tag="shift")
            nc.vector.tensor_mul(shift_t[:], bc[:, 0:NCH], bc[:, NCH:2 * NCH])
        else:
            scale_full = st_pool.tile([128, NCH], F32, tag="scalef")
            nc.vector.tensor_mul(scale_full[:], bc[:, NCH:2 * NCH], gnw_sb[:])
            tmp = st_pool.tile([128, NCH], F32, tag="tmpf")
            nc.vector.tensor_mul(tmp[:], bc[:, 0:NCH], scale_full[:])
            shift_t = st_pool.tile([128, NCH], F32, tag="shift")
            nc.vector.tensor_add(shift_t[:], tmp[:], gnb_sb[:])
            scale_t = scale_full

        ht = h_pool.tile([128, NCH, T], FP8, tag="h")
        for ci in range(NCH):
            nc.scalar.activation(
                ht[:, ci, :], xt[:, ci, :], AF.Identity,
                bias=shift_t[:, ci:ci + 1], scale=scale_t[:, ci:ci + 1],
            )

        qt = q_pool.tile([128, NCH, T], FP8, tag="q")
        kt = k_pool.tile([128, NCH, T], FP8, tag="k")
        vt = v_pool.tile([128, NTK, C], FP8, tag="v")
        for m in range(NCH):
            ps = ps_big.tile([128, T], F32, tag="big", name="ps_q")
            for n2 in range(2):
                for j in range(NCH // 2):
                    nc.tensor.matmul(
                        ps[:, n2 * 512:(n2 + 1) * 512],
                        wsb["wqT"][j][:, :, m * 128:(m + 1) * 128],
                        ht[:, 2 * j:2 * j + 2, n2 * 512:(n2 + 1) * 512],
                        start=(j == 0), stop=(j == NCH // 2 - 1),
                        perf_mode=DR,
                    )
            if qk_bias:
                nc.scalar.activation(
                    qt[:, m, :], ps[:], AF.Identity,
                    bias=bq_sb[:, m:m + 1], scale=ds_sb[:, 0:1],
                )
            else:
                nc.scalar.mul(qt[:, m, :], ps[:], ds_sb[:, 0:1])
            ps = ps_big.tile([128, T], F32, tag="big", name="ps_k")
            for n2 in range(2):
                for j in range(NCH // 2):
                    nc.tensor.matmul(
                        ps[:, n2 * 512:(n2 + 1) * 512],
                        wsb["wkT"][j][:, :, m * 128:(m + 1) * 128],
                        ht[:, 2 * j:2 * j + 2, n2 * 512:(n2 + 1) * 512],
                        start=(j == 0), stop=(j == NCH // 2 - 1),
                        perf_mode=DR,
                    )
            if qk_bias:
                nc.vector.tensor_scalar(
                    kt[:, m, :], ps[:], ds_sb[:, 1:2], bk_sb[:, m:m + 1],
                    op0=mybir.AluOpType.mult, op1=mybir.AluOpType.add,
                )
            else:
                nc.vector.tensor_scalar_mul(kt[:, m, :], ps[:], ds_sb[:, 1:2])
        for mp in range(NTK // 2):
            ps = ps_big.tile([128, T], F32, tag="big", name="ps_v")
            for half in range(2):
                m = 2 * mp + half
                for j in range(NCH // 2):
                    nc.tensor.matmul(
                        ps[:, half * 512:(half + 1) * 512],
                        ht[:, 2 * j:2 * j + 2, m * 128:(m + 1) * 128],
                        wsb["wvT"][j][:],
                        start=(j == 0), stop=(j == NCH // 2 - 1),
                        perf_mode=DR,
                    )
            nc.scalar.mul(vt[:, 2 * mp:2 * mp + 2, :], ps[:], ds_sb[:, 2:3])

        et = e_pool.tile([128, NTK, T], FP8, tag="e")
        for tk in range(NTK):
            ps_s = ps_big.tile([128, T], F32, tag="big", name="ps_s")
            for n2 in range(2):
                for j in range(NCH // 2):
                    nc.tensor.matmul(
                        ps_s[:, n2 * 512:(n2 + 1) * 512],
                        kt[:, 2 * j:2 * j + 2, tk * 128:(tk + 1) * 128],
                        qt[:, 2 * j:2 * j + 2, n2 * 512:(n2 + 1) * 512],
                        start=(j == 0), stop=(j == NCH // 2 - 1),
                        perf_mode=DR,
                    )
            nc.scalar.activation(
                et[:, tk, :], ps_s[:], AF.Exp, scale=SM_SCALE, bias=ebias_sb[:]
            )
        ont = on_pool.tile([128, NCH, T], FP8, tag="on")
        for n2 in range(2):
            tq = slice(n2 * 512, (n2 + 1) * 512)
            pc = ps_sm.tile([128, 512], F32, tag="acc", name="pc")
            for j in range(NTK // 2):
                nc.tensor.matmul(
                    pc[:], ones_sb[:], et[:, 2 * j:2 * j + 2, tq],
                    start=(j == 0), stop=(j == NTK // 2 - 1),
                    perf_mode=DR,
                )
            rec = scr_pool.tile([128, 512], F32, tag="rec")
            nc.vector.reciprocal_approx_fast(out=rec[:], in_=pc[:])
            for m in range(NCH):
                po = ps_sm.tile([128, 512], F32, tag="acc", name="po")
                for j in range(NTK // 2):
                    nc.tensor.matmul(
                        po[:],
                        vt[:, 2 * j:2 * j + 2, m * 128:(m + 1) * 128],
                        et[:, 2 * j:2 * j + 2, tq],
                        start=(j == 0), stop=(j == NTK // 2 - 1),
                        perf_mode=DR,
                    )
                nc.vector.tensor_mul(ont[:, m, tq], po[:], rec[:])

        out_t = out_pool.tile([128, NCH, T], F32, tag="out")
        for m in range(NCH):
            ps = ps_big.tile([128, T], F32, tag="big", name="ps_p")
            for n2 in range(2):
                for j in range(NCH // 2):
                    nc.tensor.matmul(
                        ps[:, n2 * 512:(n2 + 1) * 512],
                        wsb["wpT"][j][:, :, m * 128:(m + 1) * 128],
                        ont[:, 2 * j:2 * j + 2, n2 * 512:(n2 + 1) * 512],
                        start=(j == 0), stop=(j == NCH // 2 - 1),
                        perf_mode=DR,
                    )
            nc.vector.scalar_tensor_tensor(
                out_t[:, m, :], ps[:], ds_sb[:, 3:4], xt[:, m, :],
                op0=mybir.AluOpType.mult, op1=mybir.AluOpType.add,
            )
            if o_bias:
                nc.vector.tensor_scalar_add(
                    out_t[:, m, :], out_t[:, m, :], bo_sb[:, m:m + 1]
                )
        nc.sync.dma_start(out=y_ap[b], in_=out_t[:])


def _build(flags):
    from contextlib import ExitStack

    nc = bacc.Bacc(
        "TRN2",
        target_bir_lowering=False,
        debug=False,
        enable_asserts=False,
        num_devices=NCORES,
    )
    with tile.TileContext(nc) as tc:
        with ExitStack() as ctx:
            _emit(nc, tc, ctx, flags)
    nc.compile()
    return nc


_CACHE = {}


def _consts():
    p = np.arange(128)
    maskg = (p[:, None] // 16 == np.arange(GPC)[None, :]).astype(np.float32)
    bcp = maskg.T.copy()
    return {
        "ones256": np.ones((128, 2, 128), mybir.dt.np(FP8)),
        "mask_g": maskg,
        "bc_pos": np.ascontiguousarray(bcp),
        "bc_neg": np.ascontiguousarray(-bcp),
    }


def kernel(x, gn_w, gn_b, wq, bq, wk, bk, wv, bv, wp, bp):
    x = np.ascontiguousarray(np.asarray(x, dtype=np.float32))
    B = x.shape[0]
    assert (B, x.shape[1], x.shape[2] * x.shape[3]) == (NCORES * BPC, C, T)
    H, W = x.shape[2], x.shape[3]
    xr = x.reshape(B, C, T)
    gn_w = np.asarray(gn_w, np.float32)
    gn_b = np.asarray(gn_b, np.float32)
    trivial_gn = bool(np.all(gn_w == 1.0) and np.all(gn_b == 0.0))
    qk_bias = bool(np.any(np.asarray(bq)) or np.any(np.asarray(bk)))
    o_bias = bool(np.any(np.asarray(bv)) or np.any(np.asarray(bp)))
    flags = (trivial_gn, qk_bias, o_bias)
    if flags not in _CACHE:
        _CACHE[flags] = _build(flags)
    nc = _CACHE[flags]

    common = dict(_consts())
    descale = np.empty(4, np.float32)
    fp8 = mybir.dt.np(FP8)
    for i, (n, w) in enumerate(
        (("wqT", wq), ("wkT", wk), ("wvT", wv), ("wpT", wp))
    ):
        wT = np.ascontiguousarray(np.asarray(w, np.float32).T)
        amax = float(np.abs(wT).max()) or 1.0
        k = int(np.floor(np.log2(88.0 / amax)))
        descale[i] = 2.0 ** (-k)
        ws = (wT * (2.0 ** k)).astype(fp8)
        common[n] = np.ascontiguousarray(
            ws.reshape(NCH // 2, 2, 128, C).transpose(0, 2, 1, 3)
        )
    common["descale"] = descale
    if not trivial_gn:
        common["gnw"] = gn_w
        common["gnb"] = gn_b
    if qk_bias:
        common["bq"] = np.asarray(bq, np.float32)
        common["bk"] = np.asarray(bk, np.float32)
    if o_bias:
        common["bias_o"] = (
            np.asarray(wp, np.float32) @ np.asarray(bv, np.float32)
            + np.asarray(bp, np.float32)
        ).astype(np.float32)

    in_maps = [
        {"x": np.ascontiguousarray(xr[c * BPC:(c + 1) * BPC]), **common}
        for c in range(NCORES)
    ]
    res = run_bass_kernel_spmd(nc, in_maps, core_ids=list(range(NCORES)))
    y = np.concatenate([res.results[c]["y"] for c in range(NCORES)], axis=0)
    return np.ascontiguousarray(y.reshape(B, C, H, W).astype(np.float32))


# revision 33
# speedup vs baseline: 2.2134x; 1.2707x over previous
import numpy as np
import ml_dtypes

import concourse.bacc as bacc
import concourse.bass as bass
import concourse.tile as tile
from concourse import mybir
from concourse.bass_utils import run_bass_kernel_spmd

F32 = mybir.dt.float32
BF16 = mybir.dt.bfloat16
FP8 = mybir.dt.float8e4
DR = mybir.MatmulPerfMode.DoubleRow
AF = mybir.ActivationFunctionType
EXP_BIAS = -3.5

NCORES = 8
BPC = 4
C = 512
T = 1024
NCH = 4
NTK = 8
GPC = 8
EPS = 1e-5
SM_SCALE = float(C) ** -0.5


def _emit(nc, tc, ctx, flags):
    trivial_gn, qk_bias, o_bias = flags

    x_d = nc.dram_tensor("x", (BPC, C, T), F32, kind="ExternalInput")
    y_d = nc.dram_tensor("y", (BPC, C, T), F32, kind="ExternalOutput")
    w_d = {
        n: nc.dram_tensor(n, (NCH // 2, 128, 2, C), FP8, kind="ExternalInput")
        for n in ("wqT", "wkT", "wvT", "wpT")
    }
    ds_d = nc.dram_tensor("descale", (4,), F32, kind="ExternalInput")
    ones_d = nc.dram_tensor("ones256", (128, 2, 128), FP8, kind="ExternalInput")
    maskg_d = nc.dram_tensor("mask_g", (128, GPC), F32, kind="ExternalInput")
    bcp_d = nc.dram_tensor("bc_pos", (GPC, 128), F32, kind="ExternalInput")
    bcn_d = nc.dram_tensor("bc_neg", (GPC, 128), F32, kind="ExternalInput")
    if not trivial_gn:
        gnw_d = nc.dram_tensor("gnw", (C,), F32, kind="ExternalInput")
        gnb_d = nc.dram_tensor("gnb", (C,), F32, kind="ExternalInput")
    if qk_bias:
        bq_d = nc.dram_tensor("bq", (C,), F32, kind="ExternalInput")
        bk_d = nc.dram_tensor("bk", (C,), F32, kind="ExternalInput")
    if o_bias:
        bo_d = nc.dram_tensor("bias_o", (C,), F32, kind="ExternalInput")

    from concourse.hw_specs import get_activation_tables

    tabs = list(get_activation_tables(nc.m.arch))
    lnexp_id = tabs.index("natural_log_exp_and_others")
    nc.scalar.add_instruction(
        mybir.InstLoadActFuncSet(
            name=nc.get_next_instruction_name(),
            ins=[],
            outs=[],
            act_func_set_id=lnexp_id,
        )
    )

    x_ap = x_d.ap().rearrange("b (c p) t -> b p c t", p=128)
    y_ap = y_d.ap().rearrange("b (c p) t -> b p c t", p=128)

    singles = ctx.enter_context(tc.tile_pool(name="singles", bufs=1))
    x_pool = ctx.enter_context(tc.tile_pool(name="x", bufs=3))
    h_pool = ctx.enter_context(tc.tile_pool(name="h", bufs=2))
    q_pool = ctx.enter_context(tc.tile_pool(name="q", bufs=2))
    k_pool = ctx.enter_context(tc.tile_pool(name="k", bufs=2))
    v_pool = ctx.enter_context(tc.tile_pool(name="v", bufs=2))
    e_pool = ctx.enter_context(tc.tile_pool(name="e", bufs=2))
    on_pool = ctx.enter_context(tc.tile_pool(name="on", bufs=2))
    out_pool = ctx.enter_context(tc.tile_pool(name="out", bufs=1))
    scr_pool = ctx.enter_context(tc.tile_pool(name="scr", bufs=2))
    st_pool = ctx.enter_context(tc.tile_pool(name="st", bufs=2))
    ps_big = ctx.enter_context(tc.tile_pool(name="ps_big", bufs=3, space="PSUM"))
    ps_sm = ctx.enter_context(tc.tile_pool(name="ps_sm", bufs=2, space="PSUM"))

    xts = {}

    def load_x(b):
        xt = x_pool.tile([128, NCH, T], F32, tag="x", name=f"xt{b}")
        for ci in range(NCH):
            nc.sync.dma_start(out=xt[:, ci, :], in_=x_ap[b][:, ci, :])
        xts[b] = xt

    load_x(0)

    wsb = {}
    for n in ("wqT", "wkT", "wvT", "wpT"):
        tiles = []
        for j in range(NCH // 2):
            wt = singles.tile([128, 2, C], FP8, tag=f"{n}{j}")
            nc.sync.dma_start(out=wt[:], in_=w_d[n].ap()[j])
            tiles.append(wt)
        wsb[n] = tiles
    ds_sb = singles.tile([128, 4], F32, tag="descale")
    for i in range(4):
        nc.sync.dma_start(
            out=ds_sb[:, i:i + 1],
            in_=bass.AP(tensor=ds_d, offset=i, ap=[[0, 128], [1, 1]]),
        )
    ones_sb = singles.tile([128, 2, 128], FP8, tag="ones")
    nc.sync.dma_start(out=ones_sb[:], in_=ones_d.ap())
    maskg_sb = singles.tile([128, GPC], F32, tag="maskg")
    nc.sync.dma_start(out=maskg_sb[:], in_=maskg_d.ap())
    bcp_sb = singles.tile([GPC, 128], F32, tag="bcp")
    nc.sync.dma_start(out=bcp_sb[:], in_=bcp_d.ap())
    bcn_sb = singles.tile([GPC, 128], F32, tag="bcn")
    nc.sync.dma_start(out=bcn_sb[:], in_=bcn_d.ap())
    eps_sb = singles.tile([GPC, 1], F32, tag="eps")
    nc.vector.memset(eps_sb[:], EPS)
    ebias_sb = singles.tile([128, 1], F32, tag="ebias")
    nc.vector.memset(ebias_sb[:], EXP_BIAS)
    if not trivial_gn:
        gnw_sb = singles.tile([128, NCH], F32, tag="gnw")
        gnb_sb = singles.tile([128, NCH], F32, tag="gnb")
        nc.sync.dma_start(out=gnw_sb[:], in_=gnw_d.ap().rearrange("(c p) -> p c", p=128))
        nc.sync.dma_start(out=gnb_sb[:], in_=gnb_d.ap().rearrange("(c p) -> p c", p=128))
    if qk_bias:
        bq_sb = singles.tile([128, NCH], F32, tag="bq")
        bk_sb = singles.tile([128, NCH], F32, tag="bk")
        nc.sync.dma_start(out=bq_sb[:], in_=bq_d.ap().rearrange("(c p) -> p c", p=128))
        nc.sync.dma_start(out=bk_sb[:], in_=bk_d.ap().rearrange("(c p) -> p c", p=128))
    if o_bias:
        bo_sb = singles.tile([128, NCH], F32, tag="bo")
        nc.sync.dma_start(out=bo_sb[:], in_=bo_d.ap().rearrange("(c p) -> p c", p=128))

    hts, qts, kts, vts, onts, sc_sh = {}, {}, {}, {}, {}, {}

    def emit_gn(b):
        xt = xts[b]
        mv6 = st_pool.tile([128, NCH, 2, 6], F32, tag="mv6", name=f"mv6_{b}")
        mv = st_pool.tile([128, NCH, 2], F32, tag="mv", name=f"mv_{b}")
        for ci in range(NCH):
            for s in range(2):
                nc.vector.bn_stats(
                    out=mv6[:, ci, s, :], in_=xt[:, ci, s * 512:(s + 1) * 512]
                )
            nc.vector.bn_aggr(out=mv[:, ci, :], in_=mv6[:, ci, :, :])
        msq = st_pool.tile([128, NCH, 1], F32, tag="msq", name=f"msq_{b}")
        m2 = st_pool.tile([128, NCH, 1], F32, tag="m2", name=f"m2_{b}")
        nc.gpsimd.tensor_mul(msq[:], mv[:, :, 0:1], mv[:, :, 0:1])
        nc.gpsimd.tensor_add(m2[:], mv[:, :, 1:2], msq[:])
        gn_ps = ps_sm.tile([128, 16], F32, tag="acc", name=f"gn_ps_{b}")
        psum_g = gn_ps[0:GPC, 0:2 * NCH]
        psum_bc = gn_ps[:, 2 * NCH:4 * NCH]
        for ci in range(NCH):
            nc.tensor.matmul(
                psum_g[:, ci:ci + 1], maskg_sb[:], mv[:, ci, 0:1],
                start=True, stop=True,
            )
            nc.tensor.matmul(
                psum_g[:, NCH + ci:NCH + ci + 1], maskg_sb[:], m2[:, ci, :],
                start=True, stop=True,
            )
        gstat = st_pool.tile([GPC, 2 * NCH], F32, tag="gstat", name=f"gstat_{b}")
        nc.vector.tensor_scalar_mul(gstat[:], psum_g[:], 1.0 / 16.0)
        sqg = st_pool.tile([GPC, NCH], F32, tag="sqg", name=f"sqg_{b}")
        varg = st_pool.tile([GPC, NCH], F32, tag="varg", name=f"varg_{b}")
        nc.gpsimd.tensor_mul(sqg[:], gstat[:, 0:NCH], gstat[:, 0:NCH])
        nc.gpsimd.tensor_sub(varg[:], gstat[:, NCH:2 * NCH], sqg[:])
        lnv = st_pool.tile([GPC, NCH], F32, tag="lnv", name=f"lnv_{b}")
        nc.scalar.activation(lnv[:], varg[:], AF.Ln, bias=eps_sb[:])
        rstd = st_pool.tile([GPC, NCH], F32, tag="rstd", name=f"rstd_{b}")
        nc.scalar.activation(rstd[:], lnv[:], AF.Exp, scale=-0.5)
        for ci in range(NCH):
            nc.tensor.matmul(
                psum_bc[:, ci:ci + 1], bcn_sb[:], gstat[:, ci:ci + 1],
                start=True, stop=True,
            )
            nc.tensor.matmul(
                psum_bc[:, NCH + ci:NCH + ci + 1], bcp_sb[:], rstd[:, ci:ci + 1],
                start=True, stop=True,
            )
        bc = st_pool.tile([128, 2 * NCH], F32, tag="bc", name=f"bc_{b}")
        nc.vector.tensor_copy(bc[:], psum_bc[:])
        if trivial_gn:
            scale_t = bc[:, NCH:2 * NCH]
            shift_t = st_pool.tile([128, NCH], F32, tag="shift", name=f"shift_{b}")
            nc.gpsimd.tensor_mul(shift_t[:], bc[:, 0:NCH], bc[:, NCH:2 * NCH])
        else:
            scale_full = st_pool.tile([128, NCH], F32, tag="scalef", name=f"scf_{b}")
            nc.gpsimd.tensor_mul(scale_full[:], bc[:, NCH:2 * NCH], gnw_sb[:])
            tmp = st_pool.tile([128, NCH], F32, tag="tmpf", name=f"tmpf_{b}")
            nc.gpsimd.tensor_mul(tmp[:], bc[:, 0:NCH], scale_full[:])
            shift_t = st_pool.tile([128, NCH], F32, tag="shift", name=f"shift_{b}")
            nc.gpsimd.tensor_add(shift_t[:], tmp[:], gnb_sb[:])
            scale_t = scale_full
        sc_sh[b] = (scale_t, shift_t)

    def emit_h(b):
        scale_t, shift_t = sc_sh[b]
        xt = xts[b]
        ht = h_pool.tile([128, NCH, T], FP8, tag="h", name=f"ht_{b}")
        for ci in range(NCH):
            nc.scalar.activation(
                ht[:, ci, :], xt[:, ci, :], AF.Identity,
                bias=shift_t[:, ci:ci + 1], scale=scale_t[:, ci:ci + 1],
            )
        hts[b] = ht

    def emit_qkv(b):
        ht = hts[b]
        qt = q_pool.tile([128, NCH, T], FP8, tag="q", name=f"qt_{b}")
        kt = k_pool.tile([128, NCH, T], FP8, tag="k", name=f"kt_{b}")
        vt = v_pool.tile([128, NTK, C], FP8, tag="v", name=f"vt_{b}")
        for m in range(NCH):
            ps = ps_big.tile([128, T], F32, tag="big", name=f"ps_q{b}_{m}")
            for n2 in range(2):
                for j in range(NCH // 2):
                    nc.tensor.matmul(
                        ps[:, n2 * 512:(n2 + 1) * 512],
                        wsb["wqT"][j][:, :, m * 128:(m + 1) * 128],
                        ht[:, 2 * j:2 * j + 2, n2 * 512:(n2 + 1) * 512],
                        start=(j == 0), stop=(j == NCH // 2 - 1),
                        perf_mode=DR,
                    )
            if qk_bias:
                nc.scalar.activation(
                    qt[:, m, :], ps[:], AF.Identity,
                    bias=bq_sb[:, m:m + 1], scale=ds_sb[:, 0:1],
                )
            else:
                nc.scalar.mul(qt[:, m, :], ps[:], ds_sb[:, 0:1])
            ps = ps_big.tile([128, T], F32, tag="big", name=f"ps_k{b}_{m}")
            for n2 in range(2):
                for j in range(NCH // 2):
                    nc.tensor.matmul(
                        ps[:, n2 * 512:(n2 + 1) * 512],
                        wsb["wkT"][j][:, :, m * 128:(m + 1) * 128],
                        ht[:, 2 * j:2 * j + 2, n2 * 512:(n2 + 1) * 512],
                        start=(j == 0), stop=(j == NCH // 2 - 1),
                        perf_mode=DR,
                    )
            if qk_bias:
                nc.vector.tensor_scalar(
                    kt[:, m, :], ps[:], ds_sb[:, 1:2], bk_sb[:, m:m + 1],
                    op0=mybir.AluOpType.mult, op1=mybir.AluOpType.add,
                )
            else:
                nc.vector.tensor_scalar_mul(kt[:, m, :], ps[:], ds_sb[:, 1:2])
        for mp in range(NTK // 2):
            ps = ps_big.tile([128, T], F32, tag="big", name=f"ps_v{b}_{mp}")
            for half in range(2):
                m = 2 * mp + half
                for j in range(NCH // 2):
                    nc.tensor.matmul(
                        ps[:, half * 512:(half + 1) * 512],
                        ht[:, 2 * j:2 * j + 2, m * 128:(m + 1) * 128],
                        wsb["wvT"][j][:],
                        start=(j == 0), stop=(j == NCH // 2 - 1),
                        perf_mode=DR,
                    )
            nc.scalar.mul(vt[:, 2 * mp:2 * mp + 2, :], ps[:], ds_sb[:, 2:3])
        qts[b], kts[b], vts[b] = qt, kt, vt

    def emit_attn(b):
        qt, kt, vt = qts[b], kts[b], vts[b]
        et = e_pool.tile([128, NTK, T], FP8, tag="e", name=f"et_{b}")
        for tk in range(NTK):
            ps_s = ps_big.tile([128, T], F32, tag="big", name=f"ps_s{b}_{tk}")
            for n2 in range(2):
                for j in range(NCH // 2):
                    nc.tensor.matmul(
                        ps_s[:, n2 * 512:(n2 + 1) * 512],
                        kt[:, 2 * j:2 * j + 2, tk * 128:(tk + 1) * 128],
                        qt[:, 2 * j:2 * j + 2, n2 * 512:(n2 + 1) * 512],
                        start=(j == 0), stop=(j == NCH // 2 - 1),
                        perf_mode=DR,
                    )
            nc.scalar.activation(
                et[:, tk, :], ps_s[:], AF.Exp, scale=SM_SCALE, bias=ebias_sb[:]
            )
        ont = on_pool.tile([128, NCH, T], FP8, tag="on", name=f"ont_{b}")
        for n2 in range(2):
            tq = slice(n2 * 512, (n2 + 1) * 512)
            pc = ps_sm.tile([128, 512], F32, tag="acc", name=f"pc{b}_{n2}")
            for j in range(NTK // 2):
                nc.tensor.matmul(
                    pc[:], ones_sb[:], et[:, 2 * j:2 * j + 2, tq],
                    start=(j == 0), stop=(j == NTK // 2 - 1),
                    perf_mode=DR,
                )
            rec = scr_pool.tile([128, 512], F32, tag="rec", name=f"rec{b}_{n2}")
            nc.vector.reciprocal_approx_fast(out=rec[:], in_=pc[:])
            rec2 = scr_pool.tile([128, 2, 512], F32, tag="rec2", name=f"rec2{b}_{n2}")
            nc.gpsimd.tensor_copy(rec2[:, 0, :], rec[:])
            nc.gpsimd.tensor_copy(rec2[:, 1, :], rec[:])
            for mp in range(NCH // 2):
                po2 = ps_big.tile([128, T], F32, tag="big", name=f"po{b}_{n2}_{mp}")
                for j in range(NTK // 2):
                    for mi in range(2):
                        m = 2 * mp + mi
                        nc.tensor.matmul(
                            po2[:, mi * 512:(mi + 1) * 512],
                            vt[:, 2 * j:2 * j + 2, m * 128:(m + 1) * 128],
                            et[:, 2 * j:2 * j + 2, tq],
                            start=(j == 0), stop=(j == NTK // 2 - 1),
                            perf_mode=DR,
                        )
                nc.vector.tensor_mul(ont[:, 2 * mp:2 * mp + 2, tq], po2[:], rec2[:])
        onts[b] = ont

    def emit_proj(b):
        ont, xt = onts[b], xts[b]
        out_t = out_pool.tile([128, NCH, T], F32, tag="out", name=f"out_{b}")
        for m in range(NCH):
            ps = ps_big.tile([128, T], F32, tag="big", name=f"ps_p{b}_{m}")
            for n2 in range(2):
                for j in range(NCH // 2):
                    nc.tensor.matmul(
                        ps[:, n2 * 512:(n2 + 1) * 512],
                        wsb["wpT"][j][:, :, m * 128:(m + 1) * 128],
                        ont[:, 2 * j:2 * j + 2, n2 * 512:(n2 + 1) * 512],
                        start=(j == 0), stop=(j == NCH // 2 - 1),
                        perf_mode=DR,
                    )
            nc.vector.scalar_tensor_tensor(
                out_t[:, m, :], ps[:], ds_sb[:, 3:4], xt[:, m, :],
                op0=mybir.AluOpType.mult, op1=mybir.AluOpType.add,
            )
            if o_bias:
                nc.vector.tensor_scalar_add(
                    out_t[:, m, :], out_t[:, m, :], bo_sb[:, m:m + 1]
                )
        nc.sync.dma_start(out=y_ap[b], in_=out_t[:])

    emit_gn(0)
    emit_h(0)
    for b in range(BPC):
        emit_qkv(b)
        if b + 1 < BPC:
            load_x(b + 1)
            emit_gn(b + 1)
        emit_attn(b)
        if b + 1 < BPC:
            emit_h(b + 1)
        emit_proj(b)


def _build(flags):
    from contextlib import ExitStack

    nc = bacc.Bacc(
        "TRN2",
        target_bir_lowering=False,
        debug=False,
        enable_asserts=False,
        num_devices=NCORES,
    )
    with tile.TileContext(nc) as tc:
        with ExitStack() as ctx:
            _emit(nc, tc, ctx, flags)
    nc.compile()
    return nc


_CACHE = {}


def _consts():
    p = np.arange(128)
    maskg = (p[:, None] // 16 == np.arange(GPC)[None, :]).astype(np.float32)
    bcp = maskg.T.copy()
    return {
        "ones256": np.ones((128, 2, 128), mybir.dt.np(FP8)),
        "mask_g": maskg,
        "bc_pos": np.ascontiguousarray(bcp),
        "bc_neg": np.ascontiguousarray(-bcp),
    }


def kernel(x, gn_w, gn_b, wq, bq, wk, bk, wv, bv, wp, bp):
    x = np.ascontiguousarray(np.asarray(x, dtype=np.float32))
    B = x.shape[0]
    assert (B, x.shape[1], x.shape[2] * x.shape[3]) == (NCORES * BPC, C, T)
    H, W = x.shape[2], x.shape[3]
    xr = x.reshape(B, C, T)
    gn_w = np.asarray(gn_w, np.float32)
    gn_b = np.asarray(gn_b, np.float32)
    trivial_gn = bool(np.all(gn_w == 1.0) and np.all(gn_b == 0.0))
    qk_bias = bool(np.any(np.asarray(bq)) or np.any(np.asarray(bk)))
    o_bias = bool(np.any(np.asarray(bv)) or np.any(np.asarray(bp)))
    flags = (trivial_gn, qk_bias, o_bias)
    if flags not in _CACHE:
        _CACHE[flags] = _build(flags)
    nc = _CACHE[flags]

    common = dict(_consts())
    descale = np.empty(4, np.float32)
    fp8 = mybir.dt.np(FP8)
    for i, (n, w) in enumerate(
        (("wqT", wq), ("wkT", wk), ("wvT", wv), ("wpT", wp))
    ):
        wT = np.ascontiguousarray(np.asarray(w, np.float32).T)
        amax = float(np.abs(wT).max()) or 1.0
        k = int(np.floor(np.log2(88.0 / amax)))
        descale[i] = 2.0 ** (-k)
        ws = (wT * (2.0 ** k)).astype(fp8)
        common[n] = np.ascontiguousarray(
            ws.reshape(NCH // 2, 2, 128, C).transpose(0, 2, 1, 3)
        )
    common["descale"] = descale
    if not trivial_gn:
        common["gnw"] = gn_w
        common["gnb"] = gn_b
    if qk_bias:
        common["bq"] = np.asarray(bq, np.float32)
        common["bk"] = np.asarray(bk, np.float32)
    if o_bias:
        common["bias_o"] = (
            np.asarray(wp, np.float32) @ np.asarray(bv, np.float32)
            + np.asarray(bp, np.float32)
        ).astype(np.float32)

    in_maps = [
        {"x": np.ascontiguousarray(xr[c * BPC:(c + 1) * BPC]), **common}
        for c in range(NCORES)
    ]
    res = run_bass_kernel_spmd(nc, in_maps, core_ids=list(range(NCORES)))
    y = np.concatenate([res.results[c]["y"] for c in range(NCORES)], axis=0)
    return np.ascontiguousarray(y.reshape(B, C, H, W).astype(np.float32))


# revision 39
# speedup vs baseline: 2.3517x; 1.0625x over previous
import numpy as np
import ml_dtypes

import concourse.bacc as bacc
import concourse.bass as bass
import concourse.tile as tile
from concourse import mybir
from concourse.bass_utils import run_bass_kernel_spmd

F32 = mybir.dt.float32
BF16 = mybir.dt.bfloat16
FP8 = mybir.dt.float8e4
DR = mybir.MatmulPerfMode.DoubleRow
AF = mybir.ActivationFunctionType
EXP_BIAS = -3.5

NCORES = 8
BPC = 4
C = 512
T = 1024
NCH = 4
NTK = 8
GPC = 8
EPS = 1e-5
SM_SCALE = float(C) ** -0.5


def _emit(nc, tc, ctx, flags):
    trivial_gn, qk_bias, o_bias = flags

    x_d = nc.dram_tensor("x", (BPC, C, T), F32, kind="ExternalInput")
    y_d = nc.dram_tensor("y", (BPC, C, T), F32, kind="ExternalOutput")
    w_d = {
        n: nc.dram_tensor(n, (NCH // 2, 128, 2, C), FP8, kind="ExternalInput")
        for n in ("wqT", "wkT", "wvT", "wpT")
    }
    ds_d = nc.dram_tensor("descale", (4,), F32, kind="ExternalInput")
    ones_d = nc.dram_tensor("ones256", (128, 2, 128), FP8, kind="ExternalInput")
    maskg_d = nc.dram_tensor("mask_g", (128, GPC), F32, kind="ExternalInput")
    bcp_d = nc.dram_tensor("bc_pos", (GPC, 128), F32, kind="ExternalInput")
    bcn_d = nc.dram_tensor("bc_neg", (GPC, 128), F32, kind="ExternalInput")
    if not trivial_gn:
        gnw_d = nc.dram_tensor("gnw", (C,), F32, kind="ExternalInput")
        gnb_d = nc.dram_tensor("gnb", (C,), F32, kind="ExternalInput")
    if qk_bias:
        bq_d = nc.dram_tensor("bq", (C,), F32, kind="ExternalInput")
        bk_d = nc.dram_tensor("bk", (C,), F32, kind="ExternalInput")
    if o_bias:
        bo_d = nc.dram_tensor("bias_o", (C,), F32, kind="ExternalInput")

    from concourse.hw_specs import get_activation_tables

    tabs = list(get_activation_tables(nc.m.arch))
    lnexp_id = tabs.index("natural_log_exp_and_others")
    nc.scalar.add_instruction(
        mybir.InstLoadActFuncSet(
            name=nc.get_next_instruction_name(),
            ins=[],
            outs=[],
            act_func_set_id=lnexp_id,
        )
    )

    x_ap = x_d.ap().rearrange("b (c p) t -> b p c t", p=128)
    y_ap = y_d.ap().rearrange("b (c p) t -> b p c t", p=128)

    singles = ctx.enter_context(tc.tile_pool(name="singles", bufs=1))
    x_pool = ctx.enter_context(tc.tile_pool(name="x", bufs=3))
    h_pool = ctx.enter_context(tc.tile_pool(name="h", bufs=2))
    q_pool = ctx.enter_context(tc.tile_pool(name="q", bufs=2))
    k_pool = ctx.enter_context(tc.tile_pool(name="k", bufs=2))
    v_pool = ctx.enter_context(tc.tile_pool(name="v", bufs=2))
    e_pool = ctx.enter_context(tc.tile_pool(name="e", bufs=2))
    on_pool = ctx.enter_context(tc.tile_pool(name="on", bufs=2))
    out_pool = ctx.enter_context(tc.tile_pool(name="out", bufs=2))
    scr_pool = ctx.enter_context(tc.tile_pool(name="scr", bufs=2))
    st_pool = ctx.enter_context(tc.tile_pool(name="st", bufs=2))
    ps_big = ctx.enter_context(tc.tile_pool(name="ps_big", bufs=3, space="PSUM"))
    ps_sm = ctx.enter_context(tc.tile_pool(name="ps_sm", bufs=2, space="PSUM"))

    xts = {}

    def load_x(b):
        xt = x_pool.tile([128, NCH, T], F32, tag="x", name=f"xt{b}")
        for ci in range(NCH):
            nc.sync.dma_start(out=xt[:, ci, :], in_=x_ap[b][:, ci, :])
        xts[b] = xt

    load_x(0)

    wsb = {}
    for n in ("wqT", "wkT", "wvT", "wpT"):
        tiles = []
        for j in range(NCH // 2):
            wt = singles.tile([128, 2, C], FP8, tag=f"{n}{j}")
            nc.sync.dma_start(out=wt[:], in_=w_d[n].ap()[j])
            tiles.append(wt)
        wsb[n] = tiles
    ds_sb = singles.tile([128, 4], F32, tag="descale")
    for i in range(4):
        nc.sync.dma_start(
            out=ds_sb[:, i:i + 1],
            in_=bass.AP(tensor=ds_d, offset=i, ap=[[0, 128], [1, 1]]),
        )
    ones_sb = singles.tile([128, 2, 128], FP8, tag="ones")
    nc.sync.dma_start(out=ones_sb[:], in_=ones_d.ap())
    maskg_sb = singles.tile([128, GPC], F32, tag="maskg")
    nc.sync.dma_start(out=maskg_sb[:], in_=maskg_d.ap())
    bcp_sb = singles.tile([GPC, 128], F32, tag="bcp")
    nc.sync.dma_start(out=bcp_sb[:], in_=bcp_d.ap())
    bcn_sb = singles.tile([GPC, 128], F32, tag="bcn")
    nc.sync.dma_start(out=bcn_sb[:], in_=bcn_d.ap())
    eps_sb = singles.tile([GPC, 1], F32, tag="eps")
    nc.vector.memset(eps_sb[:], EPS)
    ebias_sb = singles.tile([128, 1], F32, tag="ebias")
    nc.vector.memset(ebias_sb[:], EXP_BIAS)
    if not trivial_gn:
        gnw_sb = singles.tile([128, NCH], F32, tag="gnw")
        gnb_sb = singles.tile([128, NCH], F32, tag="gnb")
        nc.sync.dma_start(out=gnw_sb[:], in_=gnw_d.ap().rearrange("(c p) -> p c", p=128))
        nc.sync.dma_start(out=gnb_sb[:], in_=gnb_d.ap().rearrange("(c p) -> p c", p=128))
    if qk_bias:
        bq_sb = singles.tile([128, NCH], F32, tag="bq")
        bk_sb = singles.tile([128, NCH], F32, tag="bk")
        nc.sync.dma_start(out=bq_sb[:], in_=bq_d.ap().rearrange("(c p) -> p c", p=128))
        nc.sync.dma_start(out=bk_sb[:], in_=bk_d.ap().rearrange("(c p) -> p c", p=128))
    if o_bias:
        bo_sb = singles.tile([128, NCH], F32, tag="bo")
        nc.sync.dma_start(out=bo_sb[:], in_=bo_d.ap().rearrange("(c p) -> p c", p=128))

    hts, qts, kts, vts, onts, sc_sh = {}, {}, {}, {}, {}, {}

    def emit_gn(b):
        xt = xts[b]
        mv6 = st_pool.tile([128, NCH, 2, 6], F32, tag="mv6", name=f"mv6_{b}")
        mv = st_pool.tile([128, NCH, 2], F32, tag="mv", name=f"mv_{b}")
        for ci in range(NCH):
            for s in range(2):
                nc.vector.bn_stats(
                    out=mv6[:, ci, s, :], in_=xt[:, ci, s * 512:(s + 1) * 512]
                )
            nc.vector.bn_aggr(out=mv[:, ci, :], in_=mv6[:, ci, :, :])
        msq = st_pool.tile([128, NCH, 1], F32, tag="msq", name=f"msq_{b}")
        m2 = st_pool.tile([128, NCH, 1], F32, tag="m2", name=f"m2_{b}")
        nc.gpsimd.tensor_mul(msq[:], mv[:, :, 0:1], mv[:, :, 0:1])
        nc.gpsimd.tensor_add(m2[:], mv[:, :, 1:2], msq[:])
        gn_ps = ps_sm.tile([128, 16], F32, tag="acc", name=f"gn_ps_{b}")
        psum_g = gn_ps[0:GPC, 0:2 * NCH]
        psum_bc = gn_ps[:, 2 * NCH:4 * NCH]
        for ci in range(NCH):
            nc.tensor.matmul(
                psum_g[:, ci:ci + 1], maskg_sb[:], mv[:, ci, 0:1],
                start=True, stop=True,
            )
            nc.tensor.matmul(
                psum_g[:, NCH + ci:NCH + ci + 1], maskg_sb[:], m2[:, ci, :],
                start=True, stop=True,
            )
        gstat = st_pool.tile([GPC, 2 * NCH], F32, tag="gstat", name=f"gstat_{b}")
        nc.vector.tensor_scalar_mul(gstat[:], psum_g[:], 1.0 / 16.0)
        sqg = st_pool.tile([GPC, NCH], F32, tag="sqg", name=f"sqg_{b}")
        varg = st_pool.tile([GPC, NCH], F32, tag="varg", name=f"varg_{b}")
        nc.gpsimd.tensor_mul(sqg[:], gstat[:, 0:NCH], gstat[:, 0:NCH])
        nc.gpsimd.tensor_sub(varg[:], gstat[:, NCH:2 * NCH], sqg[:])
        lnv = st_pool.tile([GPC, NCH], F32, tag="lnv", name=f"lnv_{b}")
        nc.scalar.activation(lnv[:], varg[:], AF.Ln, bias=eps_sb[:])
        rstd = st_pool.tile([GPC, NCH], F32, tag="rstd", name=f"rstd_{b}")
        nc.scalar.activation(rstd[:], lnv[:], AF.Exp, scale=-0.5)
        for ci in range(NCH):
            nc.tensor.matmul(
                psum_bc[:, ci:ci + 1], bcn_sb[:], gstat[:, ci:ci + 1],
                start=True, stop=True,
            )
            nc.tensor.matmul(
                psum_bc[:, NCH + ci:NCH + ci + 1], bcp_sb[:], rstd[:, ci:ci + 1],
                start=True, stop=True,
            )
        bc = st_pool.tile([128, 2 * NCH], F32, tag="bc", name=f"bc_{b}")
        nc.vector.tensor_copy(bc[:], psum_bc[:])
        if trivial_gn:
            scale_t = bc[:, NCH:2 * NCH]
            shift_t = st_pool.tile([128, NCH], F32, tag="shift", name=f"shift_{b}")
            nc.gpsimd.tensor_mul(shift_t[:], bc[:, 0:NCH], bc[:, NCH:2 * NCH])
        else:
            scale_full = st_pool.tile([128, NCH], F32, tag="scalef", name=f"scf_{b}")
            nc.gpsimd.tensor_mul(scale_full[:], bc[:, NCH:2 * NCH], gnw_sb[:])
            tmp = st_pool.tile([128, NCH], F32, tag="tmpf", name=f"tmpf_{b}")
            nc.gpsimd.tensor_mul(tmp[:], bc[:, 0:NCH], scale_full[:])
            shift_t = st_pool.tile([128, NCH], F32, tag="shift", name=f"shift_{b}")
            nc.gpsimd.tensor_add(shift_t[:], tmp[:], gnb_sb[:])
            scale_t = scale_full
        sc_sh[b] = (scale_t, shift_t)

    def emit_h(b):
        scale_t, shift_t = sc_sh[b]
        xt = xts[b]
        ht = h_pool.tile([128, NCH, T], FP8, tag="h", name=f"ht_{b}")
        for ci in range(NCH):
            nc.scalar.activation(
                ht[:, ci, :], xt[:, ci, :], AF.Identity,
                bias=shift_t[:, ci:ci + 1], scale=scale_t[:, ci:ci + 1],
            )
        hts[b] = ht

    def emit_qkv(b):
        ht = hts[b]
        qt = q_pool.tile([128, NCH, T], FP8, tag="q", name=f"qt_{b}")
        kt = k_pool.tile([128, NCH, T], FP8, tag="k", name=f"kt_{b}")
        vt = v_pool.tile([128, NTK, C], FP8, tag="v", name=f"vt_{b}")
        for m in range(NCH):
            ps = ps_big.tile([128, T], F32, tag="big", name=f"ps_q{b}_{m}")
            for n2 in range(2):
                for j in range(NCH // 2):
                    nc.tensor.matmul(
                        ps[:, n2 * 512:(n2 + 1) * 512],
                        wsb["wqT"][j][:, :, m * 128:(m + 1) * 128],
                        ht[:, 2 * j:2 * j + 2, n2 * 512:(n2 + 1) * 512],
                        start=(j == 0), stop=(j == NCH // 2 - 1),
                        perf_mode=DR,
                    )
            if qk_bias:
                nc.scalar.activation(
                    qt[:, m, :], ps[:], AF.Identity,
                    bias=bq_sb[:, m:m + 1], scale=ds_sb[:, 0:1],
                )
            else:
                nc.scalar.mul(qt[:, m, :], ps[:], ds_sb[:, 0:1])
            ps = ps_big.tile([128, T], F32, tag="big", name=f"ps_k{b}_{m}")
            for n2 in range(2):
                for j in range(NCH // 2):
                    nc.tensor.matmul(
                        ps[:, n2 * 512:(n2 + 1) * 512],
                        wsb["wkT"][j][:, :, m * 128:(m + 1) * 128],
                        ht[:, 2 * j:2 * j + 2, n2 * 512:(n2 + 1) * 512],
                        start=(j == 0), stop=(j == NCH // 2 - 1),
                        perf_mode=DR,
                    )
            if qk_bias:
                nc.vector.tensor_scalar(
                    kt[:, m, :], ps[:], ds_sb[:, 1:2], bk_sb[:, m:m + 1],
                    op0=mybir.AluOpType.mult, op1=mybir.AluOpType.add,
                )
            else:
                nc.vector.tensor_scalar_mul(kt[:, m, :], ps[:], ds_sb[:, 1:2])
        for mp in range(NTK // 2):
            ps = ps_big.tile([128, T], F32, tag="big", name=f"ps_v{b}_{mp}")
            for half in range(2):
                m = 2 * mp + half
                for j in range(NCH // 2):
                    nc.tensor.matmul(
                        ps[:, half * 512:(half + 1) * 512],
                        ht[:, 2 * j:2 * j + 2, m * 128:(m + 1) * 128],
                        wsb["wvT"][j][:],
                        start=(j == 0), stop=(j == NCH // 2 - 1),
                        perf_mode=DR,
                    )
            nc.scalar.mul(vt[:, 2 * mp:2 * mp + 2, :], ps[:], ds_sb[:, 2:3])
        qts[b], kts[b], vts[b] = qt, kt, vt

    def emit_attn(b):
        qt, kt, vt = qts[b], kts[b], vts[b]
        et = e_pool.tile([128, NTK, T], FP8, tag="e", name=f"et_{b}")
        for tk in range(NTK):
            ps_s = ps_big.tile([128, T], F32, tag="big", name=f"ps_s{b}_{tk}")
            for n2 in range(2):
                for j in range(NCH // 2):
                    nc.tensor.matmul(
                        ps_s[:, n2 * 512:(n2 + 1) * 512],
                        kt[:, 2 * j:2 * j + 2, tk * 128:(tk + 1) * 128],
                        qt[:, 2 * j:2 * j + 2, n2 * 512:(n2 + 1) * 512],
                        start=(j == 0), stop=(j == NCH // 2 - 1),
                        perf_mode=DR,
                    )
            nc.scalar.activation(
                et[:, tk, :], ps_s[:], AF.Exp, scale=SM_SCALE, bias=ebias_sb[:]
            )
        ont = on_pool.tile([128, NCH, T], FP8, tag="on", name=f"ont_{b}")
        for n2 in range(2):
            tq = slice(n2 * 512, (n2 + 1) * 512)
            pc = ps_sm.tile([128, 512], F32, tag="acc", name=f"pc{b}_{n2}")
            for j in range(NTK // 2):
                nc.tensor.matmul(
                    pc[:], ones_sb[:], et[:, 2 * j:2 * j + 2, tq],
                    start=(j == 0), stop=(j == NTK // 2 - 1),
                    perf_mode=DR,
                )
            rec = scr_pool.tile([128, 512], F32, tag="rec", name=f"rec{b}_{n2}")
            nc.vector.reciprocal_approx_fast(out=rec[:], in_=pc[:])
            rec2 = scr_pool.tile([128, 2, 512], F32, tag="rec2", name=f"rec2{b}_{n2}")
            nc.gpsimd.tensor_copy(rec2[:, 0, :], rec[:])
            nc.gpsimd.tensor_copy(rec2[:, 1, :], rec[:])
            for mp in range(NCH // 2):
                po2 = ps_big.tile([128, T], F32, tag="big", name=f"po{b}_{n2}_{mp}")
                for j in range(NTK // 2):
                    for mi in range(2):
                        m = 2 * mp + mi
                        nc.tensor.matmul(
                            po2[:, mi * 512:(mi + 1) * 512],
                            vt[:, 2 * j:2 * j + 2, m * 128:(m + 1) * 128],
                            et[:, 2 * j:2 * j + 2, tq],
                            start=(j == 0), stop=(j == NTK // 2 - 1),
                            perf_mode=DR,
                        )
                nc.vector.tensor_mul(ont[:, 2 * mp:2 * mp + 2, tq], po2[:], rec2[:])
        onts[b] = ont

    def emit_proj(b):
        ont, xt = onts[b], xts[b]
        out_t = out_pool.tile([128, NCH, T], F32, tag="out", name=f"out_{b}")
        for m in range(NCH):
            ps = ps_big.tile([128, T], F32, tag="big", name=f"ps_p{b}_{m}")
            for n2 in range(2):
                for j in range(NCH // 2):
                    nc.tensor.matmul(
                        ps[:, n2 * 512:(n2 + 1) * 512],
                        wsb["wpT"][j][:, :, m * 128:(m + 1) * 128],
                        ont[:, 2 * j:2 * j + 2, n2 * 512:(n2 + 1) * 512],
                        start=(j == 0), stop=(j == NCH // 2 - 1),
                        perf_mode=DR,
                    )
            nc.vector.scalar_tensor_tensor(
                out_t[:, m, :], ps[:], ds_sb[:, 3:4], xt[:, m, :],
                op0=mybir.AluOpType.mult, op1=mybir.AluOpType.add,
            )
            if o_bias:
                nc.vector.tensor_scalar_add(
                    out_t[:, m, :], out_t[:, m, :], bo_sb[:, m:m + 1]
                )
        nc.sync.dma_start(out=y_ap[b], in_=out_t[:])

    emit_gn(0)
    emit_h(0)
    for b in range(BPC):
        emit_qkv(b)
        if b >= 1:
            emit_proj(b - 1)
        if b + 1 < BPC:
            load_x(b + 1)
            emit_gn(b + 1)
        emit_attn(b)
        if b + 1 < BPC:
            emit_h(b + 1)
    emit_proj(BPC - 1)


def _build(flags):
    from contextlib import ExitStack

    nc = bacc.Bacc(
        "TRN2",
        target_bir_lowering=False,
        debug=False,
        enable_asserts=False,
        num_devices=NCORES,
    )
    with tile.TileContext(nc) as tc:
        with ExitStack() as ctx:
            _emit(nc, tc, ctx, flags)
    nc.compile()
    return nc


_CACHE = {}


def _consts():
    p = np.arange(128)
    maskg = (p[:, None] // 16 == np.arange(GPC)[None, :]).astype(np.float32)
    bcp = maskg.T.copy()
    return {
        "ones256": np.ones((128, 2, 128), mybir.dt.np(FP8)),
        "mask_g": maskg,
        "bc_pos": np.ascontiguousarray(bcp),
        "bc_neg": np.ascontiguousarray(-bcp),
    }


def kernel(x, gn_w, gn_b, wq, bq, wk, bk, wv, bv, wp, bp):
    x = np.ascontiguousarray(np.asarray(x, dtype=np.float32))
    B = x.shape[0]
    assert (B, x.shape[1], x.shape[2] * x.shape[3]) == (NCORES * BPC, C, T)
    H, W = x.shape[2], x.shape[3]
    xr = x.reshape(B, C, T)
    gn_w = np.asarray(gn_w, np.float32)
    gn_b = np.asarray(gn_b, np.float32)
    trivial_gn = bool(np.all(gn_w == 1.0) and np.all(gn_b == 0.0))
    qk_bias = bool(np.any(np.asarray(bq)) or np.any(np.asarray(bk)))
    o_bias = bool(np.any(np.asarray(bv)) or np.any(np.asarray(bp)))
    flags = (trivial_gn, qk_bias, o_bias)
    if flags not in _CACHE:
        _CACHE[flags] = _build(flags)
    nc = _CACHE[flags]

    common = dict(_consts())
    descale = np.empty(4, np.float32)
    fp8 = mybir.dt.np(FP8)
    for i, (n, w) in enumerate(
        (("wqT", wq), ("wkT", wk), ("wvT", wv), ("wpT", wp))
    ):
        wT = np.ascontiguousarray(np.asarray(w, np.float32).T)
        amax = float(np.abs(wT).max()) or 1.0
        k = int(np.floor(np.log2(88.0 / amax)))
        descale[i] = 2.0 ** (-k)
        ws = (wT * (2.0 ** k)).astype(fp8)
        common[n] = np.ascontiguousarray(
            ws.reshape(NCH // 2, 2, 128, C).transpose(0, 2, 1, 3)
        )
    common["descale"] = descale
    if not trivial_gn:
        common["gnw"] = gn_w
        common["gnb"] = gn_b
    if qk_bias:
        common["bq"] = np.asarray(bq, np.float32)
        common["bk"] = np.asarray(bk, np.float32)
    if o_bias:
        common["bias_o"] = (
            np.asarray(wp, np.float32) @ np.asarray(bv, np.float32)
            + np.asarray(bp, np.float32)
        ).astype(np.float32)

    in_maps = [
        {"x": np.ascontiguousarray(xr[c * BPC:(c + 1) * BPC]), **common}
        for c in range(NCORES)
    ]
    res = run_bass_kernel_spmd(nc, in_maps, core_ids=list(range(NCORES)))
    y = np.concatenate([res.results[c]["y"] for c in range(NCORES)], axis=0)
    return np.ascontiguousarray(y.reshape(B, C, H, W).astype(np.float32))


# revision 57
# speedup vs baseline: 2.5886x; 1.1007x over previous
import numpy as np
import ml_dtypes

import concourse.bacc as bacc
import concourse.bass as bass
import concourse.tile as tile
from concourse import mybir
from concourse.bass_utils import run_bass_kernel_spmd

F32 = mybir.dt.float32
BF16 = mybir.dt.bfloat16
FP8 = mybir.dt.float8e4
DR = mybir.MatmulPerfMode.DoubleRow
AF = mybir.ActivationFunctionType
EXP_BIAS = -3.5

NCORES = 8
BPC = 4
C = 512
T = 1024
NCH = 4
NTK = 8
GPC = 8
EPS = 1e-5
SM_SCALE = float(C) ** -0.5


def _emit(nc, tc, ctx, flags):
    trivial_gn, qk_bias, o_bias = flags

    x_d = nc.dram_tensor("x", (BPC, C, T), F32, kind="ExternalInput")
    y_d = nc.dram_tensor("y", (BPC, C, T), F32, kind="ExternalOutput")
    w_d = {
        n: nc.dram_tensor(n, (NCH // 2, 128, 2, C), FP8, kind="ExternalInput")
        for n in ("wqT", "wkT", "wvT", "wpT")
    }
    ds_d = nc.dram_tensor("descale", (128, 4), F32, kind="ExternalInput")
    ones_d = nc.dram_tensor("ones256", (128, 2, 128), FP8, kind="ExternalInput")
    maskg_d = nc.dram_tensor("mask_g", (128, GPC), F32, kind="ExternalInput")
    bcp_d = nc.dram_tensor("bc_pos", (GPC, 128), F32, kind="ExternalInput")
    bcn_d = nc.dram_tensor("bc_neg", (GPC, 128), F32, kind="ExternalInput")
    if not trivial_gn:
        gnw_d = nc.dram_tensor("gnw", (C,), F32, kind="ExternalInput")
        gnb_d = nc.dram_tensor("gnb", (C,), F32, kind="ExternalInput")
    if qk_bias:
        bq_d = nc.dram_tensor("bq", (C,), F32, kind="ExternalInput")
        bk_d = nc.dram_tensor("bk", (C,), F32, kind="ExternalInput")
    if o_bias:
        bo_d = nc.dram_tensor("bias_o", (C,), F32, kind="ExternalInput")

    from concourse.hw_specs import get_activation_tables

    tabs = list(get_activation_tables(nc.m.arch))
    lnexp_id = tabs.index("natural_log_exp_and_others")
    nc.scalar.add_instruction(
        mybir.InstLoadActFuncSet(
            name=nc.get_next_instruction_name(),
            ins=[],
            outs=[],
            act_func_set_id=lnexp_id,
        )
    )

    x_ap = x_d.ap().rearrange("b (c p) t -> b p c t", p=128)
    y_ap = y_d.ap().rearrange("b (c p) t -> b p c t", p=128)

    singles = ctx.enter_context(tc.tile_pool(name="singles", bufs=1))
    x_pool = ctx.enter_context(tc.tile_pool(name="x", bufs=3))
    h_pool = ctx.enter_context(tc.tile_pool(name="h", bufs=2))
    q_pool = ctx.enter_context(tc.tile_pool(name="q", bufs=2))
    k_pool = ctx.enter_context(tc.tile_pool(name="k", bufs=2))
    v_pool = ctx.enter_context(tc.tile_pool(name="v", bufs=2))
    e_pool = ctx.enter_context(tc.tile_pool(name="e", bufs=2))
    on_pool = ctx.enter_context(tc.tile_pool(name="on", bufs=2))
    out_pool = ctx.enter_context(tc.tile_pool(name="out", bufs=2))
    scr_pool = ctx.enter_context(tc.tile_pool(name="scr", bufs=2))
    st_pool = ctx.enter_context(tc.tile_pool(name="st", bufs=2))
    ps_big = ctx.enter_context(tc.tile_pool(name="ps_big", bufs=3, space="PSUM"))
    ps_sm = ctx.enter_context(tc.tile_pool(name="ps_sm", bufs=2, space="PSUM"))

    xts = {}

    def load_x(b):
        xt = x_pool.tile([128, NCH, T], F32, tag="x", name=f"xt{b}")
        for ci in range(NCH):
            nc.sync.dma_start(out=xt[:, ci, :], in_=x_ap[b][:, ci, :])
        xts[b] = xt

    load_x(0)

    maskg_sb = singles.tile([128, GPC], F32, tag="maskg")
    nc.sync.dma_start(out=maskg_sb[:], in_=maskg_d.ap())
    bcp_sb = singles.tile([GPC, 128], F32, tag="bcp")
    nc.sync.dma_start(out=bcp_sb[:], in_=bcp_d.ap())
    bcn_sb = singles.tile([GPC, 128], F32, tag="bcn")
    nc.sync.dma_start(out=bcn_sb[:], in_=bcn_d.ap())
    ds_sb = singles.tile([128, 4], F32, tag="descale")
    nc.sync.dma_start(out=ds_sb[:], in_=ds_d.ap())
    eps_sb = singles.tile([GPC, 1], F32, tag="eps")
    nc.vector.memset(eps_sb[:], EPS)
    ebias_sb = singles.tile([128, 1], F32, tag="ebias")
    nc.vector.memset(ebias_sb[:], EXP_BIAS)
    ones_sb = singles.tile([128, 2, 128], FP8, tag="ones")
    nc.sync.dma_start(out=ones_sb[:], in_=ones_d.ap())
    wsb = {}
    for n in ("wqT", "wkT", "wvT", "wpT"):
        tiles = []
        for j in range(NCH // 2):
            wt = singles.tile([128, 2, C], FP8, tag=f"{n}{j}")
            nc.sync.dma_start(out=wt[:], in_=w_d[n].ap()[j])
            tiles.append(wt)
        wsb[n] = tiles
    if not trivial_gn:
        gnw_sb = singles.tile([128, NCH], F32, tag="gnw")
        gnb_sb = singles.tile([128, NCH], F32, tag="gnb")
        nc.sync.dma_start(out=gnw_sb[:], in_=gnw_d.ap().rearrange("(c p) -> p c", p=128))
        nc.sync.dma_start(out=gnb_sb[:], in_=gnb_d.ap().rearrange("(c p) -> p c", p=128))
    if qk_bias:
        bq_sb = singles.tile([128, NCH], F32, tag="bq")
        bk_sb = singles.tile([128, NCH], F32, tag="bk")
        nc.sync.dma_start(out=bq_sb[:], in_=bq_d.ap().rearrange("(c p) -> p c", p=128))
        nc.sync.dma_start(out=bk_sb[:], in_=bk_d.ap().rearrange("(c p) -> p c", p=128))
    if o_bias:
        bo_sb = singles.tile([128, NCH], F32, tag="bo")
        nc.sync.dma_start(out=bo_sb[:], in_=bo_d.ap().rearrange("(c p) -> p c", p=128))

    hts, qts, kts, vts, onts, sc_sh = {}, {}, {}, {}, {}, {}

    def emit_gn(b):
        eng = nc.gpsimd
        xt = xts[b]
        mv6 = st_pool.tile([128, NCH, 2, 6], F32, tag="mv6", name=f"mv6_{b}")
        mv = st_pool.tile([128, NCH, 2], F32, tag="mv", name=f"mv_{b}")
        for ci in range(NCH):
            for s in range(2):
                nc.vector.bn_stats(
                    out=mv6[:, ci, s, :], in_=xt[:, ci, s * 512:(s + 1) * 512]
                )
            nc.vector.bn_aggr(out=mv[:, ci, :], in_=mv6[:, ci, :, :])
        msq = st_pool.tile([128, NCH, 1], F32, tag="msq", name=f"msq_{b}")
        m2 = st_pool.tile([128, NCH, 1], F32, tag="m2", name=f"m2_{b}")
        eng.tensor_mul(msq[:], mv[:, :, 0:1], mv[:, :, 0:1])
        eng.tensor_add(m2[:], mv[:, :, 1:2], msq[:])
        gn_ps = ps_sm.tile([128, 16], F32, tag="acc", name=f"gn_ps_{b}")
        psum_g = gn_ps[0:GPC, 0:2 * NCH]
        psum_bc = gn_ps[:, 2 * NCH:4 * NCH]
        for ci in range(NCH):
            nc.tensor.matmul(
                psum_g[:, ci:ci + 1], maskg_sb[:], mv[:, ci, 0:1],
                start=True, stop=True,
            )
            nc.tensor.matmul(
                psum_g[:, NCH + ci:NCH + ci + 1], maskg_sb[:], m2[:, ci, :],
                start=True, stop=True,
            )
        gstat = st_pool.tile([GPC, 2 * NCH], F32, tag="gstat", name=f"gstat_{b}")
        nc.vector.tensor_scalar_mul(gstat[:], psum_g[:], 1.0 / 16.0)
        sqg = st_pool.tile([GPC, NCH], F32, tag="sqg", name=f"sqg_{b}")
        varg = st_pool.tile([GPC, NCH], F32, tag="varg", name=f"varg_{b}")
        eng.tensor_mul(sqg[:], gstat[:, 0:NCH], gstat[:, 0:NCH])
        eng.tensor_sub(varg[:], gstat[:, NCH:2 * NCH], sqg[:])
        lnv = st_pool.tile([GPC, NCH], F32, tag="lnv", name=f"lnv_{b}")
        nc.scalar.activation(lnv[:], varg[:], AF.Ln, bias=eps_sb[:])
        rstd = st_pool.tile([GPC, NCH], F32, tag="rstd", name=f"rstd_{b}")
        nc.scalar.activation(rstd[:], lnv[:], AF.Exp, scale=-0.5)
        for ci in range(NCH):
            nc.tensor.matmul(
                psum_bc[:, ci:ci + 1], bcn_sb[:], gstat[:, ci:ci + 1],
                start=True, stop=True,
            )
            nc.tensor.matmul(
                psum_bc[:, NCH + ci:NCH + ci + 1], bcp_sb[:], rstd[:, ci:ci + 1],
                start=True, stop=True,
            )
        bc = st_pool.tile([128, 2 * NCH], F32, tag="bc", name=f"bc_{b}")
        nc.vector.tensor_copy(bc[:], psum_bc[:])
        if trivial_gn:
            scale_t = bc[:, NCH:2 * NCH]
            shift_t = st_pool.tile([128, NCH], F32, tag="shift", name=f"shift_{b}")
            eng.tensor_mul(shift_t[:], bc[:, 0:NCH], bc[:, NCH:2 * NCH])
        else:
            scale_full = st_pool.tile([128, NCH], F32, tag="scalef", name=f"scf_{b}")
            eng.tensor_mul(scale_full[:], bc[:, NCH:2 * NCH], gnw_sb[:])
            tmp = st_pool.tile([128, NCH], F32, tag="tmpf", name=f"tmpf_{b}")
            eng.tensor_mul(tmp[:], bc[:, 0:NCH], scale_full[:])
            shift_t = st_pool.tile([128, NCH], F32, tag="shift", name=f"shift_{b}")
            eng.tensor_add(shift_t[:], tmp[:], gnb_sb[:])
            scale_t = scale_full
        sc_sh[b] = (scale_t, shift_t)

    def emit_h(b):
        scale_t, shift_t = sc_sh[b]
        xt = xts[b]
        ht = h_pool.tile([128, NCH, T], FP8, tag="h", name=f"ht_{b}")
        for ci in range(NCH):
            if ci % 2 == 0:
                nc.scalar.activation(
                    ht[:, ci, :], xt[:, ci, :], AF.Identity,
                    bias=shift_t[:, ci:ci + 1], scale=scale_t[:, ci:ci + 1],
                )
            else:
                nc.vector.tensor_scalar(
                    ht[:, ci, :], xt[:, ci, :],
                    scale_t[:, ci:ci + 1], shift_t[:, ci:ci + 1],
                    op0=mybir.AluOpType.mult, op1=mybir.AluOpType.add,
                )
        hts[b] = ht

    def emit_qkv(b):
        ht = hts[b]
        qt = q_pool.tile([128, NCH, T], FP8, tag="q", name=f"qt_{b}")
        kt = k_pool.tile([128, NCH, T], FP8, tag="k", name=f"kt_{b}")
        vt = v_pool.tile([128, NTK, C], FP8, tag="v", name=f"vt_{b}")
        for m in range(NCH):
            ps = ps_big.tile([128, T], F32, tag="big", name=f"ps_q{b}_{m}")
            for n2 in range(2):
                for j in range(NCH // 2):
                    nc.tensor.matmul(
                        ps[:, n2 * 512:(n2 + 1) * 512],
                        wsb["wqT"][j][:, :, m * 128:(m + 1) * 128],
                        ht[:, 2 * j:2 * j + 2, n2 * 512:(n2 + 1) * 512],
                        start=(j == 0), stop=(j == NCH // 2 - 1),
                        perf_mode=DR,
                    )
            if qk_bias:
                nc.scalar.activation(
                    qt[:, m, :], ps[:], AF.Identity,
                    bias=bq_sb[:, m:m + 1], scale=ds_sb[:, 0:1],
                )
            else:
                nc.scalar.mul(qt[:, m, :], ps[:], ds_sb[:, 0:1])
            ps = ps_big.tile([128, T], F32, tag="big", name=f"ps_k{b}_{m}")
            for n2 in range(2):
                for j in range(NCH // 2):
                    nc.tensor.matmul(
                        ps[:, n2 * 512:(n2 + 1) * 512],
                        wsb["wkT"][j][:, :, m * 128:(m + 1) * 128],
                        ht[:, 2 * j:2 * j + 2, n2 * 512:(n2 + 1) * 512],
                        start=(j == 0), stop=(j == NCH // 2 - 1),
                        perf_mode=DR,
                    )
            if qk_bias:
                nc.vector.tensor_scalar(
                    kt[:, m, :], ps[:], ds_sb[:, 1:2], bk_sb[:, m:m + 1],
                    op0=mybir.AluOpType.mult, op1=mybir.AluOpType.add,
                )
            else:
                nc.vector.tensor_scalar_mul(kt[:, m, :], ps[:], ds_sb[:, 1:2])
        for mp in range(NTK // 2):
            ps = ps_big.tile([128, T], F32, tag="big", name=f"ps_v{b}_{mp}")
            for half in range(2):
                m = 2 * mp + half
                for j in range(NCH // 2):
                    nc.tensor.matmul(
                        ps[:, half * 512:(half + 1) * 512],
                        ht[:, 2 * j:2 * j + 2, m * 128:(m + 1) * 128],
                        wsb["wvT"][j][:],
                        start=(j == 0), stop=(j == NCH // 2 - 1),
                        perf_mode=DR,
                    )
            if mp % 2 == 0:
                nc.scalar.mul(vt[:, 2 * mp:2 * mp + 2, :], ps[:], ds_sb[:, 2:3])
            else:
                nc.vector.tensor_scalar_mul(
                    vt[:, 2 * mp:2 * mp + 2, :], ps[:], ds_sb[:, 2:3]
                )
        qts[b], kts[b], vts[b] = qt, kt, vt

    def emit_attn(b):
        qt, kt, vt = qts[b], kts[b], vts[b]
        et = e_pool.tile([128, NTK, T], FP8, tag="e", name=f"et_{b}")
        for tk in range(NTK):
            ps_s = ps_big.tile([128, T], F32, tag="big", name=f"ps_s{b}_{tk}")
            for n2 in range(2):
                for j in range(NCH // 2):
                    nc.tensor.matmul(
                        ps_s[:, n2 * 512:(n2 + 1) * 512],
                        kt[:, 2 * j:2 * j + 2, tk * 128:(tk + 1) * 128],
                        qt[:, 2 * j:2 * j + 2, n2 * 512:(n2 + 1) * 512],
                        start=(j == 0), stop=(j == NCH // 2 - 1),
                        perf_mode=DR,
                    )
            nc.scalar.activation(
                et[:, tk, :], ps_s[:], AF.Exp, scale=SM_SCALE, bias=ebias_sb[:]
            )
        ont = on_pool.tile([128, NCH, T], FP8, tag="on", name=f"ont_{b}")
        for n2 in range(2):
            tq = slice(n2 * 512, (n2 + 1) * 512)
            pc = ps_sm.tile([128, 512], F32, tag="acc", name=f"pc{b}_{n2}")
            for j in range(NTK // 2):
                nc.tensor.matmul(
                    pc[:], ones_sb[:], et[:, 2 * j:2 * j + 2, tq],
                    start=(j == 0), stop=(j == NTK // 2 - 1),
                    perf_mode=DR,
                )
            rec = scr_pool.tile([128, 512], F32, tag="rec", name=f"rec{b}_{n2}")
            nc.vector.reciprocal_approx_fast(out=rec[:], in_=pc[:])
            rec2 = scr_pool.tile([128, 2, 512], F32, tag="rec2", name=f"rec2{b}_{n2}")
            nc.gpsimd.tensor_copy(rec2[:, 0, :], rec[:])
            nc.gpsimd.tensor_copy(rec2[:, 1, :], rec[:])
            for mp in range(NCH // 2):
                po2 = ps_big.tile([128, T], F32, tag="big", name=f"po{b}_{n2}_{mp}")
                for j in range(NTK // 2):
                    for mi in range(2):
                        m = 2 * mp + mi
                        nc.tensor.matmul(
                            po2[:, mi * 512:(mi + 1) * 512],
                            vt[:, 2 * j:2 * j + 2, m * 128:(m + 1) * 128],
                            et[:, 2 * j:2 * j + 2, tq],
                            start=(j == 0), stop=(j == NTK // 2 - 1),
                            perf_mode=DR,
                        )
                nc.vector.tensor_mul(ont[:, 2 * mp:2 * mp + 2, tq], po2[:], rec2[:])
        onts[b] = ont

    def emit_proj(b):
        ont, xt = onts[b], xts[b]
        out_t = out_pool.tile([128, NCH, T], F32, tag="out", name=f"out_{b}")
        for m in range(NCH):
            for n2 in range(2):
                tq = slice(n2 * 512, (n2 + 1) * 512)
                ps = ps_sm.tile([128, 512], F32, tag="acc", name=f"ps_p{b}_{m}_{n2}")
                for j in range(NCH // 2):
                    nc.tensor.matmul(
                        ps[:],
                        wsb["wpT"][j][:, :, m * 128:(m + 1) * 128],
                        ont[:, 2 * j:2 * j + 2, tq],
                        start=(j == 0), stop=(j == NCH // 2 - 1),
                        perf_mode=DR,
                    )
                nc.vector.scalar_tensor_tensor(
                    out_t[:, m, tq], ps[:], ds_sb[:, 3:4], xt[:, m, tq],
                    op0=mybir.AluOpType.mult, op1=mybir.AluOpType.add,
                )
                if o_bias:
                    nc.vector.tensor_scalar_add(
                        out_t[:, m, tq], out_t[:, m, tq], bo_sb[:, m:m + 1]
                    )
            nc.sync.dma_start(out=y_ap[b][:, m, :], in_=out_t[:, m, :])

    emit_gn(0)
    emit_h(0)
    for b in range(BPC):
        emit_qkv(b)
        if b >= 1:
            emit_proj(b - 1)
        if b + 1 < BPC:
            load_x(b + 1)
            emit_gn(b + 1)
        emit_attn(b)
        if b + 1 < BPC:
            emit_h(b + 1)
    emit_proj(BPC - 1)


def _build(flags):
    from contextlib import ExitStack

    nc = bacc.Bacc(
        "TRN2",
        target_bir_lowering=False,
        debug=False,
        enable_asserts=False,
        num_devices=NCORES,
    )
    with tile.TileContext(nc) as tc:
        with ExitStack() as ctx:
            _emit(nc, tc, ctx, flags)
    nc.compile()
    return nc


_CACHE = {}


def _consts():
    p = np.arange(128)
    maskg = (p[:, None] // 16 == np.arange(GPC)[None, :]).astype(np.float32)
    bcp = maskg.T.copy()
    return {
        "ones256": np.ones((128, 2, 128), mybir.dt.np(FP8)),
        "mask_g": maskg,
        "bc_pos": np.ascontiguousarray(bcp),
        "bc_neg": np.ascontiguousarray(-bcp),
    }


def kernel(x, gn_w, gn_b, wq, bq, wk, bk, wv, bv, wp, bp):
    x = np.ascontiguousarray(np.asarray(x, dtype=np.float32))
    B = x.shape[0]
    assert (B, x.shape[1], x.shape[2] * x.shape[3]) == (NCORES * BPC, C, T)
    H, W = x.shape[2], x.shape[3]
    xr = x.reshape(B, C, T)
    gn_w = np.asarray(gn_w, np.float32)
    gn_b = np.asarray(gn_b, np.float32)
    trivial_gn = bool(np.all(gn_w == 1.0) and np.all(gn_b == 0.0))
    qk_bias = bool(np.any(np.asarray(bq)) or np.any(np.asarray(bk)))
    o_bias = bool(np.any(np.asarray(bv)) or np.any(np.asarray(bp)))
    flags = (trivial_gn, qk_bias, o_bias)
    if flags not in _CACHE:
        _CACHE[flags] = _build(flags)
    nc = _CACHE[flags]

    common = dict(_consts())
    descale = np.empty(4, np.float32)
    fp8 = mybir.dt.np(FP8)
    for i, (n, w) in enumerate(
        (("wqT", wq), ("wkT", wk), ("wvT", wv), ("wpT", wp))
    ):
        wT = np.ascontiguousarray(np.asarray(w, np.float32).T)
        amax = float(np.abs(wT).max()) or 1.0
        k = int(np.floor(np.log2(88.0 / amax)))
        descale[i] = 2.0 ** (-k)
        ws = (wT * (2.0 ** k)).astype(fp8)
        common[n] = np.ascontiguousarray(
            ws.reshape(NCH // 2, 2, 128, C).transpose(0, 2, 1, 3)
        )
    common["descale"] = np.ascontiguousarray(np.broadcast_to(descale, (128, 4)))
    if not trivial_gn:
        common["gnw"] = gn_w
        common["gnb"] = gn_b
    if qk_bias:
        common["bq"] = np.asarray(bq, np.float32)
        common["bk"] = np.asarray(bk, np.float32)
    if o_bias:
        common["bias_o"] = (
            np.asarray(wp, np.float32) @ np.asarray(bv, np.float32)
            + np.asarray(bp, np.float32)
        ).astype(np.float32)

    in_maps = [
        {"x": np.ascontiguousarray(xr[c * BPC:(c + 1) * BPC]), **common}
        for c in range(NCORES)
    ]
    res = run_bass_kernel_spmd(nc, in_maps, core_ids=list(range(NCORES)))
    y = np.concatenate([res.results[c]["y"] for c in range(NCORES)], axis=0)
    return np.ascontiguousarray(y.reshape(B, C, H, W).astype(np.float32))


# revision 61
# speedup vs baseline: 2.7006x; 1.0433x over previous
import numpy as np
import ml_dtypes

import concourse.bacc as bacc
import concourse.bass as bass
import concourse.tile as tile
from concourse import mybir
from concourse.bass_utils import run_bass_kernel_spmd

F32 = mybir.dt.float32
BF16 = mybir.dt.bfloat16
FP8 = mybir.dt.float8e4
DR = mybir.MatmulPerfMode.DoubleRow
AF = mybir.ActivationFunctionType
EXP_BIAS = -3.5

NCORES = 8
BPC = 4
C = 512
T = 1024
NCH = 4
NTK = 8
GPC = 8
EPS = 1e-5
SM_SCALE = float(C) ** -0.5


def _emit(nc, tc, ctx, flags):
    trivial_gn, qk_bias, o_bias = flags

    x_d = nc.dram_tensor("x", (BPC, C, T), F32, kind="ExternalInput")
    y_d = nc.dram_tensor("y", (BPC, C, T), F32, kind="ExternalOutput")
    w_d = {
        n: nc.dram_tensor(n, (NCH // 2, 128, 2, C), FP8, kind="ExternalInput")
        for n in ("wqT", "wkT", "wvT", "wpT")
    }
    ds_d = nc.dram_tensor("descale", (128, 4), F32, kind="ExternalInput")
    ones_d = nc.dram_tensor("ones256", (128, 2, 128), FP8, kind="ExternalInput")
    maskg_d = nc.dram_tensor("mask_g", (128, GPC), F32, kind="ExternalInput")
    bcp_d = nc.dram_tensor("bc_pos", (GPC, 128), F32, kind="ExternalInput")
    bcn_d = nc.dram_tensor("bc_neg", (GPC, 128), F32, kind="ExternalInput")
    if not trivial_gn:
        gnw_d = nc.dram_tensor("gnw", (C,), F32, kind="ExternalInput")
        gnb_d = nc.dram_tensor("gnb", (C,), F32, kind="ExternalInput")
    if qk_bias:
        bq_d = nc.dram_tensor("bq", (C,), F32, kind="ExternalInput")
        bk_d = nc.dram_tensor("bk", (C,), F32, kind="ExternalInput")
    if o_bias:
        bo_d = nc.dram_tensor("bias_o", (C,), F32, kind="ExternalInput")

    from concourse.hw_specs import get_activation_tables

    tabs = list(get_activation_tables(nc.m.arch))
    lnexp_id = tabs.index("natural_log_exp_and_others")
    nc.scalar.add_instruction(
        mybir.InstLoadActFuncSet(
            name=nc.get_next_instruction_name(),
            ins=[],
            outs=[],
            act_func_set_id=lnexp_id,
        )
    )

    x_ap = x_d.ap().rearrange("b (c p) t -> b p c t", p=128)
    y_ap = y_d.ap().rearrange("b (c p) t -> b p c t", p=128)

    singles = ctx.enter_context(tc.tile_pool(name="singles", bufs=1))
    x_pool = ctx.enter_context(tc.tile_pool(name="x", bufs=3))
    h_pool = ctx.enter_context(tc.tile_pool(name="h", bufs=2))
    q_pool = ctx.enter_context(tc.tile_pool(name="q", bufs=2))
    k_pool = ctx.enter_context(tc.tile_pool(name="k", bufs=2))
    v_pool = ctx.enter_context(tc.tile_pool(name="v", bufs=2))
    e_pool = ctx.enter_context(tc.tile_pool(name="e", bufs=2))
    on_pool = ctx.enter_context(tc.tile_pool(name="on", bufs=2))
    out_pool = ctx.enter_context(tc.tile_pool(name="out", bufs=2))
    scr_pool = ctx.enter_context(tc.tile_pool(name="scr", bufs=2))
    st_pool = ctx.enter_context(tc.tile_pool(name="st", bufs=2))
    ps_big = ctx.enter_context(tc.tile_pool(name="ps_big", bufs=3, space="PSUM"))
    ps_sm = ctx.enter_context(tc.tile_pool(name="ps_sm", bufs=2, space="PSUM"))

    xts = {}

    def load_x(b):
        xt = x_pool.tile([128, NCH, T], F32, tag="x", name=f"xt{b}")
        for ci in range(NCH):
            nc.sync.dma_start(out=xt[:, ci, :], in_=x_ap[b][:, ci, :])
        xts[b] = xt

    load_x(0)

    maskg_sb = singles.tile([128, GPC], F32, tag="maskg")
    nc.sync.dma_start(out=maskg_sb[:], in_=maskg_d.ap())
    bcp_sb = singles.tile([GPC, 128], F32, tag="bcp")
    nc.sync.dma_start(out=bcp_sb[:], in_=bcp_d.ap())
    bcn_sb = singles.tile([GPC, 128], F32, tag="bcn")
    nc.sync.dma_start(out=bcn_sb[:], in_=bcn_d.ap())
    ds_sb = singles.tile([128, 4], F32, tag="descale")
    nc.sync.dma_start(out=ds_sb[:], in_=ds_d.ap())
    eps_sb = singles.tile([GPC, 1], F32, tag="eps")
    nc.vector.memset(eps_sb[:], EPS)
    ebias_sb = singles.tile([128, 1], F32, tag="ebias")
    nc.vector.memset(ebias_sb[:], EXP_BIAS)
    ones_sb = singles.tile([128, 2, 128], FP8, tag="ones")
    nc.sync.dma_start(out=ones_sb[:], in_=ones_d.ap())
    wsb = {}
    for n in ("wqT", "wkT", "wvT", "wpT"):
        tiles = []
        for j in range(NCH // 2):
            wt = singles.tile([128, 2, C], FP8, tag=f"{n}{j}")
            nc.sync.dma_start(out=wt[:], in_=w_d[n].ap()[j])
            tiles.append(wt)
        wsb[n] = tiles
    if not trivial_gn:
        gnw_sb = singles.tile([128, NCH], F32, tag="gnw")
        gnb_sb = singles.tile([128, NCH], F32, tag="gnb")
        nc.sync.dma_start(out=gnw_sb[:], in_=gnw_d.ap().rearrange("(c p) -> p c", p=128))
        nc.sync.dma_start(out=gnb_sb[:], in_=gnb_d.ap().rearrange("(c p) -> p c", p=128))
    if qk_bias:
        bq_sb = singles.tile([128, NCH], F32, tag="bq")
        bk_sb = singles.tile([128, NCH], F32, tag="bk")
        nc.sync.dma_start(out=bq_sb[:], in_=bq_d.ap().rearrange("(c p) -> p c", p=128))
        nc.sync.dma_start(out=bk_sb[:], in_=bk_d.ap().rearrange("(c p) -> p c", p=128))
    if o_bias:
        bo_sb = singles.tile([128, NCH], F32, tag="bo")
        nc.sync.dma_start(out=bo_sb[:], in_=bo_d.ap().rearrange("(c p) -> p c", p=128))

    hts, qts, kts, vts, onts, sc_sh = {}, {}, {}, {}, {}, {}

    def emit_gn(b):
        eng = nc.gpsimd
        xt = xts[b]
        mv6 = st_pool.tile([128, NCH, 2, 6], F32, tag="mv6", name=f"mv6_{b}")
        mv = st_pool.tile([128, NCH, 2], F32, tag="mv", name=f"mv_{b}")
        for ci in range(NCH):
            for s in range(2):
                nc.vector.bn_stats(
                    out=mv6[:, ci, s, :], in_=xt[:, ci, s * 512:(s + 1) * 512]
                )
            nc.vector.bn_aggr(out=mv[:, ci, :], in_=mv6[:, ci, :, :])
        msq = st_pool.tile([128, NCH, 1], F32, tag="msq", name=f"msq_{b}")
        m2 = st_pool.tile([128, NCH, 1], F32, tag="m2", name=f"m2_{b}")
        eng.tensor_mul(msq[:], mv[:, :, 0:1], mv[:, :, 0:1])
        eng.tensor_add(m2[:], mv[:, :, 1:2], msq[:])
        gn_ps = ps_sm.tile([128, 16], F32, tag="acc", name=f"gn_ps_{b}")
        psum_g = gn_ps[0:GPC, 0:2 * NCH]
        psum_bc = gn_ps[:, 2 * NCH:4 * NCH]
        for ci in range(NCH):
            nc.tensor.matmul(
                psum_g[:, ci:ci + 1], maskg_sb[:], mv[:, ci, 0:1],
                start=True, stop=True,
            )
            nc.tensor.matmul(
                psum_g[:, NCH + ci:NCH + ci + 1], maskg_sb[:], m2[:, ci, :],
                start=True, stop=True,
            )
        gstat = st_pool.tile([GPC, 2 * NCH], F32, tag="gstat", name=f"gstat_{b}")
        nc.vector.tensor_scalar_mul(gstat[:], psum_g[:], 1.0 / 16.0)
        sqg = st_pool.tile([GPC, NCH], F32, tag="sqg", name=f"sqg_{b}")
        varg = st_pool.tile([GPC, NCH], F32, tag="varg", name=f"varg_{b}")
        eng.tensor_mul(sqg[:], gstat[:, 0:NCH], gstat[:, 0:NCH])
        eng.tensor_sub(varg[:], gstat[:, NCH:2 * NCH], sqg[:])
        lnv = st_pool.tile([GPC, NCH], F32, tag="lnv", name=f"lnv_{b}")
        nc.scalar.activation(lnv[:], varg[:], AF.Ln, bias=eps_sb[:])
        rstd = st_pool.tile([GPC, NCH], F32, tag="rstd", name=f"rstd_{b}")
        nc.scalar.activation(rstd[:], lnv[:], AF.Exp, scale=-0.5)
        for ci in range(NCH):
            nc.tensor.matmul(
                psum_bc[:, ci:ci + 1], bcn_sb[:], gstat[:, ci:ci + 1],
                start=True, stop=True,
            )
            nc.tensor.matmul(
                psum_bc[:, NCH + ci:NCH + ci + 1], bcp_sb[:], rstd[:, ci:ci + 1],
                start=True, stop=True,
            )
        bc = st_pool.tile([128, 2 * NCH], F32, tag="bc", name=f"bc_{b}")
        nc.vector.tensor_copy(bc[:], psum_bc[:])
        if trivial_gn:
            scale_t = bc[:, NCH:2 * NCH]
            shift_t = st_pool.tile([128, NCH], F32, tag="shift", name=f"shift_{b}")
            eng.tensor_mul(shift_t[:], bc[:, 0:NCH], bc[:, NCH:2 * NCH])
        else:
            scale_full = st_pool.tile([128, NCH], F32, tag="scalef", name=f"scf_{b}")
            eng.tensor_mul(scale_full[:], bc[:, NCH:2 * NCH], gnw_sb[:])
            tmp = st_pool.tile([128, NCH], F32, tag="tmpf", name=f"tmpf_{b}")
            eng.tensor_mul(tmp[:], bc[:, 0:NCH], scale_full[:])
            shift_t = st_pool.tile([128, NCH], F32, tag="shift", name=f"shift_{b}")
            eng.tensor_add(shift_t[:], tmp[:], gnb_sb[:])
            scale_t = scale_full
        sc_sh[b] = (scale_t, shift_t)

    def emit_h(b):
        scale_t, shift_t = sc_sh[b]
        xt = xts[b]
        ht = h_pool.tile([128, NCH, T], FP8, tag="h", name=f"ht_{b}")
        for ci in range(NCH):
            if ci % 2 == 0:
                nc.scalar.activation(
                    ht[:, ci, :], xt[:, ci, :], AF.Identity,
                    bias=shift_t[:, ci:ci + 1], scale=scale_t[:, ci:ci + 1],
                )
            else:
                nc.vector.tensor_scalar(
                    ht[:, ci, :], xt[:, ci, :],
                    scale_t[:, ci:ci + 1], shift_t[:, ci:ci + 1],
                    op0=mybir.AluOpType.mult, op1=mybir.AluOpType.add,
                )
        hts[b] = ht

    def emit_qkv(b):
        ht = hts[b]
        qt = q_pool.tile([128, NCH, T], FP8, tag="q", name=f"qt_{b}")
        kt = k_pool.tile([128, NCH, T], FP8, tag="k", name=f"kt_{b}")
        vt = v_pool.tile([128, NTK, C], FP8, tag="v", name=f"vt_{b}")
        for m in range(NCH):
            ps = ps_big.tile([128, T], F32, tag="big", name=f"ps_q{b}_{m}")
            for n2 in range(2):
                for j in range(NCH // 2):
                    nc.tensor.matmul(
                        ps[:, n2 * 512:(n2 + 1) * 512],
                        wsb["wqT"][j][:, :, m * 128:(m + 1) * 128],
                        ht[:, 2 * j:2 * j + 2, n2 * 512:(n2 + 1) * 512],
                        start=(j == 0), stop=(j == NCH // 2 - 1),
                        perf_mode=DR,
                    )
            if qk_bias:
                nc.scalar.activation(
                    qt[:, m, :], ps[:], AF.Identity,
                    bias=bq_sb[:, m:m + 1], scale=ds_sb[:, 0:1],
                )
            else:
                nc.scalar.mul(qt[:, m, :], ps[:], ds_sb[:, 0:1])
            ps = ps_big.tile([128, T], F32, tag="big", name=f"ps_k{b}_{m}")
            for n2 in range(2):
                for j in range(NCH // 2):
                    nc.tensor.matmul(
                        ps[:, n2 * 512:(n2 + 1) * 512],
                        wsb["wkT"][j][:, :, m * 128:(m + 1) * 128],
                        ht[:, 2 * j:2 * j + 2, n2 * 512:(n2 + 1) * 512],
                        start=(j == 0), stop=(j == NCH // 2 - 1),
                        perf_mode=DR,
                    )
            if qk_bias:
                nc.vector.tensor_scalar(
                    kt[:, m, :], ps[:], ds_sb[:, 1:2], bk_sb[:, m:m + 1],
                    op0=mybir.AluOpType.mult, op1=mybir.AluOpType.add,
                )
            else:
                nc.scalar.mul(kt[:, m, :], ps[:], ds_sb[:, 1:2])
        for mp in range(NTK // 2):
            ps = ps_big.tile([128, T], F32, tag="big", name=f"ps_v{b}_{mp}")
            for half in range(2):
                m = 2 * mp + half
                for j in range(NCH // 2):
                    nc.tensor.matmul(
                        ps[:, half * 512:(half + 1) * 512],
                        ht[:, 2 * j:2 * j + 2, m * 128:(m + 1) * 128],
                        wsb["wvT"][j][:],
                        start=(j == 0), stop=(j == NCH // 2 - 1),
                        perf_mode=DR,
                    )
            if mp % 2 == 0:
                nc.scalar.mul(vt[:, 2 * mp:2 * mp + 2, :], ps[:], ds_sb[:, 2:3])
            else:
                nc.vector.tensor_scalar_mul(
                    vt[:, 2 * mp:2 * mp + 2, :], ps[:], ds_sb[:, 2:3]
                )
        qts[b], kts[b], vts[b] = qt, kt, vt

    def emit_attn(b):
        qt, kt, vt = qts[b], kts[b], vts[b]
        et = e_pool.tile([128, NTK, T], FP8, tag="e", name=f"et_{b}")
        for tk in range(NTK):
            ps_s = ps_big.tile([128, T], F32, tag="big", name=f"ps_s{b}_{tk}")
            for n2 in range(2):
                for j in range(NCH // 2):
                    nc.tensor.matmul(
                        ps_s[:, n2 * 512:(n2 + 1) * 512],
                        kt[:, 2 * j:2 * j + 2, tk * 128:(tk + 1) * 128],
                        qt[:, 2 * j:2 * j + 2, n2 * 512:(n2 + 1) * 512],
                        start=(j == 0), stop=(j == NCH // 2 - 1),
                        perf_mode=DR,
                    )
            nc.scalar.activation(
                et[:, tk, :], ps_s[:], AF.Exp, scale=SM_SCALE, bias=ebias_sb[:]
            )
        ont = on_pool.tile([128, NCH, T], FP8, tag="on", name=f"ont_{b}")
        for n2 in range(2):
            tq = slice(n2 * 512, (n2 + 1) * 512)
            pc = ps_sm.tile([128, 512], F32, tag="acc", name=f"pc{b}_{n2}")
            for j in range(NTK // 2):
                nc.tensor.matmul(
                    pc[:], ones_sb[:], et[:, 2 * j:2 * j + 2, tq],
                    start=(j == 0), stop=(j == NTK // 2 - 1),
                    perf_mode=DR,
                )
            rec = scr_pool.tile([128, 512], F32, tag="rec", name=f"rec{b}_{n2}")
            nc.vector.reciprocal_approx_fast(out=rec[:], in_=pc[:])
            rb = rec[:]
            rec2 = bass.AP(
                tensor=rb.tensor, offset=rb.offset,
                ap=[rb.ap[0], [0, 2], rb.ap[1]],
            )
            for mp in range(NCH // 2):
                po2 = ps_big.tile([128, T], F32, tag="big", name=f"po{b}_{n2}_{mp}")
                for j in range(NTK // 2):
                    for mi in range(2):
                        m = 2 * mp + mi
                        nc.tensor.matmul(
                            po2[:, mi * 512:(mi + 1) * 512],
                            vt[:, 2 * j:2 * j + 2, m * 128:(m + 1) * 128],
                            et[:, 2 * j:2 * j + 2, tq],
                            start=(j == 0), stop=(j == NTK // 2 - 1),
                            perf_mode=DR,
                        )
                nc.vector.tensor_mul(ont[:, 2 * mp:2 * mp + 2, tq], po2[:], rec2)
        onts[b] = ont

    def emit_proj(b):
        ont, xt = onts[b], xts[b]
        out_t = out_pool.tile([128, NCH, T], F32, tag="out", name=f"out_{b}")
        for m in range(NCH):
            for n2 in range(2):
                tq = slice(n2 * 512, (n2 + 1) * 512)
                ps = ps_sm.tile([128, 512], F32, tag="acc", name=f"ps_p{b}_{m}_{n2}")
                for j in range(NCH // 2):
                    nc.tensor.matmul(
                        ps[:],
                        wsb["wpT"][j][:, :, m * 128:(m + 1) * 128],
                        ont[:, 2 * j:2 * j + 2, tq],
                        start=(j == 0), stop=(j == NCH // 2 - 1),
                        perf_mode=DR,
                    )
                nc.vector.scalar_tensor_tensor(
                    out_t[:, m, tq], ps[:], ds_sb[:, 3:4], xt[:, m, tq],
                    op0=mybir.AluOpType.mult, op1=mybir.AluOpType.add,
                )
                if o_bias:
                    nc.vector.tensor_scalar_add(
                        out_t[:, m, tq], out_t[:, m, tq], bo_sb[:, m:m + 1]
                    )
            nc.sync.dma_start(out=y_ap[b][:, m, :], in_=out_t[:, m, :])

    emit_gn(0)
    emit_h(0)
    for b in range(BPC):
        emit_qkv(b)
        if b >= 1:
            emit_proj(b - 1)
        if b + 1 < BPC:
            load_x(b + 1)
            emit_gn(b + 1)
        emit_attn(b)
        if b + 1 < BPC:
            emit_h(b + 1)
    emit_proj(BPC - 1)


def _build(flags):
    from contextlib import ExitStack

    nc = bacc.Bacc(
        "TRN2",
        target_bir_lowering=False,
        debug=False,
        enable_asserts=False,
        num_devices=NCORES,
    )
    with tile.TileContext(nc) as tc:
        with ExitStack() as ctx:
            _emit(nc, tc, ctx, flags)
    nc.compile()
    return nc


_CACHE = {}


def _consts():
    p = np.arange(128)
    maskg = (p[:, None] // 16 == np.arange(GPC)[None, :]).astype(np.float32)
    bcp = maskg.T.copy()
    return {
        "ones256": np.ones((128, 2, 128), mybir.dt.np(FP8)),
        "mask_g": maskg,
        "bc_pos": np.ascontiguousarray(bcp),
        "bc_neg": np.ascontiguousarray(-bcp),
    }


def kernel(x, gn_w, gn_b, wq, bq, wk, bk, wv, bv, wp, bp):
    x = np.ascontiguousarray(np.asarray(x, dtype=np.float32))
    B = x.shape[0]
    assert (B, x.shape[1], x.shape[2] * x.shape[3]) == (NCORES * BPC, C, T)
    H, W = x.shape[2], x.shape[3]
    xr = x.reshape(B, C, T)
    gn_w = np.asarray(gn_w, np.float32)
    gn_b = np.asarray(gn_b, np.float32)
    trivial_gn = bool(np.all(gn_w == 1.0) and np.all(gn_b == 0.0))
    qk_bias = bool(np.any(np.asarray(bq)) or np.any(np.asarray(bk)))
    o_bias = bool(np.any(np.asarray(bv)) or np.any(np.asarray(bp)))
    flags = (trivial_gn, qk_bias, o_bias)
    if flags not in _CACHE:
        _CACHE[flags] = _build(flags)
    nc = _CACHE[flags]

    common = dict(_consts())
    descale = np.empty(4, np.float32)
    fp8 = mybir.dt.np(FP8)
    for i, (n, w) in enumerate(
        (("wqT", wq), ("wkT", wk), ("wvT", wv), ("wpT", wp))
    ):
        wT = np.ascontiguousarray(np.asarray(w, np.float32).T)
        amax = float(np.abs(wT).max()) or 1.0
        k = int(np.floor(np.log2(88.0 / amax)))
        descale[i] = 2.0 ** (-k)
        ws = (wT * (2.0 ** k)).astype(fp8)
        common[n] = np.ascontiguousarray(
            ws.reshape(NCH // 2, 2, 128, C).transpose(0, 2, 1, 3)
        )
    common["descale"] = np.ascontiguousarray(np.broadcast_to(descale, (128, 4)))
    if not trivial_gn:
        common["gnw"] = gn_w
        common["gnb"] = gn_b
    if qk_bias:
        common["bq"] = np.asarray(bq, np.float32)
        common["bk"] = np.asarray(bk, np.float32)
    if o_bias:
        common["bias_o"] = (
            np.asarray(wp, np.float32) @ np.asarray(bv, np.float32)
            + np.asarray(bp, np.float32)
        ).astype(np.float32)

    in_maps = [
        {"x": np.ascontiguousarray(xr[c * BPC:(c + 1) * BPC]), **common}
        for c in range(NCORES)
    ]
    res = run_bass_kernel_spmd(nc, in_maps, core_ids=list(range(NCORES)))
    y = np.concatenate([res.results[c]["y"] for c in range(NCORES)], axis=0)
    return np.ascontiguousarray(y.reshape(B, C, H, W).astype(np.float32))


# revision 62
# speedup vs baseline: 2.7039x; 1.0012x over previous
import numpy as np
import ml_dtypes

import concourse.bacc as bacc
import concourse.bass as bass
import concourse.tile as tile
from concourse import mybir
from concourse.bass_utils import run_bass_kernel_spmd

F32 = mybir.dt.float32
BF16 = mybir.dt.bfloat16
FP8 = mybir.dt.float8e4
DR = mybir.MatmulPerfMode.DoubleRow
AF = mybir.ActivationFunctionType
EXP_BIAS = -3.5

NCORES = 8
BPC = 4
C = 512
T = 1024
NCH = 4
NTK = 8
GPC = 8
EPS = 1e-5
SM_SCALE = float(C) ** -0.5


def _emit(nc, tc, ctx, flags):
    trivial_gn, qk_bias, o_bias = flags

    x_d = nc.dram_tensor("x", (BPC, C, T), F32, kind="ExternalInput")
    y_d = nc.dram_tensor("y", (BPC, C, T), F32, kind="ExternalOutput")
    w_d = {
        n: nc.dram_tensor(n, (NCH // 2, 128, 2, C), FP8, kind="ExternalInput")
        for n in ("wqT", "wkT", "wvT", "wpT")
    }
    ds_d = nc.dram_tensor("descale", (128, 4), F32, kind="ExternalInput")
    ones_d = nc.dram_tensor("ones256", (128, 2, 128), FP8, kind="ExternalInput")
    maskg_d = nc.dram_tensor("mask_g", (128, GPC), F32, kind="ExternalInput")
    bcp_d = nc.dram_tensor("bc_pos", (GPC, 128), F32, kind="ExternalInput")
    bcn_d = nc.dram_tensor("bc_neg", (GPC, 128), F32, kind="ExternalInput")
    if not trivial_gn:
        gnw_d = nc.dram_tensor("gnw", (C,), F32, kind="ExternalInput")
        gnb_d = nc.dram_tensor("gnb", (C,), F32, kind="ExternalInput")
    if qk_bias:
        bq_d = nc.dram_tensor("bq", (C,), F32, kind="ExternalInput")
        bk_d = nc.dram_tensor("bk", (C,), F32, kind="ExternalInput")
    if o_bias:
        bo_d = nc.dram_tensor("bias_o", (C,), F32, kind="ExternalInput")

    from concourse.hw_specs import get_activation_tables

    tabs = list(get_activation_tables(nc.m.arch))
    lnexp_id = tabs.index("natural_log_exp_and_others")
    nc.scalar.add_instruction(
        mybir.InstLoadActFuncSet(
            name=nc.get_next_instruction_name(),
            ins=[],
            outs=[],
            act_func_set_id=lnexp_id,
        )
    )

    x_ap = x_d.ap().rearrange("b (c p) t -> b p c t", p=128)
    y_ap = y_d.ap().rearrange("b (c p) t -> b p c t", p=128)

    singles = ctx.enter_context(tc.tile_pool(name="singles", bufs=1))
    x_pool = ctx.enter_context(tc.tile_pool(name="x", bufs=3))
    h_pool = ctx.enter_context(tc.tile_pool(name="h", bufs=2))
    q_pool = ctx.enter_context(tc.tile_pool(name="q", bufs=2))
    k_pool = ctx.enter_context(tc.tile_pool(name="k", bufs=2))
    v_pool = ctx.enter_context(tc.tile_pool(name="v", bufs=2))
    e_pool = ctx.enter_context(tc.tile_pool(name="e", bufs=2))
    on_pool = ctx.enter_context(tc.tile_pool(name="on", bufs=2))
    out_pool = ctx.enter_context(tc.tile_pool(name="out", bufs=2))
    scr_pool = ctx.enter_context(tc.tile_pool(name="scr", bufs=2))
    st_pool = ctx.enter_context(tc.tile_pool(name="st", bufs=2))
    ps_big = ctx.enter_context(tc.tile_pool(name="ps_big", bufs=3, space="PSUM"))
    ps_sm = ctx.enter_context(tc.tile_pool(name="ps_sm", bufs=2, space="PSUM"))

    xts = {}

    def load_x(b):
        xt = x_pool.tile([128, NCH, T], F32, tag="x", name=f"xt{b}")
        for ci in range(NCH):
            nc.sync.dma_start(out=xt[:, ci, :], in_=x_ap[b][:, ci, :])
        xts[b] = xt

    load_x(0)

    maskg_sb = singles.tile([128, GPC], F32, tag="maskg")
    nc.sync.dma_start(out=maskg_sb[:], in_=maskg_d.ap())
    bcp_sb = singles.tile([GPC, 128], F32, tag="bcp")
    nc.sync.dma_start(out=bcp_sb[:], in_=bcp_d.ap())
    bcn_sb = singles.tile([GPC, 128], F32, tag="bcn")
    nc.sync.dma_start(out=bcn_sb[:], in_=bcn_d.ap())
    ds_sb = singles.tile([128, 4], F32, tag="descale")
    nc.sync.dma_start(out=ds_sb[:], in_=ds_d.ap())
    eps_sb = singles.tile([GPC, 1], F32, tag="eps")
    nc.vector.memset(eps_sb[:], EPS)
    ebias_sb = singles.tile([128, 1], F32, tag="ebias")
    nc.vector.memset(ebias_sb[:], EXP_BIAS)
    ones_sb = singles.tile([128, 2, 128], FP8, tag="ones")
    nc.sync.dma_start(out=ones_sb[:], in_=ones_d.ap())
    wsb = {}
    for n in ("wqT", "wkT", "wvT", "wpT"):
        tiles = []
        for j in range(NCH // 2):
            wt = singles.tile([128, 2, C], FP8, tag=f"{n}{j}")
            nc.sync.dma_start(out=wt[:], in_=w_d[n].ap()[j])
            tiles.append(wt)
        wsb[n] = tiles
    if not trivial_gn:
        gnw_sb = singles.tile([128, NCH], F32, tag="gnw")
        gnb_sb = singles.tile([128, NCH], F32, tag="gnb")
        nc.sync.dma_start(out=gnw_sb[:], in_=gnw_d.ap().rearrange("(c p) -> p c", p=128))
        nc.sync.dma_start(out=gnb_sb[:], in_=gnb_d.ap().rearrange("(c p) -> p c", p=128))
    if qk_bias:
        bq_sb = singles.tile([128, NCH], F32, tag="bq")
        bk_sb = singles.tile([128, NCH], F32, tag="bk")
        nc.sync.dma_start(out=bq_sb[:], in_=bq_d.ap().rearrange("(c p) -> p c", p=128))
        nc.sync.dma_start(out=bk_sb[:], in_=bk_d.ap().rearrange("(c p) -> p c", p=128))
    if o_bias:
        bo_sb = singles.tile([128, NCH], F32, tag="bo")
        nc.sync.dma_start(out=bo_sb[:], in_=bo_d.ap().rearrange("(c p) -> p c", p=128))

    hts, qts, kts, vts, onts, sc_sh = {}, {}, {}, {}, {}, {}

    def emit_gn(b):
        eng = nc.gpsimd
        xt = xts[b]
        mv6 = st_pool.tile([128, NCH, 2, 6], F32, tag="mv6", name=f"mv6_{b}")
        mv = st_pool.tile([128, NCH, 2], F32, tag="mv", name=f"mv_{b}")
        for ci in range(NCH):
            for s in range(2):
                nc.vector.bn_stats(
                    out=mv6[:, ci, s, :], in_=xt[:, ci, s * 512:(s + 1) * 512]
                )
            nc.vector.bn_aggr(out=mv[:, ci, :], in_=mv6[:, ci, :, :])
        msq = st_pool.tile([128, NCH, 1], F32, tag="msq", name=f"msq_{b}")
        m2 = st_pool.tile([128, NCH, 1], F32, tag="m2", name=f"m2_{b}")
        eng.tensor_mul(msq[:], mv[:, :, 0:1], mv[:, :, 0:1])
        eng.tensor_add(m2[:], mv[:, :, 1:2], msq[:])
        gn_ps = ps_sm.tile([128, 16], F32, tag="acc", name=f"gn_ps_{b}")
        psum_g = gn_ps[0:GPC, 0:2 * NCH]
        psum_bc = gn_ps[:, 2 * NCH:4 * NCH]
        for ci in range(NCH):
            nc.tensor.matmul(
                psum_g[:, ci:ci + 1], maskg_sb[:], mv[:, ci, 0:1],
                start=True, stop=True,
            )
            nc.tensor.matmul(
                psum_g[:, NCH + ci:NCH + ci + 1], maskg_sb[:], m2[:, ci, :],
                start=True, stop=True,
            )
        gstat = st_pool.tile([GPC, 2 * NCH], F32, tag="gstat", name=f"gstat_{b}")
        nc.vector.tensor_scalar_mul(gstat[:], psum_g[:], 1.0 / 16.0)
        sqg = st_pool.tile([GPC, NCH], F32, tag="sqg", name=f"sqg_{b}")
        varg = st_pool.tile([GPC, NCH], F32, tag="varg", name=f"varg_{b}")
        eng.tensor_mul(sqg[:], gstat[:, 0:NCH], gstat[:, 0:NCH])
        eng.tensor_sub(varg[:], gstat[:, NCH:2 * NCH], sqg[:])
        lnv = st_pool.tile([GPC, NCH], F32, tag="lnv", name=f"lnv_{b}")
        nc.scalar.activation(lnv[:], varg[:], AF.Ln, bias=eps_sb[:])
        rstd = st_pool.tile([GPC, NCH], F32, tag="rstd", name=f"rstd_{b}")
        nc.scalar.activation(rstd[:], lnv[:], AF.Exp, scale=-0.5)
        for ci in range(NCH):
            nc.tensor.matmul(
                psum_bc[:, ci:ci + 1], bcn_sb[:], gstat[:, ci:ci + 1],
                start=True, stop=True,
            )
            nc.tensor.matmul(
                psum_bc[:, NCH + ci:NCH + ci + 1], bcp_sb[:], rstd[:, ci:ci + 1],
                start=True, stop=True,
            )
        bc = st_pool.tile([128, 2 * NCH], F32, tag="bc", name=f"bc_{b}")
        nc.vector.tensor_copy(bc[:], psum_bc[:])
        if trivial_gn:
            scale_t = bc[:, NCH:2 * NCH]
            shift_t = st_pool.tile([128, NCH], F32, tag="shift", name=f"shift_{b}")
            eng.tensor_mul(shift_t[:], bc[:, 0:NCH], bc[:, NCH:2 * NCH])
        else:
            scale_full = st_pool.tile([128, NCH], F32, tag="scalef", name=f"scf_{b}")
            eng.tensor_mul(scale_full[:], bc[:, NCH:2 * NCH], gnw_sb[:])
            tmp = st_pool.tile([128, NCH], F32, tag="tmpf", name=f"tmpf_{b}")
            eng.tensor_mul(tmp[:], bc[:, 0:NCH], scale_full[:])
            shift_t = st_pool.tile([128, NCH], F32, tag="shift", name=f"shift_{b}")
            eng.tensor_add(shift_t[:], tmp[:], gnb_sb[:])
            scale_t = scale_full
        sc_sh[b] = (scale_t, shift_t)

    def emit_h(b):
        scale_t, shift_t = sc_sh[b]
        xt = xts[b]
        ht = h_pool.tile([128, NCH, T], FP8, tag="h", name=f"ht_{b}")
        for ci in range(NCH):
            if ci % 2 == 0:
                nc.scalar.activation(
                    ht[:, ci, :], xt[:, ci, :], AF.Identity,
                    bias=shift_t[:, ci:ci + 1], scale=scale_t[:, ci:ci + 1],
                )
            else:
                nc.vector.tensor_scalar(
                    ht[:, ci, :], xt[:, ci, :],
                    scale_t[:, ci:ci + 1], shift_t[:, ci:ci + 1],
                    op0=mybir.AluOpType.mult, op1=mybir.AluOpType.add,
                )
        hts[b] = ht

    def emit_qkv(b):
        ht = hts[b]
        qt = q_pool.tile([128, NCH, T], FP8, tag="q", name=f"qt_{b}")
        kt = k_pool.tile([128, NCH, T], FP8, tag="k", name=f"kt_{b}")
        vt = v_pool.tile([128, NTK, C], FP8, tag="v", name=f"vt_{b}")
        for m in range(NCH):
            ps = ps_big.tile([128, T], F32, tag="big", name=f"ps_q{b}_{m}")
            for n2 in range(2):
                for j in range(NCH // 2):
                    nc.tensor.matmul(
                        ps[:, n2 * 512:(n2 + 1) * 512],
                        wsb["wqT"][j][:, :, m * 128:(m + 1) * 128],
                        ht[:, 2 * j:2 * j + 2, n2 * 512:(n2 + 1) * 512],
                        start=(j == 0), stop=(j == NCH // 2 - 1),
                        perf_mode=DR,
                    )
            if qk_bias:
                nc.scalar.activation(
                    qt[:, m, :], ps[:], AF.Identity,
                    bias=bq_sb[:, m:m + 1], scale=ds_sb[:, 0:1],
                )
            else:
                nc.scalar.mul(qt[:, m, :], ps[:], ds_sb[:, 0:1])
            ps = ps_big.tile([128, T], F32, tag="big", name=f"ps_k{b}_{m}")
            for n2 in range(2):
                for j in range(NCH // 2):
                    nc.tensor.matmul(
                        ps[:, n2 * 512:(n2 + 1) * 512],
                        wsb["wkT"][j][:, :, m * 128:(m + 1) * 128],
                        ht[:, 2 * j:2 * j + 2, n2 * 512:(n2 + 1) * 512],
                        start=(j == 0), stop=(j == NCH // 2 - 1),
                        perf_mode=DR,
                    )
            if qk_bias:
                nc.vector.tensor_scalar(
                    kt[:, m, :], ps[:], ds_sb[:, 1:2], bk_sb[:, m:m + 1],
                    op0=mybir.AluOpType.mult, op1=mybir.AluOpType.add,
                )
            else:
                nc.scalar.mul(kt[:, m, :], ps[:], ds_sb[:, 1:2])
        for mp in range(NTK // 2):
            ps = ps_big.tile([128, T], F32, tag="big", name=f"ps_v{b}_{mp}")
            for half in range(2):
                m = 2 * mp + half
                for j in range(NCH // 2):
                    nc.tensor.matmul(
                        ps[:, half * 512:(half + 1) * 512],
                        ht[:, 2 * j:2 * j + 2, m * 128:(m + 1) * 128],
                        wsb["wvT"][j][:],
                        start=(j == 0), stop=(j == NCH // 2 - 1),
                        perf_mode=DR,
                    )
            if mp % 2 == 0:
                nc.scalar.mul(vt[:, 2 * mp:2 * mp + 2, :], ps[:], ds_sb[:, 2:3])
            else:
                nc.vector.tensor_scalar_mul(
                    vt[:, 2 * mp:2 * mp + 2, :], ps[:], ds_sb[:, 2:3]
                )
        qts[b], kts[b], vts[b] = qt, kt, vt

    ets = {}

    def emit_attn_s(b):
        qt, kt = qts[b], kts[b]
        et = e_pool.tile([128, NTK, T], FP8, tag="e", name=f"et_{b}")
        for tk in range(NTK):
            ps_s = ps_big.tile([128, T], F32, tag="big", name=f"ps_s{b}_{tk}")
            for n2 in range(2):
                for j in range(NCH // 2):
                    nc.tensor.matmul(
                        ps_s[:, n2 * 512:(n2 + 1) * 512],
                        kt[:, 2 * j:2 * j + 2, tk * 128:(tk + 1) * 128],
                        qt[:, 2 * j:2 * j + 2, n2 * 512:(n2 + 1) * 512],
                        start=(j == 0), stop=(j == NCH // 2 - 1),
                        perf_mode=DR,
                    )
            nc.scalar.activation(
                et[:, tk, :], ps_s[:], AF.Exp, scale=SM_SCALE, bias=ebias_sb[:]
            )
        ets[b] = et
        onts[b] = on_pool.tile([128, NCH, T], FP8, tag="on", name=f"ont_{b}")

    def emit_attn_half(b, n2):
        vt, et, ont = vts[b], ets[b], onts[b]
        tq = slice(n2 * 512, (n2 + 1) * 512)
        pc = ps_sm.tile([128, 512], F32, tag="acc", name=f"pc{b}_{n2}")
        for j in range(NTK // 2):
            nc.tensor.matmul(
                pc[:], ones_sb[:], et[:, 2 * j:2 * j + 2, tq],
                start=(j == 0), stop=(j == NTK // 2 - 1),
                perf_mode=DR,
            )
        rec = scr_pool.tile([128, 512], F32, tag="rec", name=f"rec{b}_{n2}")
        nc.vector.reciprocal_approx_fast(out=rec[:], in_=pc[:])
        rb = rec[:]
        rec2 = bass.AP(
            tensor=rb.tensor, offset=rb.offset,
            ap=[rb.ap[0], [0, 2], rb.ap[1]],
        )
        for mp in range(NCH // 2):
            po2 = ps_big.tile([128, T], F32, tag="big", name=f"po{b}_{n2}_{mp}")
            for j in range(NTK // 2):
                for mi in range(2):
                    m = 2 * mp + mi
                    nc.tensor.matmul(
                        po2[:, mi * 512:(mi + 1) * 512],
                        vt[:, 2 * j:2 * j + 2, m * 128:(m + 1) * 128],
                        et[:, 2 * j:2 * j + 2, tq],
                        start=(j == 0), stop=(j == NTK // 2 - 1),
                        perf_mode=DR,
                    )
            nc.vector.tensor_mul(ont[:, 2 * mp:2 * mp + 2, tq], po2[:], rec2)

    def emit_attn(b):
        emit_attn_s(b)
        emit_attn_half(b, 0)
        emit_attn_half(b, 1)

    out_ts = {}

    def emit_proj_half(b, n2, pool):
        ont, xt = onts[b], xts[b]
        if b not in out_ts:
            out_ts[b] = out_pool.tile([128, NCH, T], F32, tag="out",
                                      name=f"out_{b}")
        out_t = out_ts[b]
        tq = slice(n2 * 512, (n2 + 1) * 512)
        for m in range(NCH):
            ps = pool.tile([128, 512], F32,
                           tag="big" if pool is ps_big else "acc",
                           name=f"ps_p{b}_{m}_{n2}")
            for j in range(NCH // 2):
                nc.tensor.matmul(
                    ps[:],
                    wsb["wpT"][j][:, :, m * 128:(m + 1) * 128],
                    ont[:, 2 * j:2 * j + 2, tq],
                    start=(j == 0), stop=(j == NCH // 2 - 1),
                    perf_mode=DR,
                )
            nc.vector.scalar_tensor_tensor(
                out_t[:, m, tq], ps[:], ds_sb[:, 3:4], xt[:, m, tq],
                op0=mybir.AluOpType.mult, op1=mybir.AluOpType.add,
            )
            if o_bias:
                nc.vector.tensor_scalar_add(
                    out_t[:, m, tq], out_t[:, m, tq], bo_sb[:, m:m + 1]
                )
            nc.sync.dma_start(out=y_ap[b][:, m, tq], in_=out_t[:, m, tq])

    def emit_proj(b):
        ont, xt = onts[b], xts[b]
        out_t = out_pool.tile([128, NCH, T], F32, tag="out", name=f"out_{b}")
        out_ts[b] = out_t
        for m in range(NCH):
            for n2 in range(2):
                tq = slice(n2 * 512, (n2 + 1) * 512)
                ps = ps_sm.tile([128, 512], F32, tag="acc", name=f"ps_p{b}_{m}_{n2}")
                for j in range(NCH // 2):
                    nc.tensor.matmul(
                        ps[:],
                        wsb["wpT"][j][:, :, m * 128:(m + 1) * 128],
                        ont[:, 2 * j:2 * j + 2, tq],
                        start=(j == 0), stop=(j == NCH // 2 - 1),
                        perf_mode=DR,
                    )
                nc.vector.scalar_tensor_tensor(
                    out_t[:, m, tq], ps[:], ds_sb[:, 3:4], xt[:, m, tq],
                    op0=mybir.AluOpType.mult, op1=mybir.AluOpType.add,
                )
                if o_bias:
                    nc.vector.tensor_scalar_add(
                        out_t[:, m, tq], out_t[:, m, tq], bo_sb[:, m:m + 1]
                    )
            nc.sync.dma_start(out=y_ap[b][:, m, :], in_=out_t[:, m, :])

    emit_gn(0)
    emit_h(0)
    for b in range(BPC):
        emit_qkv(b)
        if b >= 1:
            emit_proj(b - 1)
        if b + 1 < BPC:
            load_x(b + 1)
            emit_gn(b + 1)
        if b == BPC - 1:
            emit_attn_s(b)
            emit_attn_half(b, 0)
            emit_proj_half(b, 0, ps_big)
            emit_attn_half(b, 1)
            emit_proj_half(b, 1, ps_big)
        else:
            emit_attn(b)
        if b + 1 < BPC:
            emit_h(b + 1)


def _build(flags):
    from contextlib import ExitStack

    nc = bacc.Bacc(
        "TRN2",
        target_bir_lowering=False,
        debug=False,
        enable_asserts=False,
        num_devices=NCORES,
    )
    with tile.TileContext(nc) as tc:
        with ExitStack() as ctx:
            _emit(nc, tc, ctx, flags)
    nc.compile()
    return nc


_CACHE = {}


def _consts():
    p = np.arange(128)
    maskg = (p[:, None] // 16 == np.arange(GPC)[None, :]).astype(np.float32)
    bcp = maskg.T.copy()
    return {
        "ones256": np.ones((128, 2, 128), mybir.dt.np(FP8)),
        "mask_g": maskg,
        "bc_pos": np.ascontiguousarray(bcp),
        "bc_neg": np.ascontiguousarray(-bcp),
    }


def kernel(x, gn_w, gn_b, wq, bq, wk, bk, wv, bv, wp, bp):
    x = np.ascontiguousarray(np.asarray(x, dtype=np.float32))
    B = x.shape[0]
    assert (B, x.shape[1], x.shape[2] * x.shape[3]) == (NCORES * BPC, C, T)
    H, W = x.shape[2], x.shape[3]
    xr = x.reshape(B, C, T)
    gn_w = np.asarray(gn_w, np.float32)
    gn_b = np.asarray(gn_b, np.float32)
    trivial_gn = bool(np.all(gn_w == 1.0) and np.all(gn_b == 0.0))
    qk_bias = bool(np.any(np.asarray(bq)) or np.any(np.asarray(bk)))
    o_bias = bool(np.any(np.asarray(bv)) or np.any(np.asarray(bp)))
    flags = (trivial_gn, qk_bias, o_bias)
    if flags not in _CACHE:
        _CACHE[flags] = _build(flags)
    nc = _CACHE[flags]

    common = dict(_consts())
    descale = np.empty(4, np.float32)
    fp8 = mybir.dt.np(FP8)
    for i, (n, w) in enumerate(
        (("wqT", wq), ("wkT", wk), ("wvT", wv), ("wpT", wp))
    ):
        wT = np.ascontiguousarray(np.asarray(w, np.float32).T)
        amax = float(np.abs(wT).max()) or 1.0
        k = int(np.floor(np.log2(88.0 / amax)))
        descale[i] = 2.0 ** (-k)
        ws = (wT * (2.0 ** k)).astype(fp8)
        common[n] = np.ascontiguousarray(
            ws.reshape(NCH // 2, 2, 128, C).transpose(0, 2, 1, 3)
        )
    common["descale"] = np.ascontiguousarray(np.broadcast_to(descale, (128, 4)))
    if not trivial_gn:
        common["gnw"] = gn_w
        common["gnb"] = gn_b
    if qk_bias:
        common["bq"] = np.asarray(bq, np.float32)
        common["bk"] = np.asarray(bk, np.float32)
    if o_bias:
        common["bias_o"] = (
            np.asarray(wp, np.float32) @ np.asarray(bv, np.float32)
            + np.asarray(bp, np.float32)
        ).astype(np.float32)

    in_maps = [
        {"x": np.ascontiguousarray(xr[c * BPC:(c + 1) * BPC]), **common}
        for c in range(NCORES)
    ]
    res = run_bass_kernel_spmd(nc, in_maps, core_ids=list(range(NCORES)))
    y = np.concatenate([res.results[c]["y"] for c in range(NCORES)], axis=0)
    return np.ascontiguousarray(y.reshape(B, C, H, W).astype(np.float32))


# revision 67
# speedup vs baseline: 2.7056x; 1.0006x over previous
import numpy as np
import ml_dtypes

import concourse.bacc as bacc
import concourse.bass as bass
import concourse.tile as tile
from concourse import mybir
from concourse.bass_utils import run_bass_kernel_spmd

F32 = mybir.dt.float32
BF16 = mybir.dt.bfloat16
FP8 = mybir.dt.float8e4
DR = mybir.MatmulPerfMode.DoubleRow
AF = mybir.ActivationFunctionType
EXP_BIAS = -3.5

NCORES = 8
BPC = 4
C = 512
T = 1024
NCH = 4
NTK = 8
GPC = 8
EPS = 1e-5
SM_SCALE = float(C) ** -0.5


def _emit(nc, tc, ctx, flags):
    trivial_gn, qk_bias, o_bias = flags

    x_d = nc.dram_tensor("x", (BPC, C, T), F32, kind="ExternalInput")
    y_d = nc.dram_tensor("y", (BPC, C, T), F32, kind="ExternalOutput")
    w_d = {
        n: nc.dram_tensor(n, (NCH // 2, 128, 2, C), FP8, kind="ExternalInput")
        for n in ("wqT", "wkT", "wvT", "wpT")
    }
    ds_d = nc.dram_tensor("descale", (128, 4), F32, kind="ExternalInput")
    ones_d = nc.dram_tensor("ones256", (128, 2, 128), FP8, kind="ExternalInput")
    maskg_d = nc.dram_tensor("mask_g", (128, GPC), F32, kind="ExternalInput")
    bcp_d = nc.dram_tensor("bc_pos", (GPC, 128), F32, kind="ExternalInput")
    bcn_d = nc.dram_tensor("bc_neg", (GPC, 128), F32, kind="ExternalInput")
    if not trivial_gn:
        gnw_d = nc.dram_tensor("gnw", (C,), F32, kind="ExternalInput")
        gnb_d = nc.dram_tensor("gnb", (C,), F32, kind="ExternalInput")
    if qk_bias:
        bq_d = nc.dram_tensor("bq", (C,), F32, kind="ExternalInput")
        bk_d = nc.dram_tensor("bk", (C,), F32, kind="ExternalInput")
    if o_bias:
        bo_d = nc.dram_tensor("bias_o", (C,), F32, kind="ExternalInput")

    from concourse.hw_specs import get_activation_tables

    tabs = list(get_activation_tables(nc.m.arch))
    lnexp_id = tabs.index("natural_log_exp_and_others")
    nc.scalar.add_instruction(
        mybir.InstLoadActFuncSet(
            name=nc.get_next_instruction_name(),
            ins=[],
            outs=[],
            act_func_set_id=lnexp_id,
        )
    )

    x_ap = x_d.ap().rearrange("b (c p) t -> b p c t", p=128)
    y_ap = y_d.ap().rearrange("b (c p) t -> b p c t", p=128)

    singles = ctx.enter_context(tc.tile_pool(name="singles", bufs=1))
    x_pool = ctx.enter_context(tc.tile_pool(name="x", bufs=3))
    h_pool = ctx.enter_context(tc.tile_pool(name="h", bufs=2))
    q_pool = ctx.enter_context(tc.tile_pool(name="q", bufs=2))
    k_pool = ctx.enter_context(tc.tile_pool(name="k", bufs=2))
    v_pool = ctx.enter_context(tc.tile_pool(name="v", bufs=2))
    e_pool = ctx.enter_context(tc.tile_pool(name="e", bufs=2))
    on_pool = ctx.enter_context(tc.tile_pool(name="on", bufs=2))
    out_pool = ctx.enter_context(tc.tile_pool(name="out", bufs=2))
    scr_pool = ctx.enter_context(tc.tile_pool(name="scr", bufs=2))
    st_pool = ctx.enter_context(tc.tile_pool(name="st", bufs=2))
    ps_big = ctx.enter_context(tc.tile_pool(name="ps_big", bufs=3, space="PSUM"))
    ps_sm = ctx.enter_context(tc.tile_pool(name="ps_sm", bufs=2, space="PSUM"))

    xts = {}

    def load_x(b):
        xt = x_pool.tile([128, NCH, T], F32, tag="x", name=f"xt{b}")
        for ci in range(NCH):
            nc.sync.dma_start(out=xt[:, ci, :], in_=x_ap[b][:, ci, :])
        xts[b] = xt

    load_x(0)

    maskg_sb = singles.tile([128, GPC], F32, tag="maskg")
    nc.sync.dma_start(out=maskg_sb[:], in_=maskg_d.ap())
    bcp_sb = singles.tile([GPC, 128], F32, tag="bcp")
    nc.sync.dma_start(out=bcp_sb[:], in_=bcp_d.ap())
    bcn_sb = singles.tile([GPC, 128], F32, tag="bcn")
    nc.sync.dma_start(out=bcn_sb[:], in_=bcn_d.ap())
    ds_sb = singles.tile([128, 4], F32, tag="descale")
    nc.sync.dma_start(out=ds_sb[:], in_=ds_d.ap())
    eps_sb = singles.tile([GPC, 1], F32, tag="eps")
    nc.vector.memset(eps_sb[:], EPS)
    ebias_sb = singles.tile([128, 1], F32, tag="ebias")
    nc.vector.memset(ebias_sb[:], EXP_BIAS)
    ones_sb = singles.tile([128, 2, 128], FP8, tag="ones")
    nc.sync.dma_start(out=ones_sb[:], in_=ones_d.ap())
    wsb = {}
    for n in ("wqT", "wkT", "wvT", "wpT"):
        tiles = []
        for j in range(NCH // 2):
            wt = singles.tile([128, 2, C], FP8, tag=f"{n}{j}")
            nc.sync.dma_start(out=wt[:], in_=w_d[n].ap()[j])
            tiles.append(wt)
        wsb[n] = tiles
    if not trivial_gn:
        gnw_sb = singles.tile([128, NCH], F32, tag="gnw")
        gnb_sb = singles.tile([128, NCH], F32, tag="gnb")
        nc.sync.dma_start(out=gnw_sb[:], in_=gnw_d.ap().rearrange("(c p) -> p c", p=128))
        nc.sync.dma_start(out=gnb_sb[:], in_=gnb_d.ap().rearrange("(c p) -> p c", p=128))
    if qk_bias:
        bq_sb = singles.tile([128, NCH], F32, tag="bq")
        bk_sb = singles.tile([128, NCH], F32, tag="bk")
        nc.sync.dma_start(out=bq_sb[:], in_=bq_d.ap().rearrange("(c p) -> p c", p=128))
        nc.sync.dma_start(out=bk_sb[:], in_=bk_d.ap().rearrange("(c p) -> p c", p=128))
    if o_bias:
        bo_sb = singles.tile([128, NCH], F32, tag="bo")
        nc.sync.dma_start(out=bo_sb[:], in_=bo_d.ap().rearrange("(c p) -> p c", p=128))

    hts, qts, kts, vts, onts, sc_sh = {}, {}, {}, {}, {}, {}

    def emit_gn(b):
        eng = nc.gpsimd
        xt = xts[b]
        mv6 = st_pool.tile([128, NCH, 2, 6], F32, tag="mv6", name=f"mv6_{b}")
        mv = st_pool.tile([128, NCH, 2], F32, tag="mv", name=f"mv_{b}")
        for ci in range(NCH):
            for s in range(2):
                nc.vector.bn_stats(
                    out=mv6[:, ci, s, :], in_=xt[:, ci, s * 512:(s + 1) * 512]
                )
            nc.vector.bn_aggr(out=mv[:, ci, :], in_=mv6[:, ci, :, :])
        msq = st_pool.tile([128, NCH, 1], F32, tag="msq", name=f"msq_{b}")
        m2 = st_pool.tile([128, NCH, 1], F32, tag="m2", name=f"m2_{b}")
        eng.tensor_mul(msq[:], mv[:, :, 0:1], mv[:, :, 0:1])
        eng.tensor_add(m2[:], mv[:, :, 1:2], msq[:])
        gn_ps = ps_sm.tile([128, 16], F32, tag="acc", name=f"gn_ps_{b}")
        psum_g = gn_ps[0:GPC, 0:2 * NCH]
        psum_bc = gn_ps[:, 2 * NCH:4 * NCH]
        for ci in range(NCH):
            nc.tensor.matmul(
                psum_g[:, ci:ci + 1], maskg_sb[:], mv[:, ci, 0:1],
                start=True, stop=True,
            )
            nc.tensor.matmul(
                psum_g[:, NCH + ci:NCH + ci + 1], maskg_sb[:], m2[:, ci, :],
                start=True, stop=True,
            )
        gstat = st_pool.tile([GPC, 2 * NCH], F32, tag="gstat", name=f"gstat_{b}")
        nc.vector.tensor_scalar_mul(gstat[:], psum_g[:], 1.0 / 16.0)
        sqg = st_pool.tile([GPC, NCH], F32, tag="sqg", name=f"sqg_{b}")
        varg = st_pool.tile([GPC, NCH], F32, tag="varg", name=f"varg_{b}")
        eng.tensor_mul(sqg[:], gstat[:, 0:NCH], gstat[:, 0:NCH])
        eng.tensor_sub(varg[:], gstat[:, NCH:2 * NCH], sqg[:])
        lnv = st_pool.tile([GPC, NCH], F32, tag="lnv", name=f"lnv_{b}")
        nc.scalar.activation(lnv[:], varg[:], AF.Ln, bias=eps_sb[:])
        rstd = st_pool.tile([GPC, NCH], F32, tag="rstd", name=f"rstd_{b}")
        nc.scalar.activation(rstd[:], lnv[:], AF.Exp, scale=-0.5)
        for ci in range(NCH):
            nc.tensor.matmul(
                psum_bc[:, ci:ci + 1], bcn_sb[:], gstat[:, ci:ci + 1],
                start=True, stop=True,
            )
            nc.tensor.matmul(
                psum_bc[:, NCH + ci:NCH + ci + 1], bcp_sb[:], rstd[:, ci:ci + 1],
                start=True, stop=True,
            )
        bc = st_pool.tile([128, 2 * NCH], F32, tag="bc", name=f"bc_{b}")
        nc.vector.tensor_copy(bc[:], psum_bc[:])
        if trivial_gn:
            scale_t = bc[:, NCH:2 * NCH]
            shift_t = st_pool.tile([128, NCH], F32, tag="shift", name=f"shift_{b}")
            eng.tensor_mul(shift_t[:], bc[:, 0:NCH], bc[:, NCH:2 * NCH])
        else:
            scale_full = st_pool.tile([128, NCH], F32, tag="scalef", name=f"scf_{b}")
            eng.tensor_mul(scale_full[:], bc[:, NCH:2 * NCH], gnw_sb[:])
            tmp = st_pool.tile([128, NCH], F32, tag="tmpf", name=f"tmpf_{b}")
            eng.tensor_mul(tmp[:], bc[:, 0:NCH], scale_full[:])
            shift_t = st_pool.tile([128, NCH], F32, tag="shift", name=f"shift_{b}")
            eng.tensor_add(shift_t[:], tmp[:], gnb_sb[:])
            scale_t = scale_full
        sc_sh[b] = (scale_t, shift_t)

    def emit_h(b):
        scale_t, shift_t = sc_sh[b]
        xt = xts[b]
        ht = h_pool.tile([128, NCH, T], FP8, tag="h", name=f"ht_{b}")
        for ci in range(NCH):
            if ci % 2 == 0:
                nc.scalar.activation(
                    ht[:, ci, :], xt[:, ci, :], AF.Identity,
                    bias=shift_t[:, ci:ci + 1], scale=scale_t[:, ci:ci + 1],
                )
            else:
                nc.vector.tensor_scalar(
                    ht[:, ci, :], xt[:, ci, :],
                    scale_t[:, ci:ci + 1], shift_t[:, ci:ci + 1],
                    op0=mybir.AluOpType.mult, op1=mybir.AluOpType.add,
                )
        hts[b] = ht

    def emit_qkv(b):
        ht = hts[b]
        qt = q_pool.tile([128, NCH, T], FP8, tag="q", name=f"qt_{b}")
        kt = k_pool.tile([128, NCH, T], FP8, tag="k", name=f"kt_{b}")
        vt = v_pool.tile([128, NTK, C], FP8, tag="v", name=f"vt_{b}")
        for m in range(NCH):
            ps = ps_big.tile([128, T], F32, tag="big", name=f"ps_q{b}_{m}")
            for n2 in range(2):
                for j in range(NCH // 2):
                    nc.tensor.matmul(
                        ps[:, n2 * 512:(n2 + 1) * 512],
                        wsb["wqT"][j][:, :, m * 128:(m + 1) * 128],
                        ht[:, 2 * j:2 * j + 2, n2 * 512:(n2 + 1) * 512],
                        start=(j == 0), stop=(j == NCH // 2 - 1),
                        perf_mode=DR,
                    )
            if qk_bias:
                nc.scalar.activation(
                    qt[:, m, :], ps[:], AF.Identity,
                    bias=bq_sb[:, m:m + 1], scale=ds_sb[:, 0:1],
                )
            else:
                nc.scalar.mul(qt[:, m, :], ps[:], ds_sb[:, 0:1])
            ps = ps_big.tile([128, T], F32, tag="big", name=f"ps_k{b}_{m}")
            for n2 in range(2):
                for j in range(NCH // 2):
                    nc.tensor.matmul(
                        ps[:, n2 * 512:(n2 + 1) * 512],
                        wsb["wkT"][j][:, :, m * 128:(m + 1) * 128],
                        ht[:, 2 * j:2 * j + 2, n2 * 512:(n2 + 1) * 512],
                        start=(j == 0), stop=(j == NCH // 2 - 1),
                        perf_mode=DR,
                    )
            if qk_bias:
                nc.vector.tensor_scalar(
                    kt[:, m, :], ps[:], ds_sb[:, 1:2], bk_sb[:, m:m + 1],
                    op0=mybir.AluOpType.mult, op1=mybir.AluOpType.add,
                )
            else:
                nc.vector.tensor_scalar_mul(kt[:, m, :], ps[:], ds_sb[:, 1:2])
        for mp in range(NTK // 2):
            ps = ps_big.tile([128, T], F32, tag="big", name=f"ps_v{b}_{mp}")
            for half in range(2):
                m = 2 * mp + half
                for j in range(NCH // 2):
                    nc.tensor.matmul(
                        ps[:, half * 512:(half + 1) * 512],
                        ht[:, 2 * j:2 * j + 2, m * 128:(m + 1) * 128],
                        wsb["wvT"][j][:],
                        start=(j == 0), stop=(j == NCH // 2 - 1),
                        perf_mode=DR,
                    )
            if mp % 2 == 0:
                nc.scalar.mul(vt[:, 2 * mp:2 * mp + 2, :], ps[:], ds_sb[:, 2:3])
            else:
                nc.vector.tensor_scalar_mul(
                    vt[:, 2 * mp:2 * mp + 2, :], ps[:], ds_sb[:, 2:3]
                )
        qts[b], kts[b], vts[b] = qt, kt, vt

    ets = {}

    def emit_attn_s(b):
        qt, kt = qts[b], kts[b]
        et = e_pool.tile([128, NTK, T], FP8, tag="e", name=f"et_{b}")
        for tk in range(NTK):
            ps_s = ps_big.tile([128, T], F32, tag="big", name=f"ps_s{b}_{tk}")
            for n2 in range(2):
                for j in range(NCH // 2):
                    nc.tensor.matmul(
                        ps_s[:, n2 * 512:(n2 + 1) * 512],
                        kt[:, 2 * j:2 * j + 2, tk * 128:(tk + 1) * 128],
                        qt[:, 2 * j:2 * j + 2, n2 * 512:(n2 + 1) * 512],
                        start=(j == 0), stop=(j == NCH // 2 - 1),
                        perf_mode=DR,
                    )
            nc.scalar.activation(
                et[:, tk, :], ps_s[:], AF.Exp, scale=SM_SCALE, bias=ebias_sb[:]
            )
        ets[b] = et
        onts[b] = on_pool.tile([128, NCH, T], FP8, tag="on", name=f"ont_{b}")

    def emit_attn_half(b, n2):
        vt, et, ont = vts[b], ets[b], onts[b]
        tq = slice(n2 * 512, (n2 + 1) * 512)
        pc = ps_sm.tile([128, 512], F32, tag="acc", name=f"pc{b}_{n2}")
        for j in range(NTK // 2):
            nc.tensor.matmul(
                pc[:], ones_sb[:], et[:, 2 * j:2 * j + 2, tq],
                start=(j == 0), stop=(j == NTK // 2 - 1),
                perf_mode=DR,
            )
        rec = scr_pool.tile([128, 512], F32, tag="rec", name=f"rec{b}_{n2}")
        nc.vector.reciprocal_approx_fast(out=rec[:], in_=pc[:])
        rb = rec[:]
        rec2 = bass.AP(
            tensor=rb.tensor, offset=rb.offset,
            ap=[rb.ap[0], [0, 2], rb.ap[1]],
        )
        for mp in range(NCH // 2):
            po2 = ps_big.tile([128, T], F32, tag="big", name=f"po{b}_{n2}_{mp}")
            for j in range(NTK // 2):
                for mi in range(2):
                    m = 2 * mp + mi
                    nc.tensor.matmul(
                        po2[:, mi * 512:(mi + 1) * 512],
                        vt[:, 2 * j:2 * j + 2, m * 128:(m + 1) * 128],
                        et[:, 2 * j:2 * j + 2, tq],
                        start=(j == 0), stop=(j == NTK // 2 - 1),
                        perf_mode=DR,
                    )
            nc.vector.tensor_mul(ont[:, 2 * mp:2 * mp + 2, tq], po2[:], rec2)

    def emit_attn(b):
        emit_attn_s(b)
        emit_attn_half(b, 0)
        emit_attn_half(b, 1)

    out_ts = {}

    def emit_proj_half(b, n2, pool):
        ont, xt = onts[b], xts[b]
        if b not in out_ts:
            out_ts[b] = out_pool.tile([128, NCH, T], F32, tag="out",
                                      name=f"out_{b}")
        out_t = out_ts[b]
        tq = slice(n2 * 512, (n2 + 1) * 512)
        for m in range(NCH):
            ps = pool.tile([128, 512], F32,
                           tag="big" if pool is ps_big else "acc",
                           name=f"ps_p{b}_{m}_{n2}")
            for j in range(NCH // 2):
                nc.tensor.matmul(
                    ps[:],
                    wsb["wpT"][j][:, :, m * 128:(m + 1) * 128],
                    ont[:, 2 * j:2 * j + 2, tq],
                    start=(j == 0), stop=(j == NCH // 2 - 1),
                    perf_mode=DR,
                )
            nc.vector.scalar_tensor_tensor(
                out_t[:, m, tq], ps[:], ds_sb[:, 3:4], xt[:, m, tq],
                op0=mybir.AluOpType.mult, op1=mybir.AluOpType.add,
            )
            if o_bias:
                nc.vector.tensor_scalar_add(
                    out_t[:, m, tq], out_t[:, m, tq], bo_sb[:, m:m + 1]
                )
            nc.sync.dma_start(out=y_ap[b][:, m, tq], in_=out_t[:, m, tq])

    def emit_proj(b):
        ont, xt = onts[b], xts[b]
        out_t = out_pool.tile([128, NCH, T], F32, tag="out", name=f"out_{b}")
        out_ts[b] = out_t
        for m in range(NCH):
            for n2 in range(2):
                tq = slice(n2 * 512, (n2 + 1) * 512)
                ps = ps_sm.tile([128, 512], F32, tag="acc", name=f"ps_p{b}_{m}_{n2}")
                for j in range(NCH // 2):
                    nc.tensor.matmul(
                        ps[:],
                        wsb["wpT"][j][:, :, m * 128:(m + 1) * 128],
                        ont[:, 2 * j:2 * j + 2, tq],
                        start=(j == 0), stop=(j == NCH // 2 - 1),
                        perf_mode=DR,
                    )
                nc.vector.scalar_tensor_tensor(
                    out_t[:, m, tq], ps[:], ds_sb[:, 3:4], xt[:, m, tq],
                    op0=mybir.AluOpType.mult, op1=mybir.AluOpType.add,
                )
                if o_bias:
                    nc.vector.tensor_scalar_add(
                        out_t[:, m, tq], out_t[:, m, tq], bo_sb[:, m:m + 1]
                    )
            nc.sync.dma_start(out=y_ap[b][:, m, :], in_=out_t[:, m, :])

    emit_gn(0)
    emit_h(0)
    for b in range(BPC):
        emit_qkv(b)
        if b >= 1:
            emit_proj(b - 1)
        if b + 1 < BPC:
            load_x(b + 1)
            emit_gn(b + 1)
        if b == BPC - 1:
            emit_attn_s(b)
            emit_attn_half(b, 0)
            emit_proj_half(b, 0, ps_big)
            emit_attn_half(b, 1)
            emit_proj_half(b, 1, ps_big)
        else:
            emit_attn(b)
        if b + 1 < BPC:
            emit_h(b + 1)


def _build(flags):
    from contextlib import ExitStack

    nc = bacc.Bacc(
        "TRN2",
        target_bir_lowering=False,
        debug=False,
        enable_asserts=False,
        num_devices=NCORES,
    )
    with tile.TileContext(nc) as tc:
        with ExitStack() as ctx:
            _emit(nc, tc, ctx, flags)
    nc.compile()
    return nc


_CACHE = {}


def _consts():
    p = np.arange(128)
    maskg = (p[:, None] // 16 == np.arange(GPC)[None, :]).astype(np.float32)
    bcp = maskg.T.copy()
    return {
        "ones256": np.ones((128, 2, 128), mybir.dt.np(FP8)),
        "mask_g": maskg,
        "bc_pos": np.ascontiguousarray(bcp),
        "bc_neg": np.ascontiguousarray(-bcp),
    }


def kernel(x, gn_w, gn_b, wq, bq, wk, bk, wv, bv, wp, bp):
    x = np.ascontiguousarray(np.asarray(x, dtype=np.float32))
    B = x.shape[0]
    assert (B, x.shape[1], x.shape[2] * x.shape[3]) == (NCORES * BPC, C, T)
    H, W = x.shape[2], x.shape[3]
    xr = x.reshape(B, C, T)
    gn_w = np.asarray(gn_w, np.float32)
    gn_b = np.asarray(gn_b, np.float32)
    trivial_gn = bool(np.all(gn_w == 1.0) and np.all(gn_b == 0.0))
    qk_bias = bool(np.any(np.asarray(bq)) or np.any(np.asarray(bk)))
    o_bias = bool(np.any(np.asarray(bv)) or np.any(np.asarray(bp)))
    flags = (trivial_gn, qk_bias, o_bias)
    if flags not in _CACHE:
        _CACHE[flags] = _build(flags)
    nc = _CACHE[flags]

    common = dict(_consts())
    descale = np.empty(4, np.float32)
    fp8 = mybir.dt.np(FP8)
    for i, (n, w) in enumerate(
        (("wqT", wq), ("wkT", wk), ("wvT", wv), ("wpT", wp))
    ):
        wT = np.ascontiguousarray(np.asarray(w, np.float32).T)
        amax = float(np.abs(wT).max()) or 1.0
        k = int(np.floor(np.log2(88.0 / amax)))
        descale[i] = 2.0 ** (-k)
        ws = (wT * (2.0 ** k)).astype(fp8)
        common[n] = np.ascontiguousarray(
            ws.reshape(NCH // 2, 2, 128, C).transpose(0, 2, 1, 3)
        )
    common["descale"] = np.ascontiguousarray(np.broadcast_to(descale, (128, 4)))
    if not trivial_gn:
        common["gnw"] = gn_w
        common["gnb"] = gn_b
    if qk_bias:
        common["bq"] = np.asarray(bq, np.float32)
        common["bk"] = np.asarray(bk, np.float32)
    if o_bias:
        common["bias_o"] = (
            np.asarray(wp, np.float32) @ np.asarray(bv, np.float32)
            + np.asarray(bp, np.float32)
        ).astype(np.float32)

    in_maps = [
        {"x": np.ascontiguousarray(xr[c * BPC:(c + 1) * BPC]), **common}
        for c in range(NCORES)
    ]
    res = run_bass_kernel_spmd(nc, in_maps, core_ids=list(range(NCORES)))
    y = np.concatenate([res.results[c]["y"] for c in range(NCORES)], axis=0)
    return np.ascontiguousarray(y.reshape(B, C, H, W).astype(np.float32))


# revision 73
# speedup vs baseline: 2.7101x; 1.0017x over previous
import numpy as np
import ml_dtypes

import concourse.bacc as bacc
import concourse.bass as bass
import concourse.tile as tile
from concourse import mybir
from concourse.bass_utils import run_bass_kernel_spmd

F32 = mybir.dt.float32
BF16 = mybir.dt.bfloat16
FP8 = mybir.dt.float8e4
DR = mybir.MatmulPerfMode.DoubleRow
AF = mybir.ActivationFunctionType
EXP_BIAS = -3.5

NCORES = 8
BPC = 4
C = 512
T = 1024
NCH = 4
NTK = 8
GPC = 8
EPS = 1e-5
SM_SCALE = float(C) ** -0.5


def _emit(nc, tc, ctx, flags):
    trivial_gn, qk_bias, o_bias = flags

    x_d = nc.dram_tensor("x", (BPC, C, T), F32, kind="ExternalInput")
    y_d = nc.dram_tensor("y", (BPC, C, T), F32, kind="ExternalOutput")
    w_d = {
        n: nc.dram_tensor(n, (NCH // 2, 128, 2, C), FP8, kind="ExternalInput")
        for n in ("wqT", "wkT", "wvT", "wpT")
    }
    ds_d = nc.dram_tensor("descale", (128, 4), F32, kind="ExternalInput")
    ones_d = nc.dram_tensor("ones256", (128, 2, 128), FP8, kind="ExternalInput")
    maskg_d = nc.dram_tensor("mask_g", (128, GPC), F32, kind="ExternalInput")
    bcp_d = nc.dram_tensor("bc_pos", (GPC, 128), F32, kind="ExternalInput")
    bcn_d = nc.dram_tensor("bc_neg", (GPC, 128), F32, kind="ExternalInput")
    if not trivial_gn:
        gnw_d = nc.dram_tensor("gnw", (C,), F32, kind="ExternalInput")
        gnb_d = nc.dram_tensor("gnb", (C,), F32, kind="ExternalInput")
    if qk_bias:
        bq_d = nc.dram_tensor("bq", (C,), F32, kind="ExternalInput")
        bk_d = nc.dram_tensor("bk", (C,), F32, kind="ExternalInput")
    if o_bias:
        bo_d = nc.dram_tensor("bias_o", (C,), F32, kind="ExternalInput")

    from concourse.hw_specs import get_activation_tables

    tabs = list(get_activation_tables(nc.m.arch))
    lnexp_id = tabs.index("natural_log_exp_and_others")
    nc.scalar.add_instruction(
        mybir.InstLoadActFuncSet(
            name=nc.get_next_instruction_name(),
            ins=[],
            outs=[],
            act_func_set_id=lnexp_id,
        )
    )

    x_ap = x_d.ap().rearrange("b (c p) t -> b p c t", p=128)
    y_ap = y_d.ap().rearrange("b (c p) t -> b p c t", p=128)

    singles = ctx.enter_context(tc.tile_pool(name="singles", bufs=1))
    x_pool = ctx.enter_context(tc.tile_pool(name="x", bufs=3))
    h_pool = ctx.enter_context(tc.tile_pool(name="h", bufs=2))
    q_pool = ctx.enter_context(tc.tile_pool(name="q", bufs=2))
    k_pool = ctx.enter_context(tc.tile_pool(name="k", bufs=2))
    v_pool = ctx.enter_context(tc.tile_pool(name="v", bufs=2))
    e_pool = ctx.enter_context(tc.tile_pool(name="e", bufs=2))
    on_pool = ctx.enter_context(tc.tile_pool(name="on", bufs=2))
    out_pool = ctx.enter_context(tc.tile_pool(name="out", bufs=2))
    scr_pool = ctx.enter_context(tc.tile_pool(name="scr", bufs=2))
    st_pool = ctx.enter_context(tc.tile_pool(name="st", bufs=2))
    ps_big = ctx.enter_context(tc.tile_pool(name="ps_big", bufs=3, space="PSUM"))
    ps_sm = ctx.enter_context(tc.tile_pool(name="ps_sm", bufs=2, space="PSUM"))

    xts = {}

    def load_x(b):
        xt = x_pool.tile([128, NCH, T], F32, tag="x", name=f"xt{b}")
        for ci in range(NCH):
            nc.sync.dma_start(out=xt[:, ci, :], in_=x_ap[b][:, ci, :])
        xts[b] = xt

    load_x(0)

    maskg_sb = singles.tile([128, GPC], F32, tag="maskg")
    nc.sync.dma_start(out=maskg_sb[:], in_=maskg_d.ap())
    bcp_sb = singles.tile([GPC, 128], F32, tag="bcp")
    nc.sync.dma_start(out=bcp_sb[:], in_=bcp_d.ap())
    bcn_sb = singles.tile([GPC, 128], F32, tag="bcn")
    nc.sync.dma_start(out=bcn_sb[:], in_=bcn_d.ap())
    ds_sb = singles.tile([128, 4], F32, tag="descale")
    nc.sync.dma_start(out=ds_sb[:], in_=ds_d.ap())
    eps_sb = singles.tile([GPC, 1], F32, tag="eps")
    nc.vector.memset(eps_sb[:], EPS)
    ebias_sb = singles.tile([128, 1], F32, tag="ebias")
    nc.vector.memset(ebias_sb[:], EXP_BIAS)
    ones_sb = singles.tile([128, 2, 128], FP8, tag="ones")
    nc.sync.dma_start(out=ones_sb[:], in_=ones_d.ap())
    wsb = {}
    for n in ("wqT", "wkT", "wvT", "wpT"):
        tiles = []
        for j in range(NCH // 2):
            wt = singles.tile([128, 2, C], FP8, tag=f"{n}{j}")
            nc.sync.dma_start(out=wt[:], in_=w_d[n].ap()[j])
            tiles.append(wt)
        wsb[n] = tiles
    if not trivial_gn:
        gnw_sb = singles.tile([128, NCH], F32, tag="gnw")
        gnb_sb = singles.tile([128, NCH], F32, tag="gnb")
        nc.sync.dma_start(out=gnw_sb[:], in_=gnw_d.ap().rearrange("(c p) -> p c", p=128))
        nc.sync.dma_start(out=gnb_sb[:], in_=gnb_d.ap().rearrange("(c p) -> p c", p=128))
    if qk_bias:
        bq_sb = singles.tile([128, NCH], F32, tag="bq")
        bk_sb = singles.tile([128, NCH], F32, tag="bk")
        nc.sync.dma_start(out=bq_sb[:], in_=bq_d.ap().rearrange("(c p) -> p c", p=128))
        nc.sync.dma_start(out=bk_sb[:], in_=bk_d.ap().rearrange("(c p) -> p c", p=128))
    if o_bias:
        bo_sb = singles.tile([128, NCH], F32, tag="bo")
        nc.sync.dma_start(out=bo_sb[:], in_=bo_d.ap().rearrange("(c p) -> p c", p=128))

    hts, qts, kts, vts, onts, sc_sh = {}, {}, {}, {}, {}, {}

    def emit_gn(b):
        eng = nc.gpsimd
        xt = xts[b]
        mv6 = st_pool.tile([128, NCH, 2, 6], F32, tag="mv6", name=f"mv6_{b}")
        mv = st_pool.tile([128, NCH, 2], F32, tag="mv", name=f"mv_{b}")
        for ci in range(NCH):
            for s in range(2):
                nc.vector.bn_stats(
                    out=mv6[:, ci, s, :], in_=xt[:, ci, s * 512:(s + 1) * 512]
                )
            nc.vector.bn_aggr(out=mv[:, ci, :], in_=mv6[:, ci, :, :])
        msq = st_pool.tile([128, NCH, 1], F32, tag="msq", name=f"msq_{b}")
        m2 = st_pool.tile([128, NCH, 1], F32, tag="m2", name=f"m2_{b}")
        eng.tensor_mul(msq[:], mv[:, :, 0:1], mv[:, :, 0:1])
        eng.tensor_add(m2[:], mv[:, :, 1:2], msq[:])
        gn_ps = ps_sm.tile([128, 16], F32, tag="acc", name=f"gn_ps_{b}")
        psum_g = gn_ps[0:GPC, 0:2 * NCH]
        psum_bc = gn_ps[:, 2 * NCH:4 * NCH]
        for ci in range(NCH):
            nc.tensor.matmul(
                psum_g[:, ci:ci + 1], maskg_sb[:], mv[:, ci, 0:1],
                start=True, stop=True,
            )
            nc.tensor.matmul(
                psum_g[:, NCH + ci:NCH + ci + 1], maskg_sb[:], m2[:, ci, :],
                start=True, stop=True,
            )
        gstat = st_pool.tile([GPC, 2 * NCH], F32, tag="gstat", name=f"gstat_{b}")
        nc.vector.tensor_scalar_mul(gstat[:], psum_g[:], 1.0 / 16.0)
        sqg = st_pool.tile([GPC, NCH], F32, tag="sqg", name=f"sqg_{b}")
        varg = st_pool.tile([GPC, NCH], F32, tag="varg", name=f"varg_{b}")
        eng.tensor_mul(sqg[:], gstat[:, 0:NCH], gstat[:, 0:NCH])
        eng.tensor_sub(varg[:], gstat[:, NCH:2 * NCH], sqg[:])
        lnv = st_pool.tile([GPC, NCH], F32, tag="lnv", name=f"lnv_{b}")
        nc.scalar.activation(lnv[:], varg[:], AF.Ln, bias=eps_sb[:])
        rstd = st_pool.tile([GPC, NCH], F32, tag="rstd", name=f"rstd_{b}")
        nc.scalar.activation(rstd[:], lnv[:], AF.Exp, scale=-0.5)
        for ci in range(NCH):
            nc.tensor.matmul(
                psum_bc[:, ci:ci + 1], bcn_sb[:], gstat[:, ci:ci + 1],
                start=True, stop=True,
            )
            nc.tensor.matmul(
                psum_bc[:, NCH + ci:NCH + ci + 1], bcp_sb[:], rstd[:, ci:ci + 1],
                start=True, stop=True,
            )
        bc = st_pool.tile([128, 2 * NCH], F32, tag="bc", name=f"bc_{b}")
        nc.vector.tensor_copy(bc[:], psum_bc[:])
        if trivial_gn:
            scale_t = bc[:, NCH:2 * NCH]
            shift_t = st_pool.tile([128, NCH], F32, tag="shift", name=f"shift_{b}")
            eng.tensor_mul(shift_t[:], bc[:, 0:NCH], bc[:, NCH:2 * NCH])
        else:
            scale_full = st_pool.tile([128, NCH], F32, tag="scalef", name=f"scf_{b}")
            eng.tensor_mul(scale_full[:], bc[:, NCH:2 * NCH], gnw_sb[:])
            tmp = st_pool.tile([128, NCH], F32, tag="tmpf", name=f"tmpf_{b}")
            eng.tensor_mul(tmp[:], bc[:, 0:NCH], scale_full[:])
            shift_t = st_pool.tile([128, NCH], F32, tag="shift", name=f"shift_{b}")
            eng.tensor_add(shift_t[:], tmp[:], gnb_sb[:])
            scale_t = scale_full
        sc_sh[b] = (scale_t, shift_t)

    def emit_h(b):
        scale_t, shift_t = sc_sh[b]
        xt = xts[b]
        ht = h_pool.tile([128, NCH, T], FP8, tag="h", name=f"ht_{b}")
        for ci in range(NCH):
            nc.scalar.activation(
                ht[:, ci, :], xt[:, ci, :], AF.Identity,
                bias=shift_t[:, ci:ci + 1], scale=scale_t[:, ci:ci + 1],
            )
        hts[b] = ht

    def emit_qkv(b):
        ht = hts[b]
        qt = q_pool.tile([128, NCH, T], FP8, tag="q", name=f"qt_{b}")
        kt = k_pool.tile([128, NCH, T], FP8, tag="k", name=f"kt_{b}")
        vt = v_pool.tile([128, NTK, C], FP8, tag="v", name=f"vt_{b}")
        for m in range(NCH):
            ps = ps_big.tile([128, T], F32, tag="big", name=f"ps_q{b}_{m}")
            for n2 in range(2):
                for j in range(NCH // 2):
                    nc.tensor.matmul(
                        ps[:, n2 * 512:(n2 + 1) * 512],
                        wsb["wqT"][j][:, :, m * 128:(m + 1) * 128],
                        ht[:, 2 * j:2 * j + 2, n2 * 512:(n2 + 1) * 512],
                        start=(j == 0), stop=(j == NCH // 2 - 1),
                        perf_mode=DR,
                    )
            if qk_bias:
                nc.scalar.activation(
                    qt[:, m, :], ps[:], AF.Identity,
                    bias=bq_sb[:, m:m + 1], scale=ds_sb[:, 0:1],
                )
            else:
                nc.scalar.mul(qt[:, m, :], ps[:], ds_sb[:, 0:1])
            ps = ps_big.tile([128, T], F32, tag="big", name=f"ps_k{b}_{m}")
            for n2 in range(2):
                for j in range(NCH // 2):
                    nc.tensor.matmul(
                        ps[:, n2 * 512:(n2 + 1) * 512],
                        wsb["wkT"][j][:, :, m * 128:(m + 1) * 128],
                        ht[:, 2 * j:2 * j + 2, n2 * 512:(n2 + 1) * 512],
                        start=(j == 0), stop=(j == NCH // 2 - 1),
                        perf_mode=DR,
                    )
            if qk_bias:
                nc.vector.tensor_scalar(
                    kt[:, m, :], ps[:], ds_sb[:, 1:2], bk_sb[:, m:m + 1],
                    op0=mybir.AluOpType.mult, op1=mybir.AluOpType.add,
                )
            else:
                nc.vector.tensor_scalar_mul(kt[:, m, :], ps[:], ds_sb[:, 1:2])
        for mp in range(NTK // 2):
            ps = ps_big.tile([128, T], F32, tag="big", name=f"ps_v{b}_{mp}")
            for half in range(2):
                m = 2 * mp + half
                for j in range(NCH // 2):
                    nc.tensor.matmul(
                        ps[:, half * 512:(half + 1) * 512],
                        ht[:, 2 * j:2 * j + 2, m * 128:(m + 1) * 128],
                        wsb["wvT"][j][:],
                        start=(j == 0), stop=(j == NCH // 2 - 1),
                        perf_mode=DR,
                    )
            if mp % 2 == 0:
                nc.scalar.mul(vt[:, 2 * mp:2 * mp + 2, :], ps[:], ds_sb[:, 2:3])
            else:
                nc.vector.tensor_scalar_mul(
                    vt[:, 2 * mp:2 * mp + 2, :], ps[:], ds_sb[:, 2:3]
                )
        qts[b], kts[b], vts[b] = qt, kt, vt

    ets = {}

    def emit_attn_s(b):
        qt, kt = qts[b], kts[b]
        et = e_pool.tile([128, NTK, T], FP8, tag="e", name=f"et_{b}")
        for tk in range(NTK):
            ps_s = ps_big.tile([128, T], F32, tag="big", name=f"ps_s{b}_{tk}")
            for n2 in range(2):
                for j in range(NCH // 2):
                    nc.tensor.matmul(
                        ps_s[:, n2 * 512:(n2 + 1) * 512],
                        kt[:, 2 * j:2 * j + 2, tk * 128:(tk + 1) * 128],
                        qt[:, 2 * j:2 * j + 2, n2 * 512:(n2 + 1) * 512],
                        start=(j == 0), stop=(j == NCH // 2 - 1),
                        perf_mode=DR,
                    )
            nc.scalar.activation(
                et[:, tk, :], ps_s[:], AF.Exp, scale=SM_SCALE, bias=ebias_sb[:]
            )
        ets[b] = et
        onts[b] = on_pool.tile([128, NCH, T], FP8, tag="on", name=f"ont_{b}")

    def emit_attn_half(b, n2):
        vt, et, ont = vts[b], ets[b], onts[b]
        tq = slice(n2 * 512, (n2 + 1) * 512)
        pc = ps_sm.tile([128, 512], F32, tag="acc", name=f"pc{b}_{n2}")
        for j in range(NTK // 2):
            nc.tensor.matmul(
                pc[:], ones_sb[:], et[:, 2 * j:2 * j + 2, tq],
                start=(j == 0), stop=(j == NTK // 2 - 1),
                perf_mode=DR,
            )
        rec = scr_pool.tile([128, 512], F32, tag="rec", name=f"rec{b}_{n2}")
        nc.vector.reciprocal_approx_fast(out=rec[:], in_=pc[:])
        rb = rec[:]
        rec2 = bass.AP(
            tensor=rb.tensor, offset=rb.offset,
            ap=[rb.ap[0], [0, 2], rb.ap[1]],
        )
        for mp in range(NCH // 2):
            po2 = ps_big.tile([128, T], F32, tag="big", name=f"po{b}_{n2}_{mp}")
            for j in range(NTK // 2):
                for mi in range(2):
                    m = 2 * mp + mi
                    nc.tensor.matmul(
                        po2[:, mi * 512:(mi + 1) * 512],
                        vt[:, 2 * j:2 * j + 2, m * 128:(m + 1) * 128],
                        et[:, 2 * j:2 * j + 2, tq],
                        start=(j == 0), stop=(j == NTK // 2 - 1),
                        perf_mode=DR,
                    )
            nc.vector.tensor_mul(ont[:, 2 * mp:2 * mp + 2, tq], po2[:], rec2)

    def emit_attn(b):
        emit_attn_s(b)
        emit_attn_half(b, 0)
        emit_attn_half(b, 1)

    out_ts = {}

    def emit_proj_half(b, n2, pool):
        ont, xt = onts[b], xts[b]
        if b not in out_ts:
            out_ts[b] = out_pool.tile([128, NCH, T], F32, tag="out",
                                      name=f"out_{b}")
        out_t = out_ts[b]
        tq = slice(n2 * 512, (n2 + 1) * 512)
        for m in range(NCH):
            ps = pool.tile([128, 512], F32,
                           tag="big" if pool is ps_big else "acc",
                           name=f"ps_p{b}_{m}_{n2}")
            for j in range(NCH // 2):
                nc.tensor.matmul(
                    ps[:],
                    wsb["wpT"][j][:, :, m * 128:(m + 1) * 128],
                    ont[:, 2 * j:2 * j + 2, tq],
                    start=(j == 0), stop=(j == NCH // 2 - 1),
                    perf_mode=DR,
                )
            nc.vector.scalar_tensor_tensor(
                out_t[:, m, tq], ps[:], ds_sb[:, 3:4], xt[:, m, tq],
                op0=mybir.AluOpType.mult, op1=mybir.AluOpType.add,
            )
            if o_bias:
                nc.vector.tensor_scalar_add(
                    out_t[:, m, tq], out_t[:, m, tq], bo_sb[:, m:m + 1]
                )
            nc.sync.dma_start(out=y_ap[b][:, m, tq], in_=out_t[:, m, tq])

    def emit_proj(b):
        ont, xt = onts[b], xts[b]
        out_t = out_pool.tile([128, NCH, T], F32, tag="out", name=f"out_{b}")
        out_ts[b] = out_t
        for m in range(NCH):
            for n2 in range(2):
                tq = slice(n2 * 512, (n2 + 1) * 512)
                ps = ps_sm.tile([128, 512], F32, tag="acc", name=f"ps_p{b}_{m}_{n2}")
                for j in range(NCH // 2):
                    nc.tensor.matmul(
                        ps[:],
                        wsb["wpT"][j][:, :, m * 128:(m + 1) * 128],
                        ont[:, 2 * j:2 * j + 2, tq],
                        start=(j == 0), stop=(j == NCH // 2 - 1),
                        perf_mode=DR,
                    )
                nc.vector.scalar_tensor_tensor(
                    out_t[:, m, tq], ps[:], ds_sb[:, 3:4], xt[:, m, tq],
                    op0=mybir.AluOpType.mult, op1=mybir.AluOpType.add,
                )
                if o_bias:
                    nc.vector.tensor_scalar_add(
                        out_t[:, m, tq], out_t[:, m, tq], bo_sb[:, m:m + 1]
                    )
            nc.sync.dma_start(out=y_ap[b][:, m, :], in_=out_t[:, m, :])

    emit_gn(0)
    emit_h(0)
    for b in range(BPC):
        emit_qkv(b)
        if b >= 1:
            emit_proj(b - 1)
        if b + 1 < BPC:
            load_x(b + 1)
            emit_gn(b + 1)
        if b == BPC - 1:
            emit_attn_s(b)
            emit_attn_half(b, 0)
            emit_proj_half(b, 0, ps_big)
            emit_attn_half(b, 1)
            emit_proj_half(b, 1, ps_big)
        else:
            emit_attn(b)
        if b + 1 < BPC:
            emit_h(b + 1)


def _build(flags):
    from contextlib import ExitStack

    nc = bacc.Bacc(
        "TRN2",
        target_bir_lowering=False,
        debug=False,
        enable_asserts=False,
        num_devices=NCORES,
    )
    with tile.TileContext(nc) as tc:
        with ExitStack() as ctx:
            _emit(nc, tc, ctx, flags)
    nc.compile()
    return nc


_CACHE = {}


def _consts():
    p = np.arange(128)
    maskg = (p[:, None] // 16 == np.arange(GPC)[None, :]).astype(np.float32)
    bcp = maskg.T.copy()
    return {
        "ones256": np.ones((128, 2, 128), mybir.dt.np(FP8)),
        "mask_g": maskg,
        "bc_pos": np.ascontiguousarray(bcp),
        "bc_neg": np.ascontiguousarray(-bcp),
    }


def kernel(x, gn_w, gn_b, wq, bq, wk, bk, wv, bv, wp, bp):
    x = np.ascontiguousarray(np.asarray(x, dtype=np.float32))
    B = x.shape[0]
    assert (B, x.shape[1], x.shape[2] * x.shape[3]) == (NCORES * BPC, C, T)
    H, W = x.shape[2], x.shape[3]
    xr = x.reshape(B, C, T)
    gn_w = np.asarray(gn_w, np.float32)
    gn_b = np.asarray(gn_b, np.float32)
    trivial_gn = bool(np.all(gn_w == 1.0) and np.all(gn_b == 0.0))
    qk_bias = bool(np.any(np.asarray(bq)) or np.any(np.asarray(bk)))
    o_bias = bool(np.any(np.asarray(bv)) or np.any(np.asarray(bp)))
    flags = (trivial_gn, qk_bias, o_bias)
    if flags not in _CACHE:
        _CACHE[flags] = _build(flags)
    nc = _CACHE[flags]

    common = dict(_consts())
    descale = np.empty(4, np.float32)
    fp8 = mybir.dt.np(FP8)
    for i, (n, w) in enumerate(
        (("wqT", wq), ("wkT", wk), ("wvT", wv), ("wpT", wp))
    ):
        wT = np.ascontiguousarray(np.asarray(w, np.float32).T)
        amax = float(np.abs(wT).max()) or 1.0
        k = int(np.floor(np.log2(88.0 / amax)))
        descale[i] = 2.0 ** (-k)
        ws = (wT * (2.0 ** k)).astype(fp8)
        common[n] = np.ascontiguousarray(
            ws.reshape(NCH // 2, 2, 128, C).transpose(0, 2, 1, 3)
        )
    common["descale"] = np.ascontiguousarray(np.broadcast_to(descale, (128, 4)))
    if not trivial_gn:
        common["gnw"] = gn_w
        common["gnb"] = gn_b
    if qk_bias:
        common["bq"] = np.asarray(bq, np.float32)
        common["bk"] = np.asarray(bk, np.float32)
    if o_bias:
        common["bias_o"] = (
            np.asarray(wp, np.float32) @ np.asarray(bv, np.float32)
            + np.asarray(bp, np.float32)
        ).astype(np.float32)

    in_maps = [
        {"x": np.ascontiguousarray(xr[c * BPC:(c + 1) * BPC]), **common}
        for c in range(NCORES)
    ]
    res = run_bass_kernel_spmd(nc, in_maps, core_ids=list(range(NCORES)))
    y = np.concatenate([res.results[c]["y"] for c in range(NCORES)], axis=0)
    return np.ascontiguousarray(y.reshape(B, C, H, W).astype(np.float32))


# revision 78
# speedup vs baseline: 2.7473x; 1.0137x over previous
import numpy as np
import ml_dtypes

import concourse.bacc as bacc
import concourse.bass as bass
import concourse.tile as tile
from concourse import mybir
from concourse.bass_utils import run_bass_kernel_spmd

F32 = mybir.dt.float32
BF16 = mybir.dt.bfloat16
FP8 = mybir.dt.float8e4
DR = mybir.MatmulPerfMode.DoubleRow
AF = mybir.ActivationFunctionType
EXP_BIAS = -3.5

NCORES = 8
BPC = 4
C = 512
T = 1024
NCH = 4
NTK = 8
GPC = 8
EPS = 1e-5
SM_SCALE = float(C) ** -0.5


def _emit(nc, tc, ctx, flags):
    trivial_gn, qk_bias, o_bias = flags

    x_d = nc.dram_tensor("x", (BPC, C, T), F32, kind="ExternalInput")
    y_d = nc.dram_tensor("y", (BPC, C, T), F32, kind="ExternalOutput")
    w_d = {
        n: nc.dram_tensor(n, (NCH // 2, 128, 2, C), FP8, kind="ExternalInput")
        for n in ("wqT", "wkT", "wvT", "wpT")
    }
    ds_d = nc.dram_tensor("descale", (128, 4), F32, kind="ExternalInput")
    ones_d = nc.dram_tensor("ones256", (128, 2, 128), FP8, kind="ExternalInput")
    maskg_d = nc.dram_tensor("mask_g", (128, GPC), F32, kind="ExternalInput")
    bcp_d = nc.dram_tensor("bc_pos", (GPC, 128), F32, kind="ExternalInput")
    bcn_d = nc.dram_tensor("bc_neg", (GPC, 128), F32, kind="ExternalInput")
    if not trivial_gn:
        gnw_d = nc.dram_tensor("gnw", (C,), F32, kind="ExternalInput")
        gnb_d = nc.dram_tensor("gnb", (C,), F32, kind="ExternalInput")
    if qk_bias:
        bq_d = nc.dram_tensor("bq", (C,), F32, kind="ExternalInput")
        bk_d = nc.dram_tensor("bk", (C,), F32, kind="ExternalInput")
    if o_bias:
        bo_d = nc.dram_tensor("bias_o", (C,), F32, kind="ExternalInput")

    from concourse.hw_specs import get_activation_tables

    tabs = list(get_activation_tables(nc.m.arch))
    lnexp_id = tabs.index("natural_log_exp_and_others")
    nc.scalar.add_instruction(
        mybir.InstLoadActFuncSet(
            name=nc.get_next_instruction_name(),
            ins=[],
            outs=[],
            act_func_set_id=lnexp_id,
        )
    )

    x_ap = x_d.ap().rearrange("b (c p) t -> b p c t", p=128)
    y_ap = y_d.ap().rearrange("b (c p) t -> b p c t", p=128)

    singles = ctx.enter_context(tc.tile_pool(name="singles", bufs=1))
    x_pool = ctx.enter_context(tc.tile_pool(name="x", bufs=3))
    h_pool = ctx.enter_context(tc.tile_pool(name="h", bufs=2))
    q_pool = ctx.enter_context(tc.tile_pool(name="q", bufs=2))
    k_pool = ctx.enter_context(tc.tile_pool(name="k", bufs=2))
    v_pool = ctx.enter_context(tc.tile_pool(name="v", bufs=2))
    e_pool = ctx.enter_context(tc.tile_pool(name="e", bufs=2))
    on_pool = ctx.enter_context(tc.tile_pool(name="on", bufs=2))
    out_pool = ctx.enter_context(tc.tile_pool(name="out", bufs=2))
    scr_pool = ctx.enter_context(tc.tile_pool(name="scr", bufs=2))
    st_pool = ctx.enter_context(tc.tile_pool(name="st", bufs=2))
    ps_big = ctx.enter_context(tc.tile_pool(name="ps_big", bufs=3, space="PSUM"))
    ps_sm = ctx.enter_context(tc.tile_pool(name="ps_sm", bufs=2, space="PSUM"))

    xts = {}

    def load_x(b):
        xt = x_pool.tile([128, NCH, T], F32, tag="x", name=f"xt{b}")
        for ci in range(NCH):
            nc.sync.dma_start(out=xt[:, ci, :], in_=x_ap[b][:, ci, :])
        xts[b] = xt

    load_x(0)

    maskg_sb = singles.tile([128, GPC], F32, tag="maskg")
    nc.sync.dma_start(out=maskg_sb[:], in_=maskg_d.ap())
    bcp_sb = singles.tile([GPC, 128], F32, tag="bcp")
    nc.sync.dma_start(out=bcp_sb[:], in_=bcp_d.ap())
    bcn_sb = singles.tile([GPC, 128], F32, tag="bcn")
    nc.sync.dma_start(out=bcn_sb[:], in_=bcn_d.ap())
    ds_sb = singles.tile([128, 4], F32, tag="descale")
    nc.sync.dma_start(out=ds_sb[:], in_=ds_d.ap())
    eps_sb = singles.tile([GPC, 1], F32, tag="eps")
    nc.vector.memset(eps_sb[:], EPS)
    ebias_sb = singles.tile([128, 1], F32, tag="ebias")
    nc.vector.memset(ebias_sb[:], EXP_BIAS)
    ones_sb = singles.tile([128, 2, 128], FP8, tag="ones")
    nc.sync.dma_start(out=ones_sb[:], in_=ones_d.ap())
    wsb = {}
    for n in ("wqT", "wkT", "wvT", "wpT"):
        tiles = []
        for j in range(NCH // 2):
            wt = singles.tile([128, 2, C], FP8, tag=f"{n}{j}")
            nc.sync.dma_start(out=wt[:], in_=w_d[n].ap()[j])
            tiles.append(wt)
        wsb[n] = tiles
    if not trivial_gn:
        gnw_sb = singles.tile([128, NCH], F32, tag="gnw")
        gnb_sb = singles.tile([128, NCH], F32, tag="gnb")
        nc.sync.dma_start(out=gnw_sb[:], in_=gnw_d.ap().rearrange("(c p) -> p c", p=128))
        nc.sync.dma_start(out=gnb_sb[:], in_=gnb_d.ap().rearrange("(c p) -> p c", p=128))
    if qk_bias:
        bq_sb = singles.tile([128, NCH], F32, tag="bq")
        bk_sb = singles.tile([128, NCH], F32, tag="bk")
        nc.sync.dma_start(out=bq_sb[:], in_=bq_d.ap().rearrange("(c p) -> p c", p=128))
        nc.sync.dma_start(out=bk_sb[:], in_=bk_d.ap().rearrange("(c p) -> p c", p=128))
    if o_bias:
        bo_sb = singles.tile([128, NCH], F32, tag="bo")
        nc.sync.dma_start(out=bo_sb[:], in_=bo_d.ap().rearrange("(c p) -> p c", p=128))

    hts, qts, kts, vts, onts, sc_sh = {}, {}, {}, {}, {}, {}

    def emit_gn(b):
        eng = nc.gpsimd
        xt = xts[b]
        mv6 = st_pool.tile([128, NCH, 2, 6], F32, tag="mv6", name=f"mv6_{b}")
        mv = st_pool.tile([128, NCH, 2], F32, tag="mv", name=f"mv_{b}")
        for ci in range(NCH):
            for s in range(2):
                nc.vector.bn_stats(
                    out=mv6[:, ci, s, :], in_=xt[:, ci, s * 512:(s + 1) * 512]
                )
            nc.vector.bn_aggr(out=mv[:, ci, :], in_=mv6[:, ci, :, :])
        msq = st_pool.tile([128, NCH, 1], F32, tag="msq", name=f"msq_{b}")
        m2 = st_pool.tile([128, NCH, 1], F32, tag="m2", name=f"m2_{b}")
        eng.tensor_mul(msq[:], mv[:, :, 0:1], mv[:, :, 0:1])
        eng.tensor_add(m2[:], mv[:, :, 1:2], msq[:])
        gn_ps = ps_sm.tile([128, 16], F32, tag="acc", name=f"gn_ps_{b}")
        psum_g = gn_ps[0:GPC, 0:2 * NCH]
        psum_bc = gn_ps[:, 2 * NCH:4 * NCH]
        for ci in range(NCH):
            nc.tensor.matmul(
                psum_g[:, ci:ci + 1], maskg_sb[:], mv[:, ci, 0:1],
                start=True, stop=True,
            )
            nc.tensor.matmul(
                psum_g[:, NCH + ci:NCH + ci + 1], maskg_sb[:], m2[:, ci, :],
                start=True, stop=True,
            )
        gstat = st_pool.tile([GPC, 2 * NCH], F32, tag="gstat", name=f"gstat_{b}")
        nc.vector.tensor_scalar_mul(gstat[:], psum_g[:], 1.0 / 16.0)
        sqg = st_pool.tile([GPC, NCH], F32, tag="sqg", name=f"sqg_{b}")
        varg = st_pool.tile([GPC, NCH], F32, tag="varg", name=f"varg_{b}")
        eng.tensor_mul(sqg[:], gstat[:, 0:NCH], gstat[:, 0:NCH])
        eng.tensor_sub(varg[:], gstat[:, NCH:2 * NCH], sqg[:])
        lnv = st_pool.tile([GPC, NCH], F32, tag="lnv", name=f"lnv_{b}")
        nc.scalar.activation(lnv[:], varg[:], AF.Ln, bias=eps_sb[:])
        rstd = st_pool.tile([GPC, NCH], F32, tag="rstd", name=f"rstd_{b}")
        nc.scalar.activation(rstd[:], lnv[:], AF.Exp, scale=-0.5)
        for ci in range(NCH):
            nc.tensor.matmul(
                psum_bc[:, ci:ci + 1], bcn_sb[:], gstat[:, ci:ci + 1],
                start=True, stop=True,
            )
            nc.tensor.matmul(
                psum_bc[:, NCH + ci:NCH + ci + 1], bcp_sb[:], rstd[:, ci:ci + 1],
                start=True, stop=True,
            )
        bc = st_pool.tile([128, 2 * NCH], F32, tag="bc", name=f"bc_{b}")
        nc.vector.tensor_copy(bc[:], psum_bc[:])
        if trivial_gn:
            scale_t = bc[:, NCH:2 * NCH]
            shift_t = st_pool.tile([128, NCH], F32, tag="shift", name=f"shift_{b}")
            eng.tensor_mul(shift_t[:], bc[:, 0:NCH], bc[:, NCH:2 * NCH])
        else:
            scale_full = st_pool.tile([128, NCH], F32, tag="scalef", name=f"scf_{b}")
            eng.tensor_mul(scale_full[:], bc[:, NCH:2 * NCH], gnw_sb[:])
            tmp = st_pool.tile([128, NCH], F32, tag="tmpf", name=f"tmpf_{b}")
            eng.tensor_mul(tmp[:], bc[:, 0:NCH], scale_full[:])
            shift_t = st_pool.tile([128, NCH], F32, tag="shift", name=f"shift_{b}")
            eng.tensor_add(shift_t[:], tmp[:], gnb_sb[:])
            scale_t = scale_full
        sc_sh[b] = (scale_t, shift_t)

    def emit_h(b):
        scale_t, shift_t = sc_sh[b]
        xt = xts[b]
        ht = h_pool.tile([128, NCH, T], FP8, tag="h", name=f"ht_{b}")
        for ci in range(NCH):
            nc.scalar.activation(
                ht[:, ci, :], xt[:, ci, :], AF.Identity,
                bias=shift_t[:, ci:ci + 1], scale=scale_t[:, ci:ci + 1],
            )
        hts[b] = ht

    def emit_qkv(b):
        ht = hts[b]
        qt = q_pool.tile([128, NCH, T], FP8, tag="q", name=f"qt_{b}")
        kt = k_pool.tile([128, NCH, T], FP8, tag="k", name=f"kt_{b}")
        vt = v_pool.tile([128, NTK, C], FP8, tag="v", name=f"vt_{b}")
        for m in range(NCH):
            ps = ps_big.tile([128, T], F32, tag="big", name=f"ps_q{b}_{m}")
            for n2 in range(2):
                for j in range(NCH // 2):
                    nc.tensor.matmul(
                        ps[:, n2 * 512:(n2 + 1) * 512],
                        wsb["wqT"][j][:, :, m * 128:(m + 1) * 128],
                        ht[:, 2 * j:2 * j + 2, n2 * 512:(n2 + 1) * 512],
                        start=(j == 0), stop=(j == NCH // 2 - 1),
                        perf_mode=DR,
                    )
            if qk_bias:
                nc.scalar.activation(
                    qt[:, m, :], ps[:], AF.Identity,
                    bias=bq_sb[:, m:m + 1], scale=ds_sb[:, 0:1],
                )
            else:
                nc.scalar.mul(qt[:, m, :], ps[:], ds_sb[:, 0:1])
            ps = ps_big.tile([128, T], F32, tag="big", name=f"ps_k{b}_{m}")
            for n2 in range(2):
                for j in range(NCH // 2):
                    nc.tensor.matmul(
                        ps[:, n2 * 512:(n2 + 1) * 512],
                        wsb["wkT"][j][:, :, m * 128:(m + 1) * 128],
                        ht[:, 2 * j:2 * j + 2, n2 * 512:(n2 + 1) * 512],
                        start=(j == 0), stop=(j == NCH // 2 - 1),
                        perf_mode=DR,
                    )
            if qk_bias:
                nc.vector.tensor_scalar(
                    kt[:, m, :], ps[:], ds_sb[:, 1:2], bk_sb[:, m:m + 1],
                    op0=mybir.AluOpType.mult, op1=mybir.AluOpType.add,
                )
            else:
                nc.vector.tensor_scalar_mul(kt[:, m, :], ps[:], ds_sb[:, 1:2])
        for mp in range(NTK // 2):
            ps = ps_big.tile([128, T], F32, tag="big", name=f"ps_v{b}_{mp}")
            for half in range(2):
                m = 2 * mp + half
                for j in range(NCH // 2):
                    nc.tensor.matmul(
                        ps[:, half * 512:(half + 1) * 512],
                        ht[:, 2 * j:2 * j + 2, m * 128:(m + 1) * 128],
                        wsb["wvT"][j][:],
                        start=(j == 0), stop=(j == NCH // 2 - 1),
                        perf_mode=DR,
                    )
            if mp % 2 == 0:
                nc.scalar.mul(vt[:, 2 * mp:2 * mp + 2, :], ps[:], ds_sb[:, 2:3])
            else:
                nc.vector.tensor_scalar_mul(
                    vt[:, 2 * mp:2 * mp + 2, :], ps[:], ds_sb[:, 2:3]
                )
        qts[b], kts[b], vts[b] = qt, kt, vt

    ets = {}

    def emit_attn_s(b):
        qt, kt = qts[b], kts[b]
        et = e_pool.tile([128, NTK, T], FP8, tag="e", name=f"et_{b}")
        for tk in range(NTK):
            ps_s = ps_big.tile([128, T], F32, tag="big", name=f"ps_s{b}_{tk}")
            for n2 in range(2):
                for j in range(NCH // 2):
                    nc.tensor.matmul(
                        ps_s[:, n2 * 512:(n2 + 1) * 512],
                        kt[:, 2 * j:2 * j + 2, tk * 128:(tk + 1) * 128],
                        qt[:, 2 * j:2 * j + 2, n2 * 512:(n2 + 1) * 512],
                        start=(j == 0), stop=(j == NCH // 2 - 1),
                        perf_mode=DR,
                    )
            nc.scalar.activation(
                et[:, tk, :], ps_s[:], AF.Exp, scale=SM_SCALE, bias=ebias_sb[:]
            )
        ets[b] = et
        onts[b] = on_pool.tile([128, NCH, T], FP8, tag="on", name=f"ont_{b}")

    def emit_attn_half(b, n2):
        vt, et, ont = vts[b], ets[b], onts[b]
        tq = slice(n2 * 512, (n2 + 1) * 512)
        pc = ps_sm.tile([128, 512], F32, tag="acc", name=f"pc{b}_{n2}")
        for j in range(NTK // 2):
            nc.tensor.matmul(
                pc[:], ones_sb[:], et[:, 2 * j:2 * j + 2, tq],
                start=(j == 0), stop=(j == NTK // 2 - 1),
                perf_mode=DR,
            )
        rec = scr_pool.tile([128, 512], F32, tag="rec", name=f"rec{b}_{n2}")
        nc.vector.reciprocal_approx_fast(out=rec[:], in_=pc[:])
        rb = rec[:]
        rec2 = bass.AP(
            tensor=rb.tensor, offset=rb.offset,
            ap=[rb.ap[0], [0, 2], rb.ap[1]],
        )
        for mp in range(NCH // 2):
            po2 = ps_big.tile([128, T], F32, tag="big", name=f"po{b}_{n2}_{mp}")
            for j in range(NTK // 2):
                for mi in range(2):
                    m = 2 * mp + mi
                    nc.tensor.matmul(
                        po2[:, mi * 512:(mi + 1) * 512],
                        vt[:, 2 * j:2 * j + 2, m * 128:(m + 1) * 128],
                        et[:, 2 * j:2 * j + 2, tq],
                        start=(j == 0), stop=(j == NTK // 2 - 1),
                        perf_mode=DR,
                    )
            nc.vector.tensor_mul(ont[:, 2 * mp:2 * mp + 2, tq], po2[:], rec2)

    def emit_attn(b):
        emit_attn_s(b)
        emit_attn_half(b, 0)
        emit_attn_half(b, 1)

    out_ts = {}

    def emit_proj_half(b, n2, pool):
        ont, xt = onts[b], xts[b]
        if b not in out_ts:
            out_ts[b] = out_pool.tile([128, NCH, T], F32, tag="out",
                                      name=f"out_{b}")
        out_t = out_ts[b]
        tq = slice(n2 * 512, (n2 + 1) * 512)
        for m in range(NCH):
            ps = pool.tile([128, 512], F32,
                           tag="big" if pool is ps_big else "acc",
                           name=f"ps_p{b}_{m}_{n2}")
            for j in range(NCH // 2):
                nc.tensor.matmul(
                    ps[:],
                    wsb["wpT"][j][:, :, m * 128:(m + 1) * 128],
                    ont[:, 2 * j:2 * j + 2, tq],
                    start=(j == 0), stop=(j == NCH // 2 - 1),
                    perf_mode=DR,
                )
            nc.vector.scalar_tensor_tensor(
                out_t[:, m, tq], ps[:], ds_sb[:, 3:4], xt[:, m, tq],
                op0=mybir.AluOpType.mult, op1=mybir.AluOpType.add,
            )
            if o_bias:
                nc.vector.tensor_scalar_add(
                    out_t[:, m, tq], out_t[:, m, tq], bo_sb[:, m:m + 1]
                )
            nc.sync.dma_start(out=y_ap[b][:, m, tq], in_=out_t[:, m, tq])

    def emit_proj(b):
        ont, xt = onts[b], xts[b]
        out_t = out_pool.tile([128, NCH, T], F32, tag="out", name=f"out_{b}")
        out_ts[b] = out_t
        for m in range(NCH):
            for n2 in range(2):
                tq = slice(n2 * 512, (n2 + 1) * 512)
                ps = ps_sm.tile([128, 512], F32, tag="acc", name=f"ps_p{b}_{m}_{n2}")
                for j in range(NCH // 2):
                    nc.tensor.matmul(
                        ps[:],
                        wsb["wpT"][j][:, :, m * 128:(m + 1) * 128],
                        ont[:, 2 * j:2 * j + 2, tq],
                        start=(j == 0), stop=(j == NCH // 2 - 1),
                        perf_mode=DR,
                    )
                nc.vector.scalar_tensor_tensor(
                    out_t[:, m, tq], ps[:], ds_sb[:, 3:4], xt[:, m, tq],
                    op0=mybir.AluOpType.mult, op1=mybir.AluOpType.add,
                )
                if o_bias:
                    nc.vector.tensor_scalar_add(
                        out_t[:, m, tq], out_t[:, m, tq], bo_sb[:, m:m + 1]
                    )
            nc.sync.dma_start(out=y_ap[b][:, m, :], in_=out_t[:, m, :])

    emit_gn(0)
    emit_h(0)
    for b in range(BPC):
        emit_qkv(b)
        if b + 1 < BPC:
            load_x(b + 1)
            emit_gn(b + 1)
        if b >= 1:
            emit_proj(b - 1)
        if b == BPC - 1:
            emit_attn_s(b)
            emit_attn_half(b, 0)
            emit_proj_half(b, 0, ps_big)
            emit_attn_half(b, 1)
            emit_proj_half(b, 1, ps_big)
        else:
            emit_attn(b)
        if b + 1 < BPC:
            emit_h(b + 1)


def _build(flags):
    from contextlib import ExitStack

    nc = bacc.Bacc(
        "TRN2",
        target_bir_lowering=False,
        debug=False,
        enable_asserts=False,
        num_devices=NCORES,
    )
    with tile.TileContext(nc) as tc:
        with ExitStack() as ctx:
            _emit(nc, tc, ctx, flags)
    nc.compile()
    return nc


_CACHE = {}


def _consts():
    p = np.arange(128)
    maskg = (p[:, None] // 16 == np.arange(GPC)[None, :]).astype(np.float32)
    bcp = maskg.T.copy()
    return {
        "ones256": np.ones((128, 2, 128), mybir.dt.np(FP8)),
        "mask_g": maskg,
        "bc_pos": np.ascontiguousarray(bcp),
        "bc_neg": np.ascontiguousarray(-bcp),
    }


def kernel(x, gn_w, gn_b, wq, bq, wk, bk, wv, bv, wp, bp):
    x = np.ascontiguousarray(np.asarray(x, dtype=np.float32))
    B = x.shape[0]
    assert (B, x.shape[1], x.shape[2] * x.shape[3]) == (NCORES * BPC, C, T)
    H, W = x.shape[2], x.shape[3]
    xr = x.reshape(B, C, T)
    gn_w = np.asarray(gn_w, np.float32)
    gn_b = np.asarray(gn_b, np.float32)
    trivial_gn = bool(np.all(gn_w == 1.0) and np.all(gn_b == 0.0))
    qk_bias = bool(np.any(np.asarray(bq)) or np.any(np.asarray(bk)))
    o_bias = bool(np.any(np.asarray(bv)) or np.any(np.asarray(bp)))
    flags = (trivial_gn, qk_bias, o_bias)
    if flags not in _CACHE:
        _CACHE[flags] = _build(flags)
    nc = _CACHE[flags]

    common = dict(_consts())
    descale = np.empty(4, np.float32)
    fp8 = mybir.dt.np(FP8)
    for i, (n, w) in enumerate(
        (("wqT", wq), ("wkT", wk), ("wvT", wv), ("wpT", wp))
    ):
        wT = np.ascontiguousarray(np.asarray(w, np.float32).T)
        amax = float(np.abs(wT).max()) or 1.0
        k = int(np.floor(np.log2(88.0 / amax)))
        descale[i] = 2.0 ** (-k)
        ws = (wT * (2.0 ** k)).astype(fp8)
        common[n] = np.ascontiguousarray(
            ws.reshape(NCH // 2, 2, 128, C).transpose(0, 2, 1, 3)
        )
    common["descale"] = np.ascontiguousarray(np.broadcast_to(descale, (128, 4)))
    if not trivial_gn:
        common["gnw"] = gn_w
        common["gnb"] = gn_b
    if qk_bias:
        common["bq"] = np.asarray(bq, np.float32)
        common["bk"] = np.asarray(bk, np.float32)
    if o_bias:
        common["bias_o"] = (
            np.asarray(wp, np.float32) @ np.asarray(bv, np.float32)
            + np.asarray(bp, np.float32)
        ).astype(np.float32)

    in_maps = [
        {"x": np.ascontiguousarray(xr[c * BPC:(c + 1) * BPC]), **common}
        for c in range(NCORES)
    ]
    res = run_bass_kernel_spmd(nc, in_maps, core_ids=list(range(NCORES)))
    y = np.concatenate([res.results[c]["y"] for c in range(NCORES)], axis=0)
    return np.ascontiguousarray(y.reshape(B, C, H, W).astype(np.float32))
